# revision 13
# baseline (speedup 1.0000x reference)
"""Trainium2 Bass kernel for nn_Attention_56822417326562 (dense transformer block).

Sharding: data-parallel over batch — core i computes batch element i entirely
(B=8 over 8 NeuronCores, no collectives).

Per-core math (x: [512, 1600]):
  BN folded into weights on host; softmax scale (and the Schraudolph exp
  constant A=1024/ln2) folded into q. All inputs are DMAed as channel-grouped
  [128, 4*m] tensors on the sync queue, critical tensors first.

  Stage A (PE): q, k via 1x1 convs (bf16), v in f16; biases folded into the
  PSUM drains (split between ScalarE and DVE). v is staged zero-padded for
  the depthwise conv by GPSIMD, and transposed per m-tile by PE (identity
  matmul) into vT (f16) with a ones column per head for the softmax
  denominator. pe = depthwise 3x3 as 9 diagonal f16 matmuls per channel
  group, drained by ScalarE, all before the attention phase.

  Attention (per half of n, per head PAIR): the two heads' score matmuls
  S^T[m,n]*A run CONCURRENTLY in different 32-row PE tile positions into
  different PSUM banks. exp is split by column between ScalarE (true Exp with
  scale=1/A) and DVE (Schraudolph: one tensor_scalar add+max op writing int16
  bits that ARE fp16 exp values — softmax's ratio structure cancels the ~3%
  multiplicative error). out_un[d,n] and s[n] accumulate on PE via the vT
  ones column with a one-j lag behind exp. PSUM is exactly 8 banks: 4 tags
  (sca/scb = score tiles, mma/mmb = accumulators); every other phase's PSUM
  use rotates through the same tags.

  Assembly runs inline per pair-unit: mm drained once as [65,800] (zss), s
  row copied into s_g, reciprocal per 64-row block, 1/s broadcast across
  partitions by GPSIMD partition_broadcast, z = zs * (1/s) and z += pe on
  GPSIMD. proj jobs (4 c-accumulated 1x1-conv matmuls each) fill the pair
  boundaries of the following half; proj of half 1 is the tail.

HAM note: the PE queue is kept dense (stage A -> transposes -> pe -> packed
attention with no dummy jobs); warmup matmuls cover the input-DMA window.
"""
import sys

sys.path.insert(0, "/opt/trn_rl_repo")

import numpy as np

DIM = 512
NH = 8
HD = 64
KD = 32
NPOS = 1600
EPS = 1e-5
SCALE = float(KD) ** -0.5
NMT = 13  # position tiles: 12*128 + 64
HALF = 800
A16 = 1024.0 / float(np.log(2.0))  # Schraudolph scale, folded into Wq
B16 = 15300.5  # Schraudolph offset (tuned; trunc/round differences absorbed)

_compiled_nc = None


def build_nc(dump=False, warmup=64, nbj=3):
    import concourse.tile as tile
    from concourse import bacc, mybir

    f32 = mybir.dt.float32
    f16 = mybir.dt.float16
    bf16 = mybir.dt.bfloat16
    i16 = mybir.dt.int16
    AF = mybir.ActivationFunctionType
    OP = mybir.AluOpType

    nc = bacc.Bacc("TRN2", target_bir_lowering=False, debug=False, num_devices=8)

    x16_d = nc.dram_tensor("x16", [128, 4 * NPOS], bf16, kind="ExternalInput").ap()
    wq16_d = nc.dram_tensor("wq16", [128, 4 * 256], bf16, kind="ExternalInput").ap()
    wk16_d = nc.dram_tensor("wk16", [128, 4 * 256], bf16, kind="ExternalInput").ap()
    wv16_d = nc.dram_tensor("wv16", [128, 4 * DIM], f16, kind="ExternalInput").ap()
    wp16_d = nc.dram_tensor("wp16", [128, 4 * DIM], f16, kind="ExternalInput").ap()
    bq_d = nc.dram_tensor("bq", [128, 2], f32, kind="ExternalInput").ap()
    bk_d = nc.dram_tensor("bk", [128, 2], f32, kind="ExternalInput").ap()
    bv_d = nc.dram_tensor("bv", [128, 4], f32, kind="ExternalInput").ap()
    bp_d = nc.dram_tensor("bp", [128, 4], f32, kind="ExternalInput").ap()
    bpe_d = nc.dram_tensor("bpe", [128, 4], f32, kind="ExternalInput").ap()
    ident_d = nc.dram_tensor("ident", [128, 128], f16, kind="ExternalInput").ap()
    pdg_d = nc.dram_tensor("pdg", [128, 36 * 128], f16, kind="ExternalInput").ap()
    y_d = nc.dram_tensor("y", [DIM, NPOS], bf16, kind="ExternalOutput").ap()

    def mt_sz(j):
        return 64 if j == NMT - 1 else 128

    with tile.TileContext(nc) as tc:
        with (
            tc.tile_pool(name="pers", bufs=1) as pers,
            tc.tile_pool(name="pp", bufs=1, space="PSUM") as pp,
            tc.tile_pool(name="ep", bufs=6) as ep,
            tc.tile_pool(name="ystg", bufs=4) as ystg,
        ):
            # PSUM helper: rotating general-purpose tags during non-attention
            # phases (each tag slot is sized 2 banks by the score/mm tiles).
            _rot = [0]

            def gp_tile(shape, dtype, name):
                tag = ("g0", "g1")[_rot[0] % 2]
                _rot[0] += 1
                return pp.tile(shape, dtype, name=name, tag=tag)

            x16_all = pers.tile([128, 4 * NPOS], bf16, name="x16_all")
            wq_all = pers.tile([128, 4 * 256], bf16, name="wq_all")
            wk_all = pers.tile([128, 4 * 256], bf16, name="wk_all")
            wv_all = pers.tile([128, 4 * DIM], f16, name="wv_all")
            wp_all = pers.tile([128, 4 * DIM], f16, name="wp_all")
            x16_sb = [x16_all[:, NPOS * c : NPOS * (c + 1)] for c in range(4)]
            wq_sb = [wq_all[:, 256 * c : 256 * (c + 1)] for c in range(4)]
            wk_sb = [wk_all[:, 256 * c : 256 * (c + 1)] for c in range(4)]
            wv_sb = [wv_all[:, DIM * c : DIM * (c + 1)] for c in range(4)]
            wp_sb = [wp_all[:, DIM * c : DIM * (c + 1)] for c in range(4)]
            bq_sb = pers.tile([128, 2], f32, name="bq_sb")
            bk_sb = pers.tile([128, 2], f32, name="bk_sb")
            bv_sb = pers.tile([128, 4], f32, name="bv_sb")
            bp_sb = pers.tile([128, 4], f32, name="bp_sb")
            bpe_sb = pers.tile([128, 4], f32, name="bpe_sb")
            ident_sb = pers.tile([128, 128], f16, name="ident_sb")
            q_hi = [pers.tile([128, NPOS], bf16, name=f"qhi{t}") for t in range(2)]
            k_sb = [pers.tile([128, NPOS], bf16, name=f"k{t}") for t in range(2)]
            v_sb = [pers.tile([128, NPOS], f16, name=f"v{o}") for o in range(4)]
            vT_sb = [pers.tile([128, NH * 65], f16, name=f"vT{j}") for j in range(NMT)]
            pe_sb = [pers.tile([128, NPOS], f16, name=f"pe{t}") for t in range(4)]
            vpad = [pers.tile([128, 42 * 42], f16, name=f"vpad{t}") for t in range(4)]
            pdg_all = pers.tile([128, 36 * 128], f16, name="pdg_all")
            pdg_sb = [pdg_all[:, 128 * i : 128 * (i + 1)] for i in range(36)]
            z16 = [pers.tile([128, NPOS], f16, name=f"z16{t}") for t in range(4)]
            zss = [pers.tile([65, HALF], f16, name=f"zss{g}") for g in range(16)]
            zso = [pers.tile([64, HALF], f16, name=f"zso{u}") for u in range(8)]
            s_g = [pers.tile([128, NPOS], f32, name=f"s_g{i}") for i in range(2)]
            rrow = pers.tile([1, 8 * HALF], f16, name="rrow")

            # ---- input DMAs ----
            nc.sync.dma_start(ident_sb[:], ident_d[:])
            nc.sync.dma_start(x16_all[:], x16_d[:])
            nc.sync.dma_start(wq_all[:], wq16_d[:])
            nc.sync.dma_start(wk_all[:], wk16_d[:])
            nc.sync.dma_start(wv_all[:], wv16_d[:])
            nc.scalar.dma_start(bq_sb[:], bq_d[:])
            nc.scalar.dma_start(bk_sb[:], bk_d[:])
            nc.scalar.dma_start(bv_sb[:], bv_d[:])
            nc.sync.dma_start(bpe_sb[:], bpe_d[:])
            nc.sync.dma_start(wp_all[:], wp16_d[:])
            nc.sync.dma_start(bp_sb[:], bp_d[:])
            nc.sync.dma_start(pdg_all[:], pdg_d[:])

            for i in range(2):
                nc.gpsimd.memset(s_g[i][:], 1.0)
            for t in range(4):
                vg = vpad[t].rearrange("p (a b) -> p a b", a=42)
                nc.gpsimd.memset(vg[:, 0:1, :], 0.0)
                nc.gpsimd.memset(vg[:, 41:42, :], 0.0)
                nc.gpsimd.memset(vg[:, 1:41, 0:1], 0.0)
                nc.gpsimd.memset(vg[:, 1:41, 41:42], 0.0)
            vT_g = [vT_sb[j].rearrange("p (h g) -> p h g", g=65) for j in range(NMT)]
            for j in range(NMT):
                nc.gpsimd.memset(vT_g[j][0 : mt_sz(j), :, 64:65], 1.0)

            # ---- HAM warm-up over the input-DMA window ----
            if warmup:
                wps = pp.tile([128, 512], f32, name="wup", tag="g0")
                for i in range(warmup):
                    nc.tensor.matmul(
                        wps[:, 0:128],
                        ident_sb[:],
                        ident_sb[:],
                        start=(i == 0),
                        stop=(i == warmup - 1),
                    )

            # preload the exp activation table set during the DMA window
            escr = pers.tile([1, 8], f32, name="escr")
            nc.scalar.activation(escr[0:1, :], ident_sb[0:1, 0:8], AF.Exp)

            # ---- stage A: q, k (bf16); drains alternate ScalarE/DVE ----
            for w_sb, b_sb, dst in ((wq_sb, bq_sb, q_hi), (wk_sb, bk_sb, k_sb)):
                for t in range(2):
                    for ch in range(4):
                        cs = slice(400 * ch, 400 * (ch + 1))
                        ps = gp_tile([128, 512], f32, name="psqk")
                        for c in range(4):
                            nc.tensor.matmul(
                                ps[:, 0:400],
                                w_sb[c][:, 128 * t : 128 * (t + 1)],
                                x16_sb[c][:, cs],
                                start=(c == 0),
                                stop=(c == 3),
                            )
                        if ch % 2 == 0:
                            nc.scalar.activation(
                                dst[t][:, cs], ps[:, 0:400], AF.Identity,
                                bias=b_sb[:, t : t + 1],
                            )
                        else:
                            nc.vector.tensor_scalar_add(
                                dst[t][:, cs], ps[:, 0:400], b_sb[:, t : t + 1]
                            )

            # ---- stage A: v (f16) + vpad staging; then vT transposes ----
            for o in range(4):
                for ch in range(4):
                    cs = slice(400 * ch, 400 * (ch + 1))
                    ps = gp_tile([128, 512], f32, name="psv")
                    for c in range(4):
                        nc.tensor.matmul(
                            ps[:, 0:400],
                            wv_sb[c][:, 128 * o : 128 * (o + 1)],
                            x16_sb[c][:, cs],
                            start=(c == 0),
                            stop=(c == 3),
                        )
                    nc.vector.tensor_scalar_add(
                        v_sb[o][:, cs], ps[:, 0:400], bv_sb[:, o : o + 1]
                    )
                    nc.gpsimd.tensor_copy(
                        vpad[o].rearrange("p (a b) -> p a b", a=42)[
                            :, 1 + 10 * ch : 11 + 10 * ch, 1:41
                        ],
                        v_sb[o][:, cs].rearrange("p (a b) -> p a b", a=10),
                    )

            for j in range(NMT):
                mj = mt_sz(j)
                psT = gp_tile([128, 512], f16, name="psT")
                for t in range(4):
                    nc.tensor.transpose(
                        psT[0:mj, 128 * t : 128 * (t + 1)],
                        v_sb[t][:, 128 * j : 128 * j + mj],
                        ident_sb[:],
                    )
                nc.vector.tensor_copy(
                    vT_g[j][0:mj, :, 0:64],
                    psT[0:mj, :].rearrange("p (h d) -> p h d", d=64),
                )

            # ---- pe: depthwise 3x3 as PE diagonal f16 matmuls, drained by
            # ScalarE; run as slot-filler jobs during half-0 attention ----
            def make_pejob(t, ch):
                def pejob():
                    vg = vpad[t].rearrange("p (a b) -> p a b", a=42)
                    ps = gp_tile([128, 512], f32, name="pspe")
                    for k9 in range(9):
                        dy, dx = k9 // 3 - 1, k9 % 3 - 1
                        rhs = vg[
                            :, 1 + 10 * ch + dy : 11 + 10 * ch + dy, 1 + dx : 41 + dx
                        ]
                        nc.tensor.matmul(
                            ps[:, 0:400],
                            pdg_sb[9 * t + k9][:],
                            rhs,
                            start=(k9 == 0),
                            stop=(k9 == 8),
                        )
                    nc.scalar.activation(
                        pe_sb[t][:, 400 * ch : 400 * (ch + 1)],
                        ps[:, 0:400],
                        AF.Identity,
                        bias=bpe_sb[:, t : t + 1],
                    )

                return pejob

            pe_jobs = [make_pejob(t, ch) for t in range(4) for ch in range(4)]

            # ---- proj job maker: drains alternate ScalarE/DVE ----
            def make_pjob(half, o, ch):
                def pjob():
                    cs = slice(HALF * half + 400 * ch, HALF * half + 400 * (ch + 1))
                    pj = gp_tile([128, 512], f32, name="pj")
                    for c in range(4):
                        nc.tensor.matmul(
                            pj[:, 0:400],
                            wp_sb[c][:, 128 * o : 128 * (o + 1)],
                            z16[c][:, cs],
                            start=(c == 0),
                            stop=(c == 3),
                        )
                    yt = ystg.tile([128, 400], bf16, name="yt", tag="yt")
                    if (o + ch) % 2 == 0:
                        nc.scalar.activation(
                            yt[:], pj[:, 0:400], AF.Identity,
                            bias=bp_sb[:, o : o + 1],
                        )
                    else:
                        nc.vector.tensor_scalar_add(
                            yt[:], pj[:, 0:400], bp_sb[:, o : o + 1]
                        )
                    nc.sync.dma_start(y_d[128 * o : 128 * (o + 1), cs], yt[:])

                return pjob

            # ---- per-pair assembly (DVE + GPSIMD only, issued inline) ----
            def recip_group(half, t):
                # full-128-partition reciprocal (sub-tile/base-offset recip
                # miscomputes on HW), then stage each head's row at partition 0
                # (partition_broadcast only reads partition-0-based APs right)
                hs = slice(HALF * half, HALF * (half + 1))
                nc.vector.reciprocal_approx_fast(s_g[t][:, hs], s_g[t][:, hs])
                for q4 in range(4):
                    nc.vector.tensor_copy(
                        rrow[0:1, HALF * (4 * t + q4) : HALF * (4 * t + q4 + 1)],
                        s_g[t][32 * q4 : 32 * q4 + 1, hs],
                    )

            def assembly_tch(half, tch):
                # z = zs * (1/s) for the two heads of channel group tch, + pe.
                # Broadcast outputs and all TT inputs sit at base partition 0
                # (bcast@base!=0 is broken on HW; the verifier requires SBUF
                # inputs to share a start partition — the output may shift).
                hs = slice(HALF * half, HALF * (half + 1))
                t, p = tch // 2, tch % 2
                u = 4 * half + tch
                g = 8 * half + 2 * tch
                rb2 = []
                for i in range(2):
                    rc = 4 * t + 2 * p + i
                    rb = ep.tile([64, HALF], f16, name="rb", tag="rbc", bufs=4)
                    nc.gpsimd.partition_broadcast(
                        rb[0:64, :], rrow[0:1, HALF * rc : HALF * (rc + 1)]
                    )
                    rb2.append(rb)
                zin = (zss[g][0:64, :], zso[u][0:64, :])
                for i in range(2):
                    nc.vector.tensor_tensor(
                        z16[tch][64 * i : 64 * (i + 1), hs],
                        zin[i],
                        rb2[i][0:64, :],
                        op=OP.mult,
                    )
                nc.gpsimd.tensor_tensor(
                    z16[tch][:, hs], z16[tch][:, hs], pe_sb[tch][:, hs], op=OP.add
                )

            def make_asm_jobs(half):
                jobs = []
                for t in range(2):
                    jobs.append(lambda t=t: recip_group(half, t))
                for tch in range(4):
                    jobs.append(lambda tch=tch: assembly_tch(half, tch))
                return jobs

            # ---- attention: per (half, head): scores double-buffered (2-j
            # exp lag), exp column-split ScalarE|DVE, out_un with 1-j lag.
            # Slot jobs after each head keep the PE queue dense (HAM warm):
            # half 0 slots run the pe jobs; half 1 slots run half-0 proj;
            # the tail runs half-1 assembly + proj. ----
            pe_jobs_q = list(pe_jobs)
            pending = []
            for half in range(2):
                hs = slice(HALF * half, HALF * (half + 1))
                c0 = slice(HALF * half, HALF * half + 512)
                c1 = slice(HALF * half + 512, HALF * half + 800)
                for h in range(NH):
                    t, p = h // 4, (h % 4) // 2
                    sr = 32 * (h % 4)
                    g = 8 * half + h
                    u = 4 * half + 2 * t + p
                    odd = h % 2
                    mm = pp.tile([65, HALF], f32, name="mm", tag="mm")

                    def mm3(j, Es, Ed):
                        mj = mt_sz(j)
                        lhsT = vT_g[j][0:mj, h, :]
                        nc.tensor.matmul(
                            mm[:, 0:512], lhsT, Es[0:mj, :],
                            start=(j == 0), stop=(j == NMT - 1),
                        )
                        nc.tensor.matmul(
                            mm[:, 512:800], lhsT, Ed[0:mj, :],
                            start=(j == 0), stop=(j == NMT - 1),
                        )

                    prev = None
                    for j in range(NMT):
                        mj = mt_sz(j)
                        ms = slice(128 * j, 128 * j + mj)
                        sc = pp.tile([128, HALF], f32, name="sc", tag="sc", bufs=2)
                        nc.tensor.matmul(
                            sc[0:mj, 0:512],
                            k_sb[t][sr : sr + 32, ms],
                            q_hi[t][sr : sr + 32, c0],
                            tile_position=(sr, 0),
                        )
                        nc.tensor.matmul(
                            sc[0:mj, 512:800],
                            k_sb[t][sr : sr + 32, ms],
                            q_hi[t][sr : sr + 32, c1],
                            tile_position=(sr, 0),
                        )
                        Es = ep.tile([128, 512], f16, name="Es", tag="E")
                        Ed = ep.tile([128, 288], f16, name="Ed", tag="Ed")
                        nc.scalar.activation(
                            Es[0:mj, :], sc[0:mj, 0:512],
                            AF.Exp, scale=1.0 / A16,
                        )
                        nc.vector.tensor_scalar(
                            Ed[0:mj, :].bitcast(i16),
                            sc[0:mj, 512:800],
                            B16, 0.0, op0=OP.add, op1=OP.max,
                        )
                        if prev is not None:
                            mm3(*prev)
                        prev = (j, Es, Ed)
                    mm3(*prev)
                    if not odd:
                        nc.vector.tensor_copy(zss[g][:], mm[0:65, :])
                        nc.vector.tensor_copy(
                            s_g[t][sr : sr + 1, hs], zss[g][64:65, :]
                        )
                    else:
                        nc.vector.tensor_copy(zso[u][0:64, :], mm[0:64, :])
                        nc.vector.tensor_copy(
                            s_g[t][sr : sr + 1, hs], mm[64:65, :]
                        )
                    # slot jobs: keep PE dense across the exp/drain latency
                    for _ in range(2):
                        if pending:
                            pending.pop(0)()
                        elif pe_jobs_q:
                            pe_jobs_q.pop(0)()
                # end of half: queue the engine-side assembly + PE proj jobs
                asm = make_asm_jobs(half)
                if half == 0:
                    for jf in asm:
                        jf()
                    pending = [make_pjob(0, o, ch) for o in range(4) for ch in range(2)]
                else:
                    for jf in asm:
                        jf()
                    for o in range(4):
                        for ch in range(2):
                            make_pjob(1, o, ch)()
            for jobf in pending:
                jobf()

            if dump:
                dbg_specs = [
                    ("q0", q_hi[0]),
                    ("k0", k_sb[0]),
                    ("vt0", vT_sb[0]),
                    ("pe0", pe_sb[0]),
                    ("zss0", zss[0]),
                    ("zss2", zss[2]),
                    ("zso0", zso[0]),
                    ("zso1", zso[1]),
                    ("z160", z16[0]),
                    ("z161", z16[1]),
                    ("z162", z16[2]),
                    ("z163", z16[3]),
                    ("sg0", s_g[0]),
                    ("sg1", s_g[1]),
                    ("rrow", rrow),
                ]
                for nm, t_sb in dbg_specs:
                    t_d = nc.dram_tensor(
                        f"dbg_{nm}", list(t_sb.shape), t_sb.dtype, kind="ExternalOutput"
                    ).ap()
                    nc.sync.dma_start(t_d[:], t_sb[:])

    nc.compile()
    return nc


def prep_weights(inputs):
    import ml_dtypes

    bfl = ml_dtypes.bfloat16
    d = lambda k: np.asarray(inputs[k], dtype=np.float64)
    inv = d("qkv_gamma") / np.sqrt(d("qkv_var") + EPS)
    W = d("qkv_w") * inv[:, None]
    bb = d("qkv_beta") - d("qkv_mean") * inv
    Wh = W.reshape(NH, 2 * KD + HD, DIM)
    bh = bb.reshape(NH, 2 * KD + HD)
    Wq = (Wh[:, :KD] * (SCALE * A16)).reshape(NH * KD, DIM)
    bq = (bh[:, :KD] * (SCALE * A16)).reshape(-1)
    Wk = Wh[:, KD : 2 * KD].reshape(NH * KD, DIM)
    bk = bh[:, KD : 2 * KD].reshape(-1)
    Wv = Wh[:, 2 * KD :].reshape(NH * HD, DIM)
    bv = bh[:, 2 * KD :].reshape(-1)

    ipe = d("pe_gamma") / np.sqrt(d("pe_var") + EPS)
    wpe = d("pe_w")[:, 0] * ipe[:, None, None]  # [512, 3, 3]
    bpe = d("pe_beta") - d("pe_mean") * ipe
    pdg = np.zeros((36, 128, 128), np.float64)
    ar = np.arange(128)
    for t in range(4):
        for k9 in range(9):
            pdg[t * 9 + k9, ar, ar] = wpe[128 * t : 128 * (t + 1), k9 // 3, k9 % 3]

    ip = d("proj_gamma") / np.sqrt(d("proj_var") + EPS)
    Wp = d("proj_w") * ip[:, None]
    bp = d("proj_beta") - d("proj_mean") * ip

    c32 = lambda a: np.ascontiguousarray(a, dtype=np.float32)
    c16 = lambda a: np.ascontiguousarray(a.astype(np.float32), dtype=bfl)
    ch16 = lambda a: np.ascontiguousarray(a.astype(np.float32), dtype=np.float16)

    def grp(wT):
        # [512, m] -> [128, 4*m]: row p = concat over c of wT[128c+p, :]
        m = wT.shape[1]
        return wT.reshape(4, 128, m).transpose(1, 0, 2).reshape(128, 4 * m)

    return dict(
        wq16=c16(grp(Wq.T)),
        wk16=c16(grp(Wk.T)),
        wv16=ch16(grp(Wv.T)),
        wp16=ch16(grp(Wp.T)),
        bq=c32(bq.reshape(2, 128).T),
        bk=c32(bk.reshape(2, 128).T),
        bv=c32(bv.reshape(4, 128).T),
        bp=c32(bp.reshape(4, 128).T),
        bpe=c32(bpe.reshape(4, 128).T),
        ident=ch16(np.eye(128)),
        pdg=ch16(pdg.transpose(1, 0, 2).reshape(128, 36 * 128)),
    )


def make_in_maps(inputs):
    import ml_dtypes

    w = prep_weights(inputs)
    x = np.asarray(inputs["x"], dtype=np.float32)
    B = x.shape[0]
    maps = []
    for i in range(B):
        xi = x[i].reshape(4, 128, NPOS).transpose(1, 0, 2).reshape(128, 4 * NPOS)
        maps.append({"x16": np.ascontiguousarray(xi).astype(ml_dtypes.bfloat16), **w})
    return maps


def kernel(**inputs):
    global _compiled_nc
    from concourse.bass_utils import run_bass_kernel_spmd

    if _compiled_nc is None:
        _compiled_nc = build_nc()
    in_maps = make_in_maps(inputs)
    res = run_bass_kernel_spmd(_compiled_nc, in_maps, core_ids=list(range(8)))
    y = np.stack(
        [
            np.asarray(res.results[i]["y"], dtype=np.float32).reshape(DIM, 40, 40)
            for i in range(8)
        ]
    )
    return y


if __name__ == "__main__":
    nc = build_nc()
    print("built ok")


# revision 14
# speedup vs baseline: 1.1654x; 1.1654x over previous
"""Trainium2 Bass kernel for nn_Attention_56822417326562 (dense transformer block).

Sharding: data-parallel over batch — core i computes batch element i entirely
(B=8 over 8 NeuronCores, no collectives).

Per-core math (x: [512, 1600]):
  BN folded into weights on host; softmax scale (and the Schraudolph exp
  constant A=1024/ln2) folded into q. All inputs are DMAed as channel-grouped
  [128, 4*m] tensors on the sync queue, critical tensors first.

  Stage A (PE): q, k via 1x1 convs (bf16), v in f16; biases folded into the
  PSUM drains (split between ScalarE and DVE). v is staged zero-padded for
  the depthwise conv by GPSIMD, and transposed per m-tile by PE (identity
  matmul) into vT (f16) with a ones column per head for the softmax
  denominator. pe = depthwise 3x3 as 9 diagonal f16 matmuls per channel
  group, drained by ScalarE, all before the attention phase.

  Attention (per half of n, per head PAIR): the two heads' score matmuls
  S^T[m,n]*A run CONCURRENTLY in different 32-row PE tile positions into
  different PSUM banks. exp is split by column between ScalarE (true Exp with
  scale=1/A) and DVE (Schraudolph: one tensor_scalar add+max op writing int16
  bits that ARE fp16 exp values — softmax's ratio structure cancels the ~3%
  multiplicative error). out_un[d,n] and s[n] accumulate on PE via the vT
  ones column with a one-j lag behind exp. PSUM is exactly 8 banks: 4 tags
  (sca/scb = score tiles, mma/mmb = accumulators); every other phase's PSUM
  use rotates through the same tags.

  Assembly runs inline per pair-unit: mm drained once as [65,800] (zss), s
  row copied into s_g, reciprocal per 64-row block, 1/s broadcast across
  partitions by GPSIMD partition_broadcast, z = zs * (1/s) and z += pe on
  GPSIMD. proj jobs (4 c-accumulated 1x1-conv matmuls each) fill the pair
  boundaries of the following half; proj of half 1 is the tail.

HAM note: the PE queue is kept dense (stage A -> transposes -> pe -> packed
attention with no dummy jobs); warmup matmuls cover the input-DMA window.
"""
import sys

sys.path.insert(0, "/opt/trn_rl_repo")

import numpy as np

DIM = 512
NH = 8
HD = 64
KD = 32
NPOS = 1600
EPS = 1e-5
SCALE = float(KD) ** -0.5
NMT = 13  # position tiles: 12*128 + 64
HALF = 800
A16 = 1024.0 / float(np.log(2.0))  # Schraudolph scale, folded into Wq
B16 = 15300.5  # Schraudolph offset (tuned; trunc/round differences absorbed)

_compiled_nc = None


def build_nc(dump=False, warmup=64, nbj=3):
    import concourse.tile as tile
    from concourse import bacc, mybir

    f32 = mybir.dt.float32
    f16 = mybir.dt.float16
    bf16 = mybir.dt.bfloat16
    i16 = mybir.dt.int16
    AF = mybir.ActivationFunctionType
    OP = mybir.AluOpType

    nc = bacc.Bacc("TRN2", target_bir_lowering=False, debug=False, num_devices=8)

    x16_d = nc.dram_tensor("x16", [128, 4 * NPOS], bf16, kind="ExternalInput").ap()
    wq16_d = nc.dram_tensor("wq16", [128, 4 * 256], bf16, kind="ExternalInput").ap()
    wk16_d = nc.dram_tensor("wk16", [128, 4 * 256], bf16, kind="ExternalInput").ap()
    wv16_d = nc.dram_tensor("wv16", [128, 4 * DIM], f16, kind="ExternalInput").ap()
    wp16_d = nc.dram_tensor("wp16", [128, 4 * DIM], f16, kind="ExternalInput").ap()
    bq_d = nc.dram_tensor("bq", [128, 2], f32, kind="ExternalInput").ap()
    bk_d = nc.dram_tensor("bk", [128, 2], f32, kind="ExternalInput").ap()
    bv_d = nc.dram_tensor("bv", [128, 4], f32, kind="ExternalInput").ap()
    bp_d = nc.dram_tensor("bp", [128, 4], f32, kind="ExternalInput").ap()
    bpe_d = nc.dram_tensor("bpe", [128, 4], f32, kind="ExternalInput").ap()
    ident_d = nc.dram_tensor("ident", [128, 128], f16, kind="ExternalInput").ap()
    pdg_d = nc.dram_tensor("pdg", [128, 36 * 128], f16, kind="ExternalInput").ap()
    y_d = nc.dram_tensor("y", [DIM, NPOS], bf16, kind="ExternalOutput").ap()

    def mt_sz(j):
        return 64 if j == NMT - 1 else 128

    with tile.TileContext(nc) as tc:
        with (
            tc.tile_pool(name="pers", bufs=1) as pers,
            tc.tile_pool(name="pp", bufs=1, space="PSUM") as pp,
            tc.tile_pool(name="ep", bufs=6) as ep,
            tc.tile_pool(name="ystg", bufs=4) as ystg,
        ):
            # PSUM helper: rotating general-purpose tags during non-attention
            # phases (each tag slot is sized 2 banks by the score/mm tiles).
            _rot = [0]

            def gp_tile(shape, dtype, name):
                tag = ("g0", "g1")[_rot[0] % 2]
                _rot[0] += 1
                return pp.tile(shape, dtype, name=name, tag=tag)

            x16_all = pers.tile([128, 4 * NPOS], bf16, name="x16_all")
            wq_all = pers.tile([128, 4 * 256], bf16, name="wq_all")
            wk_all = pers.tile([128, 4 * 256], bf16, name="wk_all")
            wv_all = pers.tile([128, 4 * DIM], f16, name="wv_all")
            wp_all = pers.tile([128, 4 * DIM], f16, name="wp_all")
            x16_sb = [x16_all[:, NPOS * c : NPOS * (c + 1)] for c in range(4)]
            wq_sb = [wq_all[:, 256 * c : 256 * (c + 1)] for c in range(4)]
            wk_sb = [wk_all[:, 256 * c : 256 * (c + 1)] for c in range(4)]
            wv_sb = [wv_all[:, DIM * c : DIM * (c + 1)] for c in range(4)]
            wp_sb = [wp_all[:, DIM * c : DIM * (c + 1)] for c in range(4)]
            bq_sb = pers.tile([128, 2], f32, name="bq_sb")
            bk_sb = pers.tile([128, 2], f32, name="bk_sb")
            bv_sb = pers.tile([128, 4], f32, name="bv_sb")
            bp_sb = pers.tile([128, 4], f32, name="bp_sb")
            bpe_sb = pers.tile([128, 4], f32, name="bpe_sb")
            ident_sb = pers.tile([128, 128], f16, name="ident_sb")
            q_hi = [pers.tile([128, NPOS], bf16, name=f"qhi{t}") for t in range(2)]
            k_sb = [pers.tile([128, NPOS], bf16, name=f"k{t}") for t in range(2)]
            v_sb = [pers.tile([128, NPOS], f16, name=f"v{o}") for o in range(4)]
            vT_sb = [pers.tile([128, NH * 65], f16, name=f"vT{j}") for j in range(NMT)]
            pe_sb = [pers.tile([128, NPOS], f16, name=f"pe{t}") for t in range(4)]
            vpad = [pers.tile([128, 42 * 42], f16, name=f"vpad{t}") for t in range(4)]
            pdg_all = pers.tile([128, 36 * 128], f16, name="pdg_all")
            pdg_sb = [pdg_all[:, 128 * i : 128 * (i + 1)] for i in range(36)]
            z16 = [pers.tile([128, NPOS], f16, name=f"z16{t}") for t in range(4)]
            zss = [pers.tile([65, HALF], f16, name=f"zss{g}") for g in range(16)]
            zso = [pers.tile([64, HALF], f16, name=f"zso{u}") for u in range(8)]
            s_g = [pers.tile([128, NPOS], f32, name=f"s_g{i}") for i in range(2)]
            rrow = pers.tile([1, 8 * HALF], f16, name="rrow")

            # ---- input DMAs ----
            nc.sync.dma_start(ident_sb[:], ident_d[:])
            nc.sync.dma_start(x16_all[:], x16_d[:])
            nc.sync.dma_start(wq_all[:], wq16_d[:])
            nc.sync.dma_start(wk_all[:], wk16_d[:])
            nc.sync.dma_start(wv_all[:], wv16_d[:])
            nc.scalar.dma_start(bq_sb[:], bq_d[:])
            nc.scalar.dma_start(bk_sb[:], bk_d[:])
            nc.scalar.dma_start(bv_sb[:], bv_d[:])
            nc.sync.dma_start(bpe_sb[:], bpe_d[:])
            nc.sync.dma_start(wp_all[:], wp16_d[:])
            nc.sync.dma_start(bp_sb[:], bp_d[:])
            nc.sync.dma_start(pdg_all[:], pdg_d[:])

            for i in range(2):
                nc.gpsimd.memset(s_g[i][:], 1.0)
            for t in range(4):
                vg = vpad[t].rearrange("p (a b) -> p a b", a=42)
                nc.gpsimd.memset(vg[:, 0:1, :], 0.0)
                nc.gpsimd.memset(vg[:, 41:42, :], 0.0)
                nc.gpsimd.memset(vg[:, 1:41, 0:1], 0.0)
                nc.gpsimd.memset(vg[:, 1:41, 41:42], 0.0)
            vT_g = [vT_sb[j].rearrange("p (h g) -> p h g", g=65) for j in range(NMT)]
            for j in range(NMT):
                nc.gpsimd.memset(vT_g[j][0 : mt_sz(j), :, 64:65], 1.0)

            # ---- HAM warm-up over the input-DMA window ----
            if warmup:
                wps = pp.tile([128, 512], f32, name="wup", tag="g0")
                for i in range(warmup):
                    nc.tensor.matmul(
                        wps[:, 0:128],
                        ident_sb[:],
                        ident_sb[:],
                        start=(i == 0),
                        stop=(i == warmup - 1),
                    )

            # preload the exp activation table set during the DMA window
            escr = pers.tile([1, 8], f32, name="escr")
            nc.scalar.activation(escr[0:1, :], ident_sb[0:1, 0:8], AF.Exp)

            # ---- stage A: q, k (bf16); drains alternate ScalarE/DVE ----
            for w_sb, b_sb, dst in ((wq_sb, bq_sb, q_hi), (wk_sb, bk_sb, k_sb)):
                for t in range(2):
                    for ch in range(4):
                        cs = slice(400 * ch, 400 * (ch + 1))
                        ps = gp_tile([128, 512], f32, name="psqk")
                        for c in range(4):
                            nc.tensor.matmul(
                                ps[:, 0:400],
                                w_sb[c][:, 128 * t : 128 * (t + 1)],
                                x16_sb[c][:, cs],
                                start=(c == 0),
                                stop=(c == 3),
                            )
                        if ch % 2 == 0:
                            nc.scalar.activation(
                                dst[t][:, cs], ps[:, 0:400], AF.Identity,
                                bias=b_sb[:, t : t + 1],
                            )
                        else:
                            nc.vector.tensor_scalar_add(
                                dst[t][:, cs], ps[:, 0:400], b_sb[:, t : t + 1]
                            )

            # ---- stage A: v (f16) + vpad staging; then vT transposes ----
            for o in range(4):
                for ch in range(4):
                    cs = slice(400 * ch, 400 * (ch + 1))
                    ps = gp_tile([128, 512], f32, name="psv")
                    for c in range(4):
                        nc.tensor.matmul(
                            ps[:, 0:400],
                            wv_sb[c][:, 128 * o : 128 * (o + 1)],
                            x16_sb[c][:, cs],
                            start=(c == 0),
                            stop=(c == 3),
                        )
                    nc.vector.tensor_scalar_add(
                        v_sb[o][:, cs], ps[:, 0:400], bv_sb[:, o : o + 1]
                    )
                    nc.gpsimd.tensor_copy(
                        vpad[o].rearrange("p (a b) -> p a b", a=42)[
                            :, 1 + 10 * ch : 11 + 10 * ch, 1:41
                        ],
                        v_sb[o][:, cs].rearrange("p (a b) -> p a b", a=10),
                    )

            for j in range(NMT):
                mj = mt_sz(j)
                psT = gp_tile([128, 512], f16, name="psT")
                for t in range(4):
                    nc.tensor.transpose(
                        psT[0:mj, 128 * t : 128 * (t + 1)],
                        v_sb[t][:, 128 * j : 128 * j + mj],
                        ident_sb[:],
                    )
                nc.vector.tensor_copy(
                    vT_g[j][0:mj, :, 0:64],
                    psT[0:mj, :].rearrange("p (h d) -> p h d", d=64),
                )

            # ---- pe: depthwise 3x3 as PE diagonal f16 matmuls, drained by
            # ScalarE; run as slot-filler jobs during half-0 attention ----
            def make_pejob(t, ch):
                def pejob():
                    vg = vpad[t].rearrange("p (a b) -> p a b", a=42)
                    ps = gp_tile([128, 512], f32, name="pspe")
                    for k9 in range(9):
                        dy, dx = k9 // 3 - 1, k9 % 3 - 1
                        rhs = vg[
                            :, 1 + 10 * ch + dy : 11 + 10 * ch + dy, 1 + dx : 41 + dx
                        ]
                        nc.tensor.matmul(
                            ps[:, 0:400],
                            pdg_sb[9 * t + k9][:],
                            rhs,
                            start=(k9 == 0),
                            stop=(k9 == 8),
                        )
                    nc.scalar.activation(
                        pe_sb[t][:, 400 * ch : 400 * (ch + 1)],
                        ps[:, 0:400],
                        AF.Identity,
                        bias=bpe_sb[:, t : t + 1],
                    )

                return pejob

            pe_scr = pers.tile([128, 400], f16, name="pe_scr")

            def make_dummy(nmm=9):
                def djob():
                    vg = vpad[0].rearrange("p (a b) -> p a b", a=42)
                    ps = gp_tile([128, 512], f32, name="psdm")
                    for k9 in range(nmm):
                        dy, dx = k9 % 3 - 1, k9 // 3 - 1
                        rhs = vg[:, 1 + dy : 11 + dy, 1 + dx : 41 + dx]
                        nc.tensor.matmul(
                            ps[:, 0:400],
                            pdg_sb[k9][:],
                            rhs,
                            start=(k9 == 0),
                            stop=(k9 == nmm - 1),
                        )
                    nc.vector.tensor_copy(pe_scr[:], ps[:, 0:400])

                return djob

            # pe chunks ch<2 feed half-0 assembly; ch>=2 only needed at the
            # half-1 tail -> usable as half-1 slot filler
            pe_jobs_h0 = [make_pejob(t, ch) for t in range(4) for ch in range(2)]
            pe_jobs_h1 = [make_pejob(t, ch) for t in range(4) for ch in range(2, 4)]

            # ---- proj job maker: drains alternate ScalarE/DVE ----
            def make_pjob(half, o, ch):
                def pjob():
                    cs = slice(HALF * half + 400 * ch, HALF * half + 400 * (ch + 1))
                    pj = gp_tile([128, 512], f32, name="pj")
                    for c in range(4):
                        nc.tensor.matmul(
                            pj[:, 0:400],
                            wp_sb[c][:, 128 * o : 128 * (o + 1)],
                            z16[c][:, cs],
                            start=(c == 0),
                            stop=(c == 3),
                        )
                    yt = ystg.tile([128, 400], bf16, name="yt", tag="yt")
                    if (o + ch) % 2 == 0:
                        nc.scalar.activation(
                            yt[:], pj[:, 0:400], AF.Identity,
                            bias=bp_sb[:, o : o + 1],
                        )
                    else:
                        nc.vector.tensor_scalar_add(
                            yt[:], pj[:, 0:400], bp_sb[:, o : o + 1]
                        )
                    nc.sync.dma_start(y_d[128 * o : 128 * (o + 1), cs], yt[:])

                return pjob

            # ---- per-pair assembly (DVE + GPSIMD only, issued inline) ----
            def recip_group(half, t):
                # full-128-partition reciprocal (sub-tile/base-offset recip
                # miscomputes on HW), then stage each head's row at partition 0
                # (partition_broadcast only reads partition-0-based APs right)
                hs = slice(HALF * half, HALF * (half + 1))
                nc.vector.reciprocal_approx_fast(s_g[t][:, hs], s_g[t][:, hs])
                for q4 in range(4):
                    nc.vector.tensor_copy(
                        rrow[0:1, HALF * (4 * t + q4) : HALF * (4 * t + q4 + 1)],
                        s_g[t][32 * q4 : 32 * q4 + 1, hs],
                    )

            def assembly_tch(half, tch):
                # z = zs * (1/s) for the two heads of channel group tch, + pe.
                # Broadcast outputs and all TT inputs sit at base partition 0
                # (bcast@base!=0 is broken on HW; the verifier requires SBUF
                # inputs to share a start partition — the output may shift).
                hs = slice(HALF * half, HALF * (half + 1))
                t, p = tch // 2, tch % 2
                u = 4 * half + tch
                g = 8 * half + 2 * tch
                rb2 = []
                for i in range(2):
                    rc = 4 * t + 2 * p + i
                    rb = ep.tile([64, HALF], f16, name="rb", tag="rbc", bufs=4)
                    nc.gpsimd.partition_broadcast(
                        rb[0:64, :], rrow[0:1, HALF * rc : HALF * (rc + 1)]
                    )
                    rb2.append(rb)
                zin = (zss[g][0:64, :], zso[u][0:64, :])
                for i in range(2):
                    nc.vector.tensor_tensor(
                        z16[tch][64 * i : 64 * (i + 1), hs],
                        zin[i],
                        rb2[i][0:64, :],
                        op=OP.mult,
                    )
                nc.gpsimd.tensor_tensor(
                    z16[tch][:, hs], z16[tch][:, hs], pe_sb[tch][:, hs], op=OP.add
                )

            def make_asm_jobs(half):
                jobs = []
                for t in range(2):
                    jobs.append(lambda t=t: recip_group(half, t))
                for tch in range(4):
                    jobs.append(lambda tch=tch: assembly_tch(half, tch))
                return jobs

            # ---- attention: per (half, head): scores double-buffered (2-j
            # exp lag), exp column-split ScalarE|DVE, out_un with 1-j lag.
            # Slot jobs after each head keep the PE queue dense (HAM warm):
            # half 0 slots run the pe jobs; half 1 slots run half-0 proj;
            # the tail runs half-1 assembly + proj. ----
            slotq = {0: [], 1: []}
            for i in range(8):
                slotq[0].append([pe_jobs_h0[i], make_dummy()])
            pending = []
            for half in range(2):
                hs = slice(HALF * half, HALF * (half + 1))
                c0 = slice(HALF * half, HALF * half + 512)
                c1 = slice(HALF * half + 512, HALF * half + 800)
                for h in range(NH):
                    t, p = h // 4, (h % 4) // 2
                    sr = 32 * (h % 4)
                    g = 8 * half + h
                    u = 4 * half + 2 * t + p
                    odd = h % 2
                    mm = pp.tile([65, HALF], f32, name="mm", tag="mm")

                    def mm3(j, Es, Ed):
                        mj = mt_sz(j)
                        lhsT = vT_g[j][0:mj, h, :]
                        nc.tensor.matmul(
                            mm[:, 0:512], lhsT, Es[0:mj, :],
                            start=(j == 0), stop=(j == NMT - 1),
                        )
                        nc.tensor.matmul(
                            mm[:, 512:800], lhsT, Ed[0:mj, :],
                            start=(j == 0), stop=(j == NMT - 1),
                        )

                    prev = None
                    for j in range(NMT):
                        mj = mt_sz(j)
                        ms = slice(128 * j, 128 * j + mj)
                        sc = pp.tile([128, HALF], f32, name="sc", tag="sc", bufs=2)
                        nc.tensor.matmul(
                            sc[0:mj, 0:512],
                            k_sb[t][sr : sr + 32, ms],
                            q_hi[t][sr : sr + 32, c0],
                            tile_position=(sr, 0),
                        )
                        nc.tensor.matmul(
                            sc[0:mj, 512:800],
                            k_sb[t][sr : sr + 32, ms],
                            q_hi[t][sr : sr + 32, c1],
                            tile_position=(sr, 0),
                        )
                        Es = ep.tile([128, 512], f16, name="Es", tag="E")
                        Ed = ep.tile([128, 288], f16, name="Ed", tag="Ed")
                        nc.scalar.activation(
                            Es[0:mj, :], sc[0:mj, 0:512],
                            AF.Exp, scale=1.0 / A16,
                        )
                        nc.vector.tensor_scalar(
                            Ed[0:mj, :].bitcast(i16),
                            sc[0:mj, 512:800],
                            B16, 0.0, op0=OP.add, op1=OP.max,
                        )
                        if prev is not None:
                            mm3(*prev)
                        prev = (j, Es, Ed)
                    mm3(*prev)
                    if not odd:
                        nc.vector.tensor_copy(zss[g][:], mm[0:65, :])
                        nc.vector.tensor_copy(
                            s_g[t][sr : sr + 1, hs], zss[g][64:65, :]
                        )
                    else:
                        nc.vector.tensor_copy(zso[u][0:64, :], mm[0:64, :])
                        nc.vector.tensor_copy(
                            s_g[t][sr : sr + 1, hs], mm[64:65, :]
                        )
                    # slot jobs: contiguous dep-free PE bursts after each
                    # head keep/retrigger HAM K=8/8
                    if slotq[half]:
                        for jf in slotq[half].pop(0):
                            jf()
                # end of half: engine-side assembly; fill half-1 slots with
                # the remaining pe jobs + half-0 proj
                for jf in make_asm_jobs(half):
                    jf()
                if half == 0:
                    pj0 = [make_pjob(0, o, ch) for o in range(4) for ch in range(2)]
                    for i in range(8):
                        slotq[1].append([pe_jobs_h1[i], pj0[i]])
            # tail: half-1 proj interleaved with dummy bursts
            for o in range(4):
                for ch in range(2):
                    make_pjob(1, o, ch)()
                    if o < 3:
                        make_dummy(5)()

            if dump:
                dbg_specs = [
                    ("q0", q_hi[0]),
                    ("k0", k_sb[0]),
                    ("vt0", vT_sb[0]),
                    ("pe0", pe_sb[0]),
                    ("zss0", zss[0]),
                    ("zss2", zss[2]),
                    ("zso0", zso[0]),
                    ("zso1", zso[1]),
                    ("z160", z16[0]),
                    ("z161", z16[1]),
                    ("z162", z16[2]),
                    ("z163", z16[3]),
                    ("sg0", s_g[0]),
                    ("sg1", s_g[1]),
                    ("rrow", rrow),
                ]
                for nm, t_sb in dbg_specs:
                    t_d = nc.dram_tensor(
                        f"dbg_{nm}", list(t_sb.shape), t_sb.dtype, kind="ExternalOutput"
                    ).ap()
                    nc.sync.dma_start(t_d[:], t_sb[:])

    nc.compile()
    return nc


def prep_weights(inputs):
    import ml_dtypes

    bfl = ml_dtypes.bfloat16
    d = lambda k: np.asarray(inputs[k], dtype=np.float64)
    inv = d("qkv_gamma") / np.sqrt(d("qkv_var") + EPS)
    W = d("qkv_w") * inv[:, None]
    bb = d("qkv_beta") - d("qkv_mean") * inv
    Wh = W.reshape(NH, 2 * KD + HD, DIM)
    bh = bb.reshape(NH, 2 * KD + HD)
    Wq = (Wh[:, :KD] * (SCALE * A16)).reshape(NH * KD, DIM)
    bq = (bh[:, :KD] * (SCALE * A16)).reshape(-1)
    Wk = Wh[:, KD : 2 * KD].reshape(NH * KD, DIM)
    bk = bh[:, KD : 2 * KD].reshape(-1)
    Wv = Wh[:, 2 * KD :].reshape(NH * HD, DIM)
    bv = bh[:, 2 * KD :].reshape(-1)

    ipe = d("pe_gamma") / np.sqrt(d("pe_var") + EPS)
    wpe = d("pe_w")[:, 0] * ipe[:, None, None]  # [512, 3, 3]
    bpe = d("pe_beta") - d("pe_mean") * ipe
    pdg = np.zeros((36, 128, 128), np.float64)
    ar = np.arange(128)
    for t in range(4):
        for k9 in range(9):
            pdg[t * 9 + k9, ar, ar] = wpe[128 * t : 128 * (t + 1), k9 // 3, k9 % 3]

    ip = d("proj_gamma") / np.sqrt(d("proj_var") + EPS)
    Wp = d("proj_w") * ip[:, None]
    bp = d("proj_beta") - d("proj_mean") * ip

    c32 = lambda a: np.ascontiguousarray(a, dtype=np.float32)
    c16 = lambda a: np.ascontiguousarray(a.astype(np.float32), dtype=bfl)
    ch16 = lambda a: np.ascontiguousarray(a.astype(np.float32), dtype=np.float16)

    def grp(wT):
        # [512, m] -> [128, 4*m]: row p = concat over c of wT[128c+p, :]
        m = wT.shape[1]
        return wT.reshape(4, 128, m).transpose(1, 0, 2).reshape(128, 4 * m)

    return dict(
        wq16=c16(grp(Wq.T)),
        wk16=c16(grp(Wk.T)),
        wv16=ch16(grp(Wv.T)),
        wp16=ch16(grp(Wp.T)),
        bq=c32(bq.reshape(2, 128).T),
        bk=c32(bk.reshape(2, 128).T),
        bv=c32(bv.reshape(4, 128).T),
        bp=c32(bp.reshape(4, 128).T),
        bpe=c32(bpe.reshape(4, 128).T),
        ident=ch16(np.eye(128)),
        pdg=ch16(pdg.transpose(1, 0, 2).reshape(128, 36 * 128)),
    )


def make_in_maps(inputs):
    import ml_dtypes

    w = prep_weights(inputs)
    x = np.asarray(inputs["x"], dtype=np.float32)
    B = x.shape[0]
    maps = []
    for i in range(B):
        xi = x[i].reshape(4, 128, NPOS).transpose(1, 0, 2).reshape(128, 4 * NPOS)
        maps.append({"x16": np.ascontiguousarray(xi).astype(ml_dtypes.bfloat16), **w})
    return maps


def kernel(**inputs):
    global _compiled_nc
    from concourse.bass_utils import run_bass_kernel_spmd

    if _compiled_nc is None:
        _compiled_nc = build_nc()
    in_maps = make_in_maps(inputs)
    res = run_bass_kernel_spmd(_compiled_nc, in_maps, core_ids=list(range(8)))
    y = np.stack(
        [
            np.asarray(res.results[i]["y"], dtype=np.float32).reshape(DIM, 40, 40)
            for i in range(8)
        ]
    )
    return y


if __name__ == "__main__":
    nc = build_nc()
    print("built ok")


# revision 16
# speedup vs baseline: 1.2644x; 1.0849x over previous
"""Trainium2 Bass kernel for nn_Attention_56822417326562 (dense transformer block).

Sharding: data-parallel over batch — core i computes batch element i entirely
(B=8 over 8 NeuronCores, no collectives).

Per-core math (x: [512, 1600]):
  BN folded into weights on host; softmax scale (and the Schraudolph exp
  constant A=1024/ln2) folded into q. All inputs are DMAed as channel-grouped
  [128, 4*m] tensors on the sync queue, critical tensors first.

  Stage A (PE): q, k via 1x1 convs (bf16), v in f16; biases folded into the
  PSUM drains (split between ScalarE and DVE). v is staged zero-padded for
  the depthwise conv by GPSIMD, and transposed per m-tile by PE (identity
  matmul) into vT (f16) with a ones column per head for the softmax
  denominator. pe = depthwise 3x3 as 9 diagonal f16 matmuls per channel
  group, drained by ScalarE, all before the attention phase.

  Attention (per half of n, per head PAIR): the two heads' score matmuls
  S^T[m,n]*A run CONCURRENTLY in different 32-row PE tile positions into
  different PSUM banks. exp is split by column between ScalarE (true Exp with
  scale=1/A) and DVE (Schraudolph: one tensor_scalar add+max op writing int16
  bits that ARE fp16 exp values — softmax's ratio structure cancels the ~3%
  multiplicative error). out_un[d,n] and s[n] accumulate on PE via the vT
  ones column with a one-j lag behind exp. PSUM is exactly 8 banks: 4 tags
  (sca/scb = score tiles, mma/mmb = accumulators); every other phase's PSUM
  use rotates through the same tags.

  Assembly runs inline per pair-unit: mm drained once as [65,800] (zss), s
  row copied into s_g, reciprocal per 64-row block, 1/s broadcast across
  partitions by GPSIMD partition_broadcast, z = zs * (1/s) and z += pe on
  GPSIMD. proj jobs (4 c-accumulated 1x1-conv matmuls each) fill the pair
  boundaries of the following half; proj of half 1 is the tail.

HAM note: the PE queue is kept dense (stage A -> transposes -> pe -> packed
attention with no dummy jobs); warmup matmuls cover the input-DMA window.
"""
import sys

sys.path.insert(0, "/opt/trn_rl_repo")

import numpy as np

DIM = 512
NH = 8
HD = 64
KD = 32
NPOS = 1600
EPS = 1e-5
SCALE = float(KD) ** -0.5
NMT = 13  # position tiles: 12*128 + 64
HALF = 800
A16 = 1024.0 / float(np.log(2.0))  # Schraudolph scale, folded into Wq
B16 = 15300.5  # Schraudolph offset (tuned; trunc/round differences absorbed)

_compiled_nc = None


def build_nc(dump=False, warmup=64, nbj=3):
    import concourse.tile as tile
    from concourse import bacc, mybir

    f32 = mybir.dt.float32
    f16 = mybir.dt.float16
    bf16 = mybir.dt.bfloat16
    i16 = mybir.dt.int16
    AF = mybir.ActivationFunctionType
    OP = mybir.AluOpType

    nc = bacc.Bacc("TRN2", target_bir_lowering=False, debug=False, num_devices=8)

    x16_d = nc.dram_tensor("x16", [128, 4 * NPOS], bf16, kind="ExternalInput").ap()
    wq16_d = nc.dram_tensor("wq16", [128, 4 * 256], bf16, kind="ExternalInput").ap()
    wk16_d = nc.dram_tensor("wk16", [128, 4 * 256], bf16, kind="ExternalInput").ap()
    wv16_d = nc.dram_tensor("wv16", [128, 4 * DIM], f16, kind="ExternalInput").ap()
    wp16_d = nc.dram_tensor("wp16", [128, 4 * DIM], f16, kind="ExternalInput").ap()
    bq_d = nc.dram_tensor("bq", [128, 2], f32, kind="ExternalInput").ap()
    bk_d = nc.dram_tensor("bk", [128, 2], f32, kind="ExternalInput").ap()
    bv_d = nc.dram_tensor("bv", [128, 4], f32, kind="ExternalInput").ap()
    bp_d = nc.dram_tensor("bp", [128, 4], f32, kind="ExternalInput").ap()
    bpe_d = nc.dram_tensor("bpe", [128, 4], f32, kind="ExternalInput").ap()
    ident_d = nc.dram_tensor("ident", [128, 128], f16, kind="ExternalInput").ap()
    pdg_d = nc.dram_tensor("pdg", [128, 36 * 128], f16, kind="ExternalInput").ap()
    y_d = nc.dram_tensor("y", [DIM, NPOS], bf16, kind="ExternalOutput").ap()

    def mt_sz(j):
        return 64 if j == NMT - 1 else 128

    with tile.TileContext(nc) as tc:
        with (
            tc.tile_pool(name="pers", bufs=1) as pers,
            tc.tile_pool(name="pp", bufs=1, space="PSUM") as pp,
            tc.tile_pool(name="ep", bufs=6) as ep,
            tc.tile_pool(name="ystg", bufs=4) as ystg,
        ):
            # PSUM helper: rotating general-purpose tags during non-attention
            # phases (each tag slot is sized 2 banks by the score/mm tiles).
            _rot = [0]

            def gp_tile(shape, dtype, name):
                tag = ("g0", "g1")[_rot[0] % 2]
                _rot[0] += 1
                return pp.tile(shape, dtype, name=name, tag=tag)

            x16_all = pers.tile([128, 4 * NPOS], bf16, name="x16_all")
            wq_all = pers.tile([128, 4 * 256], bf16, name="wq_all")
            wk_all = pers.tile([128, 4 * 256], bf16, name="wk_all")
            wv_all = pers.tile([128, 4 * DIM], f16, name="wv_all")
            wp_all = pers.tile([128, 4 * DIM], f16, name="wp_all")
            x16_sb = [x16_all[:, NPOS * c : NPOS * (c + 1)] for c in range(4)]
            wq_sb = [wq_all[:, 256 * c : 256 * (c + 1)] for c in range(4)]
            wk_sb = [wk_all[:, 256 * c : 256 * (c + 1)] for c in range(4)]
            wv_sb = [wv_all[:, DIM * c : DIM * (c + 1)] for c in range(4)]
            wp_sb = [wp_all[:, DIM * c : DIM * (c + 1)] for c in range(4)]
            bq_sb = pers.tile([128, 2], f32, name="bq_sb")
            bk_sb = pers.tile([128, 2], f32, name="bk_sb")
            bv_sb = pers.tile([128, 4], f32, name="bv_sb")
            bp_sb = pers.tile([128, 4], f32, name="bp_sb")
            bpe_sb = pers.tile([128, 4], f32, name="bpe_sb")
            ident_sb = pers.tile([128, 128], f16, name="ident_sb")
            q_hi = [pers.tile([128, NPOS], bf16, name=f"qhi{t}") for t in range(2)]
            k_sb = [pers.tile([128, NPOS], bf16, name=f"k{t}") for t in range(2)]
            v_sb = [pers.tile([128, NPOS], f16, name=f"v{o}") for o in range(4)]
            vT_sb = [pers.tile([128, NH * 65], f16, name=f"vT{j}") for j in range(NMT)]
            pe_sb = [pers.tile([128, NPOS], f16, name=f"pe{t}") for t in range(4)]
            vpad = [pers.tile([128, 42 * 42], f16, name=f"vpad{t}") for t in range(4)]
            pdg_all = pers.tile([128, 36 * 128], f16, name="pdg_all")
            pdg_sb = [pdg_all[:, 128 * i : 128 * (i + 1)] for i in range(36)]
            z16 = [pers.tile([128, NPOS], f16, name=f"z16{t}") for t in range(4)]
            zss = [pers.tile([65, HALF], f16, name=f"zss{g}") for g in range(16)]
            zso = [pers.tile([64, HALF], f16, name=f"zso{u}") for u in range(8)]
            s_g = [pers.tile([128, NPOS], f32, name=f"s_g{i}") for i in range(2)]
            rrow = pers.tile([1, 8 * HALF], f16, name="rrow")

            # ---- input DMAs ----
            nc.sync.dma_start(ident_sb[:], ident_d[:])
            nc.sync.dma_start(x16_all[:], x16_d[:])
            nc.sync.dma_start(wq_all[:], wq16_d[:])
            nc.sync.dma_start(wk_all[:], wk16_d[:])
            nc.sync.dma_start(wv_all[:], wv16_d[:])
            nc.scalar.dma_start(bq_sb[:], bq_d[:])
            nc.scalar.dma_start(bk_sb[:], bk_d[:])
            nc.scalar.dma_start(bv_sb[:], bv_d[:])
            nc.sync.dma_start(bpe_sb[:], bpe_d[:])
            nc.sync.dma_start(wp_all[:], wp16_d[:])
            nc.sync.dma_start(bp_sb[:], bp_d[:])
            nc.sync.dma_start(pdg_all[:], pdg_d[:])

            for i in range(2):
                nc.gpsimd.memset(s_g[i][:], 1.0)
            for t in range(4):
                vg = vpad[t].rearrange("p (a b) -> p a b", a=42)
                nc.gpsimd.memset(vg[:, 0:1, :], 0.0)
                nc.gpsimd.memset(vg[:, 41:42, :], 0.0)
                nc.gpsimd.memset(vg[:, 1:41, 0:1], 0.0)
                nc.gpsimd.memset(vg[:, 1:41, 41:42], 0.0)
            vT_g = [vT_sb[j].rearrange("p (h g) -> p h g", g=65) for j in range(NMT)]
            for j in range(NMT):
                nc.gpsimd.memset(vT_g[j][0 : mt_sz(j), :, 64:65], 1.0)

            # ---- HAM warm-up over the input-DMA window ----
            if warmup:
                wps = pp.tile([128, 512], f32, name="wup", tag="g0")
                for i in range(warmup):
                    nc.tensor.matmul(
                        wps[:, 0:128],
                        ident_sb[:],
                        ident_sb[:],
                        start=(i == 0),
                        stop=(i == warmup - 1),
                    )

            # preload the exp activation table set during the DMA window
            escr = pers.tile([1, 8], f32, name="escr")
            nc.scalar.activation(escr[0:1, :], ident_sb[0:1, 0:8], AF.Exp)

            # ---- stage A: q, k (bf16); drains alternate ScalarE/DVE ----
            for w_sb, b_sb, dst in ((wq_sb, bq_sb, q_hi), (wk_sb, bk_sb, k_sb)):
                for t in range(2):
                    for ch in range(4):
                        cs = slice(400 * ch, 400 * (ch + 1))
                        ps = gp_tile([128, 512], f32, name="psqk")
                        for c in range(4):
                            nc.tensor.matmul(
                                ps[:, 0:400],
                                w_sb[c][:, 128 * t : 128 * (t + 1)],
                                x16_sb[c][:, cs],
                                start=(c == 0),
                                stop=(c == 3),
                            )
                        if ch % 2 == 0:
                            nc.scalar.activation(
                                dst[t][:, cs], ps[:, 0:400], AF.Identity,
                                bias=b_sb[:, t : t + 1],
                            )
                        else:
                            nc.vector.tensor_scalar_add(
                                dst[t][:, cs], ps[:, 0:400], b_sb[:, t : t + 1]
                            )

            # ---- stage A: v (f16) + vpad staging; then vT transposes ----
            for o in range(4):
                for ch in range(4):
                    cs = slice(400 * ch, 400 * (ch + 1))
                    ps = gp_tile([128, 512], f32, name="psv")
                    for c in range(4):
                        nc.tensor.matmul(
                            ps[:, 0:400],
                            wv_sb[c][:, 128 * o : 128 * (o + 1)],
                            x16_sb[c][:, cs],
                            start=(c == 0),
                            stop=(c == 3),
                        )
                    nc.vector.tensor_scalar_add(
                        v_sb[o][:, cs], ps[:, 0:400], bv_sb[:, o : o + 1]
                    )
                    nc.gpsimd.tensor_copy(
                        vpad[o].rearrange("p (a b) -> p a b", a=42)[
                            :, 1 + 10 * ch : 11 + 10 * ch, 1:41
                        ],
                        v_sb[o][:, cs].rearrange("p (a b) -> p a b", a=10),
                    )

            for j in range(NMT):
                mj = mt_sz(j)
                psT = gp_tile([128, 512], f16, name="psT")
                for t in range(4):
                    nc.tensor.transpose(
                        psT[0:mj, 128 * t : 128 * (t + 1)],
                        v_sb[t][:, 128 * j : 128 * j + mj],
                        ident_sb[:],
                    )
                nc.vector.tensor_copy(
                    vT_g[j][0:mj, :, 0:64],
                    psT[0:mj, :].rearrange("p (h d) -> p h d", d=64),
                )

            # ---- pe: depthwise 3x3 as PE diagonal f16 matmuls, drained by
            # ScalarE; run as slot-filler jobs during half-0 attention ----
            def make_pejob(t, ch):
                def pejob():
                    vg = vpad[t].rearrange("p (a b) -> p a b", a=42)
                    ps = gp_tile([128, 512], f32, name="pspe")
                    for k9 in range(9):
                        dy, dx = k9 // 3 - 1, k9 % 3 - 1
                        rhs = vg[
                            :, 1 + 10 * ch + dy : 11 + 10 * ch + dy, 1 + dx : 41 + dx
                        ]
                        nc.tensor.matmul(
                            ps[:, 0:400],
                            pdg_sb[9 * t + k9][:],
                            rhs,
                            start=(k9 == 0),
                            stop=(k9 == 8),
                        )
                    nc.scalar.activation(
                        pe_sb[t][:, 400 * ch : 400 * (ch + 1)],
                        ps[:, 0:400],
                        AF.Identity,
                        bias=bpe_sb[:, t : t + 1],
                    )

                return pejob

            pe_scr = pers.tile([128, 400], f16, name="pe_scr")

            def make_dummy(nmm=9):
                def djob():
                    vg = vpad[0].rearrange("p (a b) -> p a b", a=42)
                    ps = gp_tile([128, 512], f32, name="psdm")
                    for k9 in range(nmm):
                        dy, dx = k9 % 3 - 1, k9 // 3 - 1
                        rhs = vg[:, 1 + dy : 11 + dy, 1 + dx : 41 + dx]
                        nc.tensor.matmul(
                            ps[:, 0:400],
                            pdg_sb[k9][:],
                            rhs,
                            start=(k9 == 0),
                            stop=(k9 == nmm - 1),
                        )
                    nc.vector.tensor_copy(pe_scr[:], ps[:, 0:400])

                return djob

            # pe chunks ch<2 feed half-0 assembly; ch>=2 only needed at the
            # half-1 tail -> usable as half-1 slot filler
            pe_jobs_h0 = [make_pejob(t, ch) for t in range(4) for ch in range(2)]
            pe_jobs_h1 = [make_pejob(t, ch) for t in range(4) for ch in range(2, 4)]

            # ---- proj job maker: drains alternate ScalarE/DVE ----
            def make_pjob(half, o, ch):
                def pjob():
                    cs = slice(HALF * half + 400 * ch, HALF * half + 400 * (ch + 1))
                    pj = gp_tile([128, 512], f32, name="pj")
                    for c in range(4):
                        nc.tensor.matmul(
                            pj[:, 0:400],
                            wp_sb[c][:, 128 * o : 128 * (o + 1)],
                            z16[c][:, cs],
                            start=(c == 0),
                            stop=(c == 3),
                        )
                    yt = ystg.tile([128, 400], bf16, name="yt", tag="yt")
                    if (o + ch) % 2 == 0:
                        nc.scalar.activation(
                            yt[:], pj[:, 0:400], AF.Identity,
                            bias=bp_sb[:, o : o + 1],
                        )
                    else:
                        nc.vector.tensor_scalar_add(
                            yt[:], pj[:, 0:400], bp_sb[:, o : o + 1]
                        )
                    nc.sync.dma_start(y_d[128 * o : 128 * (o + 1), cs], yt[:])

                return pjob

            # ---- per-pair assembly (DVE + GPSIMD only, issued inline) ----
            def recip_group(half, t):
                # full-128-partition reciprocal (sub-tile/base-offset recip
                # miscomputes on HW), then stage each head's row at partition 0
                # (partition_broadcast only reads partition-0-based APs right)
                hs = slice(HALF * half, HALF * (half + 1))
                nc.vector.reciprocal_approx_fast(s_g[t][:, hs], s_g[t][:, hs])
                for q4 in range(4):
                    nc.vector.tensor_copy(
                        rrow[0:1, HALF * (4 * t + q4) : HALF * (4 * t + q4 + 1)],
                        s_g[t][32 * q4 : 32 * q4 + 1, hs],
                    )

            def assembly_tch(half, tch):
                # z = zs * (1/s) for the two heads of channel group tch, + pe.
                # Broadcast outputs and all TT inputs sit at base partition 0
                # (bcast@base!=0 is broken on HW; the verifier requires SBUF
                # inputs to share a start partition — the output may shift).
                hs = slice(HALF * half, HALF * (half + 1))
                t, p = tch // 2, tch % 2
                u = 4 * half + tch
                g = 8 * half + 2 * tch
                rb2 = []
                for i in range(2):
                    rc = 4 * t + 2 * p + i
                    rb = ep.tile([64, HALF], f16, name="rb", tag="rbc", bufs=4)
                    nc.gpsimd.partition_broadcast(
                        rb[0:64, :], rrow[0:1, HALF * rc : HALF * (rc + 1)]
                    )
                    rb2.append(rb)
                zin = (zss[g][0:64, :], zso[u][0:64, :])
                for i in range(2):
                    nc.vector.tensor_tensor(
                        z16[tch][64 * i : 64 * (i + 1), hs],
                        zin[i],
                        rb2[i][0:64, :],
                        op=OP.mult,
                    )
                nc.gpsimd.tensor_tensor(
                    z16[tch][:, hs], z16[tch][:, hs], pe_sb[tch][:, hs], op=OP.add
                )

            def make_asm_jobs(half):
                jobs = []
                for t in range(2):
                    jobs.append(lambda t=t: recip_group(half, t))
                for tch in range(4):
                    jobs.append(lambda tch=tch: assembly_tch(half, tch))
                return jobs

            # ---- attention: per (half, head): scores double-buffered (2-j
            # exp lag), exp column-split ScalarE|DVE, out_un with 1-j lag.
            # Slot jobs after each head keep the PE queue dense (HAM warm):
            # half 0 slots run the pe jobs; half 1 slots run half-0 proj;
            # the tail runs half-1 assembly + proj. ----
            slotq = {0: [], 1: []}
            for i in range(8):
                slotq[0].append([pe_jobs_h0[i], make_dummy()])
            # half-1 slots are filled at the end of half 0 (below)
            for half in range(2):
                hs = slice(HALF * half, HALF * (half + 1))
                c0 = slice(HALF * half, HALF * half + 512)
                c1 = slice(HALF * half + 512, HALF * half + 800)
                for h in range(NH):
                    t, p = h // 4, (h % 4) // 2
                    sr = 32 * (h % 4)
                    g = 8 * half + h
                    u = 4 * half + 2 * t + p
                    odd = h % 2
                    mm = pp.tile([65, HALF], f32, name="mm", tag="mm")

                    def mm3(j, Es, Ed):
                        mj = mt_sz(j)
                        lhsT = vT_g[j][0:mj, h, :]
                        nc.tensor.matmul(
                            mm[:, 0:512], lhsT, Es[0:mj, :],
                            start=(j == 0), stop=(j == NMT - 1),
                        )
                        nc.tensor.matmul(
                            mm[:, 512:800], lhsT, Ed[0:mj, :],
                            start=(j == 0), stop=(j == NMT - 1),
                        )

                    prev = None
                    for j in range(NMT):
                        mj = mt_sz(j)
                        ms = slice(128 * j, 128 * j + mj)
                        sc = pp.tile([128, HALF], f32, name="sc", tag="sc", bufs=2)
                        nc.tensor.matmul(
                            sc[0:mj, 0:512],
                            k_sb[t][sr : sr + 32, ms],
                            q_hi[t][sr : sr + 32, c0],
                            tile_position=(sr, 0),
                        )
                        nc.tensor.matmul(
                            sc[0:mj, 512:800],
                            k_sb[t][sr : sr + 32, ms],
                            q_hi[t][sr : sr + 32, c1],
                            tile_position=(sr, 0),
                        )
                        Es = ep.tile([128, 512], f16, name="Es", tag="E")
                        Ed = ep.tile([128, 288], f16, name="Ed", tag="Ed")
                        nc.scalar.activation(
                            Es[0:mj, :], sc[0:mj, 0:512],
                            AF.Exp, scale=1.0 / A16,
                        )
                        nc.vector.tensor_scalar(
                            Ed[0:mj, :].bitcast(i16),
                            sc[0:mj, 512:800],
                            B16, 0.0, op0=OP.add, op1=OP.max,
                        )
                        if prev is not None:
                            mm3(*prev)
                        prev = (j, Es, Ed)
                    mm3(*prev)
                    if not odd:
                        nc.vector.tensor_copy(zss[g][:], mm[0:65, :])
                        nc.vector.tensor_copy(
                            s_g[t][sr : sr + 1, hs], zss[g][64:65, :]
                        )
                    else:
                        nc.vector.tensor_copy(zso[u][0:64, :], mm[0:64, :])
                        nc.vector.tensor_copy(
                            s_g[t][sr : sr + 1, hs], mm[64:65, :]
                        )
                    # slot jobs: contiguous dep-free PE bursts after each
                    # head keep/retrigger HAM K=8/8
                    if slotq[half]:
                        for jf in slotq[half].pop(0):
                            jf()
                if half == 0:
                    # spread half-0 assembly (engine work) one job per half-1
                    # slot so DVE/gpsimd never stall the exp pipeline; proj-0
                    # jobs strictly AFTER the last assembly job in program
                    # order (they read all four z16 groups)
                    asm0 = make_asm_jobs(0)
                    pj0 = [make_pjob(0, o, ch) for o in range(4) for ch in range(2)]
                    slotq[1] = [
                        [pe_jobs_h1[0], asm0[0], make_dummy(5)],
                        [pe_jobs_h1[1], asm0[1], make_dummy(5)],
                        [pe_jobs_h1[2], asm0[2], make_dummy(5)],
                        [pe_jobs_h1[3], asm0[3], make_dummy(5)],
                        [pe_jobs_h1[4], asm0[4], make_dummy(5)],
                        [pe_jobs_h1[5], asm0[5], pj0[0]],
                        [pe_jobs_h1[6], pj0[1], pj0[2]],
                        [pe_jobs_h1[7], pj0[3], pj0[4]],
                    ]
            # tail: leftover proj-0 overlaps half-1 assembly, dummies keep the
            # PE warm through the engine-side work, then half-1 proj
            asm1 = make_asm_jobs(1)
            tail_pe = [pj0[5], pj0[6], pj0[7]] + [make_dummy(5) for _ in range(3)]
            for i, jf in enumerate(asm1):
                jf()
                tail_pe[i]()
            for o in range(4):
                for ch in range(2):
                    make_pjob(1, o, ch)()

            if dump:
                dbg_specs = [
                    ("q0", q_hi[0]),
                    ("k0", k_sb[0]),
                    ("vt0", vT_sb[0]),
                    ("pe0", pe_sb[0]),
                    ("zss0", zss[0]),
                    ("zss2", zss[2]),
                    ("zso0", zso[0]),
                    ("zso1", zso[1]),
                    ("z160", z16[0]),
                    ("z161", z16[1]),
                    ("z162", z16[2]),
                    ("z163", z16[3]),
                    ("sg0", s_g[0]),
                    ("sg1", s_g[1]),
                    ("rrow", rrow),
                ]
                for nm, t_sb in dbg_specs:
                    t_d = nc.dram_tensor(
                        f"dbg_{nm}", list(t_sb.shape), t_sb.dtype, kind="ExternalOutput"
                    ).ap()
                    nc.sync.dma_start(t_d[:], t_sb[:])

    nc.compile()
    return nc


def prep_weights(inputs):
    import ml_dtypes

    bfl = ml_dtypes.bfloat16
    d = lambda k: np.asarray(inputs[k], dtype=np.float64)
    inv = d("qkv_gamma") / np.sqrt(d("qkv_var") + EPS)
    W = d("qkv_w") * inv[:, None]
    bb = d("qkv_beta") - d("qkv_mean") * inv
    Wh = W.reshape(NH, 2 * KD + HD, DIM)
    bh = bb.reshape(NH, 2 * KD + HD)
    Wq = (Wh[:, :KD] * (SCALE * A16)).reshape(NH * KD, DIM)
    bq = (bh[:, :KD] * (SCALE * A16)).reshape(-1)
    Wk = Wh[:, KD : 2 * KD].reshape(NH * KD, DIM)
    bk = bh[:, KD : 2 * KD].reshape(-1)
    Wv = Wh[:, 2 * KD :].reshape(NH * HD, DIM)
    bv = bh[:, 2 * KD :].reshape(-1)

    ipe = d("pe_gamma") / np.sqrt(d("pe_var") + EPS)
    wpe = d("pe_w")[:, 0] * ipe[:, None, None]  # [512, 3, 3]
    bpe = d("pe_beta") - d("pe_mean") * ipe
    pdg = np.zeros((36, 128, 128), np.float64)
    ar = np.arange(128)
    for t in range(4):
        for k9 in range(9):
            pdg[t * 9 + k9, ar, ar] = wpe[128 * t : 128 * (t + 1), k9 // 3, k9 % 3]

    ip = d("proj_gamma") / np.sqrt(d("proj_var") + EPS)
    Wp = d("proj_w") * ip[:, None]
    bp = d("proj_beta") - d("proj_mean") * ip

    c32 = lambda a: np.ascontiguousarray(a, dtype=np.float32)
    c16 = lambda a: np.ascontiguousarray(a.astype(np.float32), dtype=bfl)
    ch16 = lambda a: np.ascontiguousarray(a.astype(np.float32), dtype=np.float16)

    def grp(wT):
        # [512, m] -> [128, 4*m]: row p = concat over c of wT[128c+p, :]
        m = wT.shape[1]
        return wT.reshape(4, 128, m).transpose(1, 0, 2).reshape(128, 4 * m)

    return dict(
        wq16=c16(grp(Wq.T)),
        wk16=c16(grp(Wk.T)),
        wv16=ch16(grp(Wv.T)),
        wp16=ch16(grp(Wp.T)),
        bq=c32(bq.reshape(2, 128).T),
        bk=c32(bk.reshape(2, 128).T),
        bv=c32(bv.reshape(4, 128).T),
        bp=c32(bp.reshape(4, 128).T),
        bpe=c32(bpe.reshape(4, 128).T),
        ident=ch16(np.eye(128)),
        pdg=ch16(pdg.transpose(1, 0, 2).reshape(128, 36 * 128)),
    )


def make_in_maps(inputs):
    import ml_dtypes

    w = prep_weights(inputs)
    x = np.asarray(inputs["x"], dtype=np.float32)
    B = x.shape[0]
    maps = []
    for i in range(B):
        xi = x[i].reshape(4, 128, NPOS).transpose(1, 0, 2).reshape(128, 4 * NPOS)
        maps.append({"x16": np.ascontiguousarray(xi).astype(ml_dtypes.bfloat16), **w})
    return maps


def kernel(**inputs):
    global _compiled_nc
    from concourse.bass_utils import run_bass_kernel_spmd

    if _compiled_nc is None:
        _compiled_nc = build_nc()
    in_maps = make_in_maps(inputs)
    res = run_bass_kernel_spmd(_compiled_nc, in_maps, core_ids=list(range(8)))
    y = np.stack(
        [
            np.asarray(res.results[i]["y"], dtype=np.float32).reshape(DIM, 40, 40)
            for i in range(8)
        ]
    )
    return y


if __name__ == "__main__":
    nc = build_nc()
    print("built ok")


# revision 18
# speedup vs baseline: 1.2753x; 1.0086x over previous
"""Trainium2 Bass kernel for nn_Attention_56822417326562 (dense transformer block).

Sharding: data-parallel over batch — core i computes batch element i entirely
(B=8 over 8 NeuronCores, no collectives).

Per-core math (x: [512, 1600]):
  BN folded into weights on host; softmax scale (and the Schraudolph exp
  constant A=1024/ln2) folded into q. All inputs are DMAed as channel-grouped
  [128, 4*m] tensors on the sync queue, critical tensors first.

  Stage A (PE): q, k via 1x1 convs (bf16), v in f16; biases folded into the
  PSUM drains (split between ScalarE and DVE). v is staged zero-padded for
  the depthwise conv by GPSIMD, and transposed per m-tile by PE (identity
  matmul) into vT (f16) with a ones column per head for the softmax
  denominator. pe = depthwise 3x3 as 9 diagonal f16 matmuls per channel
  group, drained by ScalarE, all before the attention phase.

  Attention (per half of n, per head PAIR): the two heads' score matmuls
  S^T[m,n]*A run CONCURRENTLY in different 32-row PE tile positions into
  different PSUM banks. exp is split by column between ScalarE (true Exp with
  scale=1/A) and DVE (Schraudolph: one tensor_scalar add+max op writing int16
  bits that ARE fp16 exp values — softmax's ratio structure cancels the ~3%
  multiplicative error). out_un[d,n] and s[n] accumulate on PE via the vT
  ones column with a one-j lag behind exp. PSUM is exactly 8 banks: 4 tags
  (sca/scb = score tiles, mma/mmb = accumulators); every other phase's PSUM
  use rotates through the same tags.

  Assembly runs inline per pair-unit: mm drained once as [65,800] (zss), s
  row copied into s_g, reciprocal per 64-row block, 1/s broadcast across
  partitions by GPSIMD partition_broadcast, z = zs * (1/s) and z += pe on
  GPSIMD. proj jobs (4 c-accumulated 1x1-conv matmuls each) fill the pair
  boundaries of the following half; proj of half 1 is the tail.

HAM note: the PE queue is kept dense (stage A -> transposes -> pe -> packed
attention with no dummy jobs); warmup matmuls cover the input-DMA window.
"""
import sys

sys.path.insert(0, "/opt/trn_rl_repo")

import numpy as np

DIM = 512
NH = 8
HD = 64
KD = 32
NPOS = 1600
EPS = 1e-5
SCALE = float(KD) ** -0.5
NMT = 13  # position tiles: 12*128 + 64
HALF = 800
A16 = 1024.0 / float(np.log(2.0))  # Schraudolph scale, folded into Wq
B16 = 15300.5  # Schraudolph offset (tuned; trunc/round differences absorbed)

_compiled_nc = None


def build_nc(dump=False, warmup=64, nbj=3):
    import concourse.tile as tile
    from concourse import bacc, mybir

    f32 = mybir.dt.float32
    f16 = mybir.dt.float16
    bf16 = mybir.dt.bfloat16
    i16 = mybir.dt.int16
    AF = mybir.ActivationFunctionType
    OP = mybir.AluOpType

    nc = bacc.Bacc("TRN2", target_bir_lowering=False, debug=False, num_devices=8)

    x16_d = nc.dram_tensor("x16", [128, 4 * NPOS], bf16, kind="ExternalInput").ap()
    wq16_d = nc.dram_tensor("wq16", [128, 4 * 256], bf16, kind="ExternalInput").ap()
    wk16_d = nc.dram_tensor("wk16", [128, 4 * 256], bf16, kind="ExternalInput").ap()
    wv16_d = nc.dram_tensor("wv16", [128, 4 * DIM], f16, kind="ExternalInput").ap()
    wp16_d = nc.dram_tensor("wp16", [128, 4 * DIM], f16, kind="ExternalInput").ap()
    bq_d = nc.dram_tensor("bq", [128, 2], f32, kind="ExternalInput").ap()
    bk_d = nc.dram_tensor("bk", [128, 2], f32, kind="ExternalInput").ap()
    bv_d = nc.dram_tensor("bv", [128, 4], f32, kind="ExternalInput").ap()
    bp_d = nc.dram_tensor("bp", [128, 4], f32, kind="ExternalInput").ap()
    bpe_d = nc.dram_tensor("bpe", [128, 4], f32, kind="ExternalInput").ap()
    ident_d = nc.dram_tensor("ident", [128, 128], f16, kind="ExternalInput").ap()
    pdg_d = nc.dram_tensor("pdg", [128, 36 * 128], f16, kind="ExternalInput").ap()
    y_d = nc.dram_tensor("y", [DIM, NPOS], bf16, kind="ExternalOutput").ap()

    def mt_sz(j):
        return 64 if j == NMT - 1 else 128

    with tile.TileContext(nc) as tc:
        with (
            tc.tile_pool(name="pers", bufs=1) as pers,
            tc.tile_pool(name="pp", bufs=1, space="PSUM") as pp,
            tc.tile_pool(name="ep", bufs=6) as ep,
            tc.tile_pool(name="ystg", bufs=4) as ystg,
        ):
            # PSUM helper: rotating general-purpose tags during non-attention
            # phases (each tag slot is sized 2 banks by the score/mm tiles).
            _rot = [0]

            def gp_tile(shape, dtype, name):
                tag = ("g0", "g1")[_rot[0] % 2]
                _rot[0] += 1
                return pp.tile(shape, dtype, name=name, tag=tag)

            x16_all = pers.tile([128, 4 * NPOS], bf16, name="x16_all")
            wq_all = pers.tile([128, 4 * 256], bf16, name="wq_all")
            wk_all = pers.tile([128, 4 * 256], bf16, name="wk_all")
            wv_all = pers.tile([128, 4 * DIM], f16, name="wv_all")
            wp_all = pers.tile([128, 4 * DIM], f16, name="wp_all")
            x16_sb = [x16_all[:, NPOS * c : NPOS * (c + 1)] for c in range(4)]
            wq_sb = [wq_all[:, 256 * c : 256 * (c + 1)] for c in range(4)]
            wk_sb = [wk_all[:, 256 * c : 256 * (c + 1)] for c in range(4)]
            wv_sb = [wv_all[:, DIM * c : DIM * (c + 1)] for c in range(4)]
            wp_sb = [wp_all[:, DIM * c : DIM * (c + 1)] for c in range(4)]
            bq_sb = pers.tile([128, 2], f32, name="bq_sb")
            bk_sb = pers.tile([128, 2], f32, name="bk_sb")
            bv_sb = pers.tile([128, 4], f32, name="bv_sb")
            bp_sb = pers.tile([128, 4], f32, name="bp_sb")
            bpe_sb = pers.tile([128, 4], f32, name="bpe_sb")
            ident_sb = pers.tile([128, 128], f16, name="ident_sb")
            q_hi = [pers.tile([128, NPOS], bf16, name=f"qhi{t}") for t in range(2)]
            k_sb = [pers.tile([128, NPOS], bf16, name=f"k{t}") for t in range(2)]
            v_sb = [pers.tile([128, NPOS], f16, name=f"v{o}") for o in range(4)]
            vT_sb = [pers.tile([128, NH * 65], f16, name=f"vT{j}") for j in range(NMT)]
            pe_sb = [pers.tile([128, NPOS], f16, name=f"pe{t}") for t in range(4)]
            vpad = [pers.tile([128, 42 * 42], f16, name=f"vpad{t}") for t in range(4)]
            pdg_all = pers.tile([128, 36 * 128], f16, name="pdg_all")
            pdg_sb = [pdg_all[:, 128 * i : 128 * (i + 1)] for i in range(36)]
            z16 = [pers.tile([128, NPOS], f16, name=f"z16{t}") for t in range(4)]
            zss = [pers.tile([65, HALF], f16, name=f"zss{g}") for g in range(16)]
            zso = [pers.tile([64, HALF], f16, name=f"zso{u}") for u in range(8)]
            s_g = [pers.tile([128, NPOS], f32, name=f"s_g{i}") for i in range(2)]
            rrow = pers.tile([1, 8 * HALF], f16, name="rrow")

            # ---- input DMAs ----
            nc.sync.dma_start(ident_sb[:], ident_d[:])
            nc.sync.dma_start(x16_all[:], x16_d[:])
            nc.sync.dma_start(wq_all[:], wq16_d[:])
            nc.sync.dma_start(wk_all[:], wk16_d[:])
            nc.sync.dma_start(wv_all[:], wv16_d[:])
            nc.scalar.dma_start(bq_sb[:], bq_d[:])
            nc.scalar.dma_start(bk_sb[:], bk_d[:])
            nc.scalar.dma_start(bv_sb[:], bv_d[:])
            nc.sync.dma_start(bpe_sb[:], bpe_d[:])
            nc.sync.dma_start(wp_all[:], wp16_d[:])
            nc.sync.dma_start(bp_sb[:], bp_d[:])
            nc.sync.dma_start(pdg_all[:], pdg_d[:])

            for i in range(2):
                nc.gpsimd.memset(s_g[i][:], 1.0)
            for t in range(4):
                vg = vpad[t].rearrange("p (a b) -> p a b", a=42)
                nc.gpsimd.memset(vg[:, 0:1, :], 0.0)
                nc.gpsimd.memset(vg[:, 41:42, :], 0.0)
                nc.gpsimd.memset(vg[:, 1:41, 0:1], 0.0)
                nc.gpsimd.memset(vg[:, 1:41, 41:42], 0.0)
            vT_g = [vT_sb[j].rearrange("p (h g) -> p h g", g=65) for j in range(NMT)]
            for j in range(NMT):
                nc.gpsimd.memset(vT_g[j][0 : mt_sz(j), :, 64:65], 1.0)

            # ---- HAM warm-up over the input-DMA window ----
            if warmup:
                wps = pp.tile([128, 512], f32, name="wup", tag="g0")
                for i in range(warmup):
                    nc.tensor.matmul(
                        wps[:, 0:128],
                        ident_sb[:],
                        ident_sb[:],
                        start=(i == 0),
                        stop=(i == warmup - 1),
                    )

            # preload the exp activation table set during the DMA window
            escr = pers.tile([1, 8], f32, name="escr")
            nc.scalar.activation(escr[0:1, :], ident_sb[0:1, 0:8], AF.Exp)

            # ---- stage A: q, k (bf16); drains alternate ScalarE/DVE ----
            for w_sb, b_sb, dst in ((wq_sb, bq_sb, q_hi), (wk_sb, bk_sb, k_sb)):
                for t in range(2):
                    for ch in range(4):
                        cs = slice(400 * ch, 400 * (ch + 1))
                        ps = gp_tile([128, 512], f32, name="psqk")
                        for c in range(4):
                            nc.tensor.matmul(
                                ps[:, 0:400],
                                w_sb[c][:, 128 * t : 128 * (t + 1)],
                                x16_sb[c][:, cs],
                                start=(c == 0),
                                stop=(c == 3),
                            )
                        if ch % 2 == 0:
                            nc.scalar.activation(
                                dst[t][:, cs], ps[:, 0:400], AF.Identity,
                                bias=b_sb[:, t : t + 1],
                            )
                        else:
                            nc.vector.tensor_scalar_add(
                                dst[t][:, cs], ps[:, 0:400], b_sb[:, t : t + 1]
                            )

            # ---- stage A: v (f16) + vpad staging; then vT transposes ----
            for o in range(4):
                for ch in range(4):
                    cs = slice(400 * ch, 400 * (ch + 1))
                    ps = gp_tile([128, 512], f32, name="psv")
                    for c in range(4):
                        nc.tensor.matmul(
                            ps[:, 0:400],
                            wv_sb[c][:, 128 * o : 128 * (o + 1)],
                            x16_sb[c][:, cs],
                            start=(c == 0),
                            stop=(c == 3),
                        )
                    nc.vector.tensor_scalar_add(
                        v_sb[o][:, cs], ps[:, 0:400], bv_sb[:, o : o + 1]
                    )
                    nc.gpsimd.tensor_copy(
                        vpad[o].rearrange("p (a b) -> p a b", a=42)[
                            :, 1 + 10 * ch : 11 + 10 * ch, 1:41
                        ],
                        v_sb[o][:, cs].rearrange("p (a b) -> p a b", a=10),
                    )

            for j in range(NMT):
                mj = mt_sz(j)
                psT = gp_tile([128, 512], f16, name="psT")
                for t in range(4):
                    nc.tensor.transpose(
                        psT[0:mj, 128 * t : 128 * (t + 1)],
                        v_sb[t][:, 128 * j : 128 * j + mj],
                        ident_sb[:],
                    )
                nc.vector.tensor_copy(
                    vT_g[j][0:mj, :, 0:64],
                    psT[0:mj, :].rearrange("p (h d) -> p h d", d=64),
                )

            # ---- pe: depthwise 3x3 as PE diagonal f16 matmuls, drained by
            # ScalarE; run as slot-filler jobs during half-0 attention ----
            def make_pejob(t, ch):
                def pejob():
                    vg = vpad[t].rearrange("p (a b) -> p a b", a=42)
                    ps = gp_tile([128, 512], f32, name="pspe")
                    for k9 in range(9):
                        dy, dx = k9 // 3 - 1, k9 % 3 - 1
                        rhs = vg[
                            :, 1 + 10 * ch + dy : 11 + 10 * ch + dy, 1 + dx : 41 + dx
                        ]
                        nc.tensor.matmul(
                            ps[:, 0:400],
                            pdg_sb[9 * t + k9][:],
                            rhs,
                            start=(k9 == 0),
                            stop=(k9 == 8),
                        )
                    nc.scalar.activation(
                        pe_sb[t][:, 400 * ch : 400 * (ch + 1)],
                        ps[:, 0:400],
                        AF.Identity,
                        bias=bpe_sb[:, t : t + 1],
                    )

                return pejob

            pe_scr = pers.tile([128, 400], f16, name="pe_scr")

            def make_dummy(nmm=9):
                def djob():
                    vg = vpad[0].rearrange("p (a b) -> p a b", a=42)
                    ps = gp_tile([128, 512], f32, name="psdm")
                    for k9 in range(nmm):
                        dy, dx = k9 % 3 - 1, k9 // 3 - 1
                        rhs = vg[:, 1 + dy : 11 + dy, 1 + dx : 41 + dx]
                        nc.tensor.matmul(
                            ps[:, 0:400],
                            pdg_sb[k9][:],
                            rhs,
                            start=(k9 == 0),
                            stop=(k9 == nmm - 1),
                        )
                    nc.vector.tensor_copy(pe_scr[:], ps[:, 0:400])

                return djob

            # pe chunks ch<2 feed half-0 assembly; ch>=2 only needed at the
            # half-1 tail -> usable as half-1 slot filler
            pe_jobs_h0 = [make_pejob(t, ch) for t in range(4) for ch in range(2)]
            pe_jobs_h1 = [make_pejob(t, ch) for t in range(4) for ch in range(2, 4)]

            # ---- proj job maker: drains alternate ScalarE/DVE ----
            def make_pjob(half, o, ch):
                def pjob():
                    cs = slice(HALF * half + 400 * ch, HALF * half + 400 * (ch + 1))
                    pj = gp_tile([128, 512], f32, name="pj")
                    for c in range(4):
                        nc.tensor.matmul(
                            pj[:, 0:400],
                            wp_sb[c][:, 128 * o : 128 * (o + 1)],
                            z16[c][:, cs],
                            start=(c == 0),
                            stop=(c == 3),
                        )
                    yt = ystg.tile([128, 400], bf16, name="yt", tag="yt")
                    if (o + ch) % 2 == 0:
                        nc.scalar.activation(
                            yt[:], pj[:, 0:400], AF.Identity,
                            bias=bp_sb[:, o : o + 1],
                        )
                    else:
                        nc.vector.tensor_scalar_add(
                            yt[:], pj[:, 0:400], bp_sb[:, o : o + 1]
                        )
                    nc.sync.dma_start(y_d[128 * o : 128 * (o + 1), cs], yt[:])

                return pjob

            # ---- per-pair assembly (DVE + GPSIMD only, issued inline) ----
            def recip_group(half, t):
                # full-128-partition reciprocal (sub-tile/base-offset recip
                # miscomputes on HW), then stage each head's row at partition 0
                # (partition_broadcast only reads partition-0-based APs right)
                hs = slice(HALF * half, HALF * (half + 1))
                nc.vector.reciprocal_approx_fast(s_g[t][:, hs], s_g[t][:, hs])
                for q4 in range(4):
                    nc.vector.tensor_copy(
                        rrow[0:1, HALF * (4 * t + q4) : HALF * (4 * t + q4 + 1)],
                        s_g[t][32 * q4 : 32 * q4 + 1, hs],
                    )

            def assembly_tch(half, tch):
                # z = zs * (1/s) for the two heads of channel group tch, + pe.
                # Broadcast outputs and all TT inputs sit at base partition 0
                # (bcast@base!=0 is broken on HW; the verifier requires SBUF
                # inputs to share a start partition — the output may shift).
                hs = slice(HALF * half, HALF * (half + 1))
                t, p = tch // 2, tch % 2
                u = 4 * half + tch
                g = 8 * half + 2 * tch
                rb2 = []
                for i in range(2):
                    rc = 4 * t + 2 * p + i
                    rb = ep.tile([64, HALF], f16, name="rb", tag="rbc", bufs=4)
                    nc.gpsimd.partition_broadcast(
                        rb[0:64, :], rrow[0:1, HALF * rc : HALF * (rc + 1)]
                    )
                    rb2.append(rb)
                zin = (zss[g][0:64, :], zso[u][0:64, :])
                for i in range(2):
                    nc.vector.tensor_tensor(
                        z16[tch][64 * i : 64 * (i + 1), hs],
                        zin[i],
                        rb2[i][0:64, :],
                        op=OP.mult,
                    )
                nc.gpsimd.tensor_tensor(
                    z16[tch][:, hs], z16[tch][:, hs], pe_sb[tch][:, hs], op=OP.add
                )

            def make_asm_jobs(half):
                jobs = []
                for t in range(2):
                    jobs.append(lambda t=t: recip_group(half, t))
                for tch in range(4):
                    jobs.append(lambda tch=tch: assembly_tch(half, tch))
                return jobs

            # ---- attention: per (half, head): scores double-buffered (2-j
            # exp lag), exp column-split ScalarE|DVE, out_un with 1-j lag.
            # Slot jobs after each head keep the PE queue dense (HAM warm):
            # half 0 slots run the pe jobs; half 1 slots run half-0 proj;
            # the tail runs half-1 assembly + proj. ----
            slotq = {0: [], 1: []}
            for i in range(8):
                slotq[0].append([pe_jobs_h0[i], make_dummy()])
            # half-1 slots are filled at the end of half 0 (below)
            for half in range(2):
                hs = slice(HALF * half, HALF * (half + 1))
                c0 = slice(HALF * half, HALF * half + 512)
                c1 = slice(HALF * half + 512, HALF * half + 800)
                for h in range(NH):
                    t, p = h // 4, (h % 4) // 2
                    sr = 32 * (h % 4)
                    g = 8 * half + h
                    u = 4 * half + 2 * t + p
                    odd = h % 2
                    mm = pp.tile([65, HALF], f32, name="mm", tag="mm")

                    def mm3(j, Es, Ed):
                        mj = mt_sz(j)
                        lhsT = vT_g[j][0:mj, h, :]
                        nc.tensor.matmul(
                            mm[:, 0:512], lhsT, Es[0:mj, :],
                            start=(j == 0), stop=(j == NMT - 1),
                        )
                        nc.tensor.matmul(
                            mm[:, 512:800], lhsT, Ed[0:mj, :],
                            start=(j == 0), stop=(j == NMT - 1),
                        )

                    pipe = []
                    for j in range(NMT):
                        mj = mt_sz(j)
                        ms = slice(128 * j, 128 * j + mj)
                        sc = pp.tile([128, HALF], f32, name="sc", tag="sc", bufs=2)
                        nc.tensor.matmul(
                            sc[0:mj, 0:512],
                            k_sb[t][sr : sr + 32, ms],
                            q_hi[t][sr : sr + 32, c0],
                            tile_position=(sr, 0),
                        )
                        nc.tensor.matmul(
                            sc[0:mj, 512:800],
                            k_sb[t][sr : sr + 32, ms],
                            q_hi[t][sr : sr + 32, c1],
                            tile_position=(sr, 0),
                        )
                        Es = ep.tile([128, 512], f16, name="Es", tag="E")
                        Ed = ep.tile([128, 288], f16, name="Ed", tag="Ed")
                        nc.scalar.activation(
                            Es[0:mj, :], sc[0:mj, 0:512],
                            AF.Exp, scale=1.0 / A16,
                        )
                        nc.vector.tensor_scalar(
                            Ed[0:mj, :].bitcast(i16),
                            sc[0:mj, 512:800],
                            B16, 0.0, op0=OP.add, op1=OP.max,
                        )
                        pipe.append((j, Es, Ed))
                        if len(pipe) > 2:
                            mm3(*pipe.pop(0))
                    for it in pipe:
                        mm3(*it)
                    if not odd:
                        nc.vector.tensor_copy(zss[g][:], mm[0:65, :])
                        nc.vector.tensor_copy(
                            s_g[t][sr : sr + 1, hs], zss[g][64:65, :]
                        )
                    else:
                        nc.vector.tensor_copy(zso[u][0:64, :], mm[0:64, :])
                        nc.vector.tensor_copy(
                            s_g[t][sr : sr + 1, hs], mm[64:65, :]
                        )
                    # slot jobs: contiguous dep-free PE bursts after each
                    # head keep/retrigger HAM K=8/8
                    if slotq[half]:
                        for jf in slotq[half].pop(0):
                            jf()
                if half == 0:
                    # spread half-0 assembly (engine work) one job per half-1
                    # slot so DVE/gpsimd never stall the exp pipeline; proj-0
                    # jobs strictly AFTER the last assembly job in program
                    # order (they read all four z16 groups)
                    asm0 = make_asm_jobs(0)
                    pj0 = [make_pjob(0, o, ch) for o in range(4) for ch in range(2)]
                    slotq[1] = [
                        [pe_jobs_h1[0], asm0[0], make_dummy(5)],
                        [pe_jobs_h1[1], asm0[1], make_dummy(5)],
                        [pe_jobs_h1[2], asm0[2], make_dummy(5)],
                        [pe_jobs_h1[3], asm0[3], make_dummy(5)],
                        [pe_jobs_h1[4], asm0[4], make_dummy(5)],
                        [pe_jobs_h1[5], asm0[5], pj0[0]],
                        [pe_jobs_h1[6], pj0[1], pj0[2]],
                        [pe_jobs_h1[7], pj0[3], pj0[4]],
                    ]
            # tail: leftover proj-0 overlaps half-1 assembly, dummies keep the
            # PE warm through the engine-side work, then half-1 proj
            asm1 = make_asm_jobs(1)
            tail_pe = [pj0[5], pj0[6], pj0[7]] + [make_dummy(5) for _ in range(3)]
            for i, jf in enumerate(asm1):
                jf()
                tail_pe[i]()
            for o in range(4):
                for ch in range(2):
                    make_pjob(1, o, ch)()

            if dump:
                dbg_specs = [
                    ("q0", q_hi[0]),
                    ("k0", k_sb[0]),
                    ("vt0", vT_sb[0]),
                    ("pe0", pe_sb[0]),
                    ("zss0", zss[0]),
                    ("zss2", zss[2]),
                    ("zso0", zso[0]),
                    ("zso1", zso[1]),
                    ("z160", z16[0]),
                    ("z161", z16[1]),
                    ("z162", z16[2]),
                    ("z163", z16[3]),
                    ("sg0", s_g[0]),
                    ("sg1", s_g[1]),
                    ("rrow", rrow),
                ]
                for nm, t_sb in dbg_specs:
                    t_d = nc.dram_tensor(
                        f"dbg_{nm}", list(t_sb.shape), t_sb.dtype, kind="ExternalOutput"
                    ).ap()
                    nc.sync.dma_start(t_d[:], t_sb[:])

    nc.compile()
    return nc


def prep_weights(inputs):
    import ml_dtypes

    bfl = ml_dtypes.bfloat16
    d = lambda k: np.asarray(inputs[k], dtype=np.float64)
    inv = d("qkv_gamma") / np.sqrt(d("qkv_var") + EPS)
    W = d("qkv_w") * inv[:, None]
    bb = d("qkv_beta") - d("qkv_mean") * inv
    Wh = W.reshape(NH, 2 * KD + HD, DIM)
    bh = bb.reshape(NH, 2 * KD + HD)
    Wq = (Wh[:, :KD] * (SCALE * A16)).reshape(NH * KD, DIM)
    bq = (bh[:, :KD] * (SCALE * A16)).reshape(-1)
    Wk = Wh[:, KD : 2 * KD].reshape(NH * KD, DIM)
    bk = bh[:, KD : 2 * KD].reshape(-1)
    Wv = Wh[:, 2 * KD :].reshape(NH * HD, DIM)
    bv = bh[:, 2 * KD :].reshape(-1)

    ipe = d("pe_gamma") / np.sqrt(d("pe_var") + EPS)
    wpe = d("pe_w")[:, 0] * ipe[:, None, None]  # [512, 3, 3]
    bpe = d("pe_beta") - d("pe_mean") * ipe
    pdg = np.zeros((36, 128, 128), np.float64)
    ar = np.arange(128)
    for t in range(4):
        for k9 in range(9):
            pdg[t * 9 + k9, ar, ar] = wpe[128 * t : 128 * (t + 1), k9 // 3, k9 % 3]

    ip = d("proj_gamma") / np.sqrt(d("proj_var") + EPS)
    Wp = d("proj_w") * ip[:, None]
    bp = d("proj_beta") - d("proj_mean") * ip

    c32 = lambda a: np.ascontiguousarray(a, dtype=np.float32)
    c16 = lambda a: np.ascontiguousarray(a.astype(np.float32), dtype=bfl)
    ch16 = lambda a: np.ascontiguousarray(a.astype(np.float32), dtype=np.float16)

    def grp(wT):
        # [512, m] -> [128, 4*m]: row p = concat over c of wT[128c+p, :]
        m = wT.shape[1]
        return wT.reshape(4, 128, m).transpose(1, 0, 2).reshape(128, 4 * m)

    return dict(
        wq16=c16(grp(Wq.T)),
        wk16=c16(grp(Wk.T)),
        wv16=ch16(grp(Wv.T)),
        wp16=ch16(grp(Wp.T)),
        bq=c32(bq.reshape(2, 128).T),
        bk=c32(bk.reshape(2, 128).T),
        bv=c32(bv.reshape(4, 128).T),
        bp=c32(bp.reshape(4, 128).T),
        bpe=c32(bpe.reshape(4, 128).T),
        ident=ch16(np.eye(128)),
        pdg=ch16(pdg.transpose(1, 0, 2).reshape(128, 36 * 128)),
    )


def make_in_maps(inputs):
    import ml_dtypes

    w = prep_weights(inputs)
    x = np.asarray(inputs["x"], dtype=np.float32)
    B = x.shape[0]
    maps = []
    for i in range(B):
        xi = x[i].reshape(4, 128, NPOS).transpose(1, 0, 2).reshape(128, 4 * NPOS)
        maps.append({"x16": np.ascontiguousarray(xi).astype(ml_dtypes.bfloat16), **w})
    return maps


def kernel(**inputs):
    global _compiled_nc
    from concourse.bass_utils import run_bass_kernel_spmd

    if _compiled_nc is None:
        _compiled_nc = build_nc()
    in_maps = make_in_maps(inputs)
    res = run_bass_kernel_spmd(_compiled_nc, in_maps, core_ids=list(range(8)))
    y = np.stack(
        [
            np.asarray(res.results[i]["y"], dtype=np.float32).reshape(DIM, 40, 40)
            for i in range(8)
        ]
    )
    return y


if __name__ == "__main__":
    nc = build_nc()
    print("built ok")


# revision 20
# speedup vs baseline: 1.4746x; 1.1563x over previous
"""Trainium2 Bass kernel for nn_Attention_56822417326562 (dense transformer block).

Sharding: data-parallel over batch — core i computes batch element i entirely
(B=8 over 8 NeuronCores, no collectives).

Per-core math (x: [512, 1600]):
  BN folded into weights on host; softmax scale folded into q. All inputs are
  DMAed as channel-grouped [128, 4*m] tensors (one big-packet DMA each) on the
  sync queue, critical tensors first.
  Stage A (PE): q, k, v via 1x1 convs (bf16), then vT built by PE transposes
  of v (identity matmul) with a ones column appended per head for the softmax
  denominator. k is consumed via 32-row tile_position matmuls (no zero
  padding of the contraction dim), biases folded into the DVE PSUM drains.
  Attention (per half of n, per head): scores S^T[m,n] on PE (bf16), exp on
  ScalarE (its only work — the pacing engine), out_un[d,n] and s[n] in one PE
  accumulation via the vT ones column. The mm accumulator is drained by DVE
  copies (zs per head, s row into s_g); 1/s via full-tile DVE reciprocal at
  assembly time (single-partition reciprocal miscomputes on HW).
  pe = depthwise 3x3 as 9 diagonal bf16 matmuls over zero-padded v.
  Assembly (per half): 1/s broadcast via bf16 ones-matmul (tile_position row
  = head), z16 = out_un * (1/s) + pe (DVE), proj on PE, proj bias folded into
  the PSUM->SBUF drain, y DMAed out as bf16 per 400-col block.

HAM clock-gate management (the dominant perf effect): the PE clock sits at
1.2 GHz unless each free-running 3.4us activity window is ~fully busy; any
sparse window re-throttles to K=4/8 and halves PE speed for >=13.6us. The
per-head slot schedule interleaves contiguous ~1.7us PE bursts (the real
depthwise-conv jobs, the previous half's normalize/proj assembly jobs, and a
few discarded dummy bursts) between attention heads so the PE queue always
holds a multi-us backlog. Cross-engine dependencies get at least a one-head
lead so the in-order PE queue never stalls on DVE results. Separately, the
chip has a persistent P-state that can inflate all engines ~18% run to run;
only same-process paired A/B comparisons are meaningful.
"""
import sys

sys.path.insert(0, "/opt/trn_rl_repo")

import numpy as np

DIM = 512
NH = 8
HD = 64
KD = 32
NPOS = 1600
EPS = 1e-5
SCALE = float(KD) ** -0.5
NMT = 13  # position tiles: 12*128 + 64
HALF = 800

_compiled_nc = None


def build_nc(dump=False, f32c1=False, extra_d=1, extra_nmm=9, warmup=64, trim=True, gadd=True, gcp=False, syt=False, epb=4, dpre=False):
    import concourse.tile as tile
    from concourse import bacc, mybir

    f32 = mybir.dt.float32
    f32r = mybir.dt.float32r
    f16 = mybir.dt.float16
    bf16 = mybir.dt.bfloat16
    AF = mybir.ActivationFunctionType
    OP = mybir.AluOpType

    nc = bacc.Bacc("TRN2", target_bir_lowering=False, debug=False, num_devices=8)

    x16_d = nc.dram_tensor("x16", [128, 4 * NPOS], bf16, kind="ExternalInput").ap()
    wq16_d = nc.dram_tensor("wq16", [128, 4 * 256], bf16, kind="ExternalInput").ap()
    wk16_d = nc.dram_tensor("wk16", [128, 4 * 256], bf16, kind="ExternalInput").ap()
    wv16_d = nc.dram_tensor("wv16", [128, 4 * DIM], bf16, kind="ExternalInput").ap()
    wp16_d = nc.dram_tensor("wp16", [128, 4 * DIM], bf16, kind="ExternalInput").ap()
    bq_d = nc.dram_tensor("bq", [128, 2], f32, kind="ExternalInput").ap()
    bk_d = nc.dram_tensor("bk", [128, 2], f32, kind="ExternalInput").ap()
    bv_d = nc.dram_tensor("bv", [128, 4], f32, kind="ExternalInput").ap()
    bp_d = nc.dram_tensor("bp", [128, 4], f32, kind="ExternalInput").ap()
    wpe_d = nc.dram_tensor("wpe", [128, 36], f32, kind="ExternalInput").ap()
    bpe_d = nc.dram_tensor("bpe", [128, 4], f32, kind="ExternalInput").ap()
    ident_d = nc.dram_tensor("ident", [128, 128], bf16, kind="ExternalInput").ap()
    ones_d = nc.dram_tensor("ones", [128, 64], bf16, kind="ExternalInput").ap()
    pdg_d = nc.dram_tensor("pdg", [128, 36 * 128], bf16, kind="ExternalInput").ap()
    y_d = nc.dram_tensor("y", [DIM, NPOS], bf16, kind="ExternalOutput").ap()

    def mt_sz(j):
        return 64 if j == NMT - 1 else 128

    with tile.TileContext(nc) as tc:
        with (
            tc.tile_pool(name="pers", bufs=1) as pers,
            tc.tile_pool(name="ps2", bufs=2, space="PSUM") as ps2,
            tc.tile_pool(name="scp", bufs=2, space="PSUM") as scp,
            tc.tile_pool(name="mmp", bufs=1, space="PSUM") as mmp,
            tc.tile_pool(name="ep", bufs=epb) as ep,
            tc.tile_pool(name="ystg", bufs=4) as ystg,
        ):
            x16_all = pers.tile([128, 4 * NPOS], bf16, name="x16_all")
            wq_all = pers.tile([128, 4 * 256], bf16, name="wq_all")
            wk_all = pers.tile([128, 4 * 256], bf16, name="wk_all")
            wv_all = pers.tile([128, 4 * DIM], bf16, name="wv_all")
            wp_all = pers.tile([128, 4 * DIM], bf16, name="wp_all")
            x16_sb = [x16_all[:, NPOS * c : NPOS * (c + 1)] for c in range(4)]
            wq_sb = [wq_all[:, 256 * c : 256 * (c + 1)] for c in range(4)]
            wk_sb = [wk_all[:, 256 * c : 256 * (c + 1)] for c in range(4)]
            wv_sb = [wv_all[:, DIM * c : DIM * (c + 1)] for c in range(4)]
            wp_sb = [wp_all[:, DIM * c : DIM * (c + 1)] for c in range(4)]
            bq_sb = pers.tile([128, 2], f32, name="bq_sb")
            bk_sb = pers.tile([128, 2], f32, name="bk_sb")
            bv_sb = pers.tile([128, 4], f32, name="bv_sb")
            bp_sb = pers.tile([128, 4], f32, name="bp_sb")
            wpe_sb = pers.tile([128, 36], f32, name="wpe_sb")
            bpe_sb = pers.tile([128, 4], f32, name="bpe_sb")
            ident_sb = pers.tile([128, 128], bf16, name="ident_sb")
            ones32 = pers.tile([128, 64], bf16, name="ones32")
            q_hi = [pers.tile([128, NPOS], bf16, name=f"qhi{t}") for t in range(2)]
            k_sb = [pers.tile([128, NPOS], bf16, name=f"k{t}") for t in range(2)]
            if f32c1:
                q32 = [pers.tile([128, NPOS], f32, name=f"q32{t}") for t in range(2)]
                k32 = [pers.tile([128, NPOS], f32, name=f"k32{t}") for t in range(2)]
            v_sb = [pers.tile([128, NPOS], bf16, name=f"v{o}") for o in range(4)]
            vT_sb = [pers.tile([128, NH * 65], bf16, name=f"vT{j}") for j in range(NMT)]
            pe_sb = [pers.tile([128, NPOS], bf16, name=f"pe{t}") for t in range(4)]
            vpad = [pers.tile([128, 42 * 42], bf16, name=f"vpad{t}") for t in range(4)]
            pdg_all = pers.tile([128, 36 * 128], bf16, name="pdg_all")
            pdg_sb = [pdg_all[:, 128 * i : 128 * (i + 1)] for i in range(36)]
            z16 = [pers.tile([128, NPOS], bf16, name=f"z16{t}") for t in range(4)]
            zs = [pers.tile([64, HALF], bf16, name=f"zs{g}") for g in range(16)]
            s_g = [pers.tile([128, NPOS], f32, name=f"s_g{i}") for i in range(2)]
            rbf = [pers.tile([128, NPOS], bf16, name=f"rbf{i}") for i in range(2)]

            # ---- input DMAs: consolidated channel-grouped tensors,
            # x split across two hw queues; critical tensors first ----
            nc.sync.dma_start(ident_sb[:], ident_d[:])
            nc.sync.dma_start(x16_all[:, 0 : 2 * NPOS], x16_d[:, 0 : 2 * NPOS])
            nc.scalar.dma_start(
                x16_all[:, 2 * NPOS : 4 * NPOS], x16_d[:, 2 * NPOS : 4 * NPOS]
            )
            nc.sync.dma_start(wq_all[:], wq16_d[:])
            nc.sync.dma_start(wk_all[:], wk16_d[:])
            nc.sync.dma_start(wv_all[:], wv16_d[:])
            nc.scalar.dma_start(bq_sb[:], bq_d[:])
            nc.scalar.dma_start(bk_sb[:], bk_d[:])
            nc.scalar.dma_start(bv_sb[:], bv_d[:])
            nc.sync.dma_start(ones32[:], ones_d[:])
            nc.sync.dma_start(wpe_sb[:], wpe_d[:])
            nc.sync.dma_start(bpe_sb[:], bpe_d[:])
            nc.sync.dma_start(wp_all[:], wp16_d[:])
            nc.sync.dma_start(bp_sb[:], bp_d[:])
            nc.sync.dma_start(pdg_all[:], pdg_d[:])

            for i in range(2):
                nc.gpsimd.memset(s_g[i][:], 1.0)
            for t in range(4):
                vg = vpad[t].rearrange("p (a b) -> p a b", a=42)
                nc.gpsimd.memset(vg[:, 0:1, :], 0.0)
                nc.gpsimd.memset(vg[:, 41:42, :], 0.0)
                nc.gpsimd.memset(vg[:, 1:41, 0:1], 0.0)
                nc.gpsimd.memset(vg[:, 1:41, 41:42], 0.0)
            vT_g = [vT_sb[j].rearrange("p (h g) -> p h g", g=65) for j in range(NMT)]
            for j in range(NMT):
                nc.gpsimd.memset(vT_g[j][0 : mt_sz(j), :, 64:65], 1.0)

            # ---- HAM warm-up: occupy the PE on the identity tile during
            # the input-DMA window so stage A starts at full clock ----
            if warmup:
                wps = ps2.tile([128, 512], f32, name="wup", tag="ps2")
                for i in range(warmup):
                    nc.tensor.matmul(
                        wps[:, 0:128],
                        ident_sb[:],
                        ident_sb[:],
                        start=(i == 0),
                        stop=(i == warmup - 1),
                    )

            # ---- stage A: q, k (bf16, bias via DVE drain) ----
            for w_sb, b_sb, dst in ((wq_sb, bq_sb, q_hi), (wk_sb, bk_sb, k_sb)):
                for t in range(2):
                    for ch in range(4):
                        cs = slice(400 * ch, 400 * (ch + 1))
                        ps = ps2.tile([128, 512], f32, name="psqk", tag="ps2")
                        for c in range(4):
                            nc.tensor.matmul(
                                ps[:, 0:400],
                                w_sb[c][:, 128 * t : 128 * (t + 1)],
                                x16_sb[c][:, cs],
                                start=(c == 0),
                                stop=(c == 3),
                            )
                        nc.vector.tensor_scalar_add(
                            dst[t][:, cs], ps[:, 0:400], b_sb[:, t : t + 1]
                        )
                        if f32c1:
                            dst32 = q32 if dst is q_hi else k32
                            nc.vector.tensor_scalar_add(
                                dst32[t][:, cs], ps[:, 0:400], b_sb[:, t : t + 1]
                            )

            # ---- stage A: v natural, then vT via PE transpose ----
            for o in range(4):
                for ch in range(4):
                    cs = slice(400 * ch, 400 * (ch + 1))
                    ps = ps2.tile([128, 512], f32, name="psv", tag="ps2")
                    for c in range(4):
                        nc.tensor.matmul(
                            ps[:, 0:400],
                            wv_sb[c][:, 128 * o : 128 * (o + 1)],
                            x16_sb[c][:, cs],
                            start=(c == 0),
                            stop=(c == 3),
                        )
                    nc.vector.tensor_scalar_add(
                        v_sb[o][:, cs], ps[:, 0:400], bv_sb[:, o : o + 1]
                    )
                    nc.gpsimd.tensor_copy(
                        vpad[o].rearrange("p (a b) -> p a b", a=42)[
                            :, 1 + 10 * ch : 11 + 10 * ch, 1:41
                        ],
                        v_sb[o][:, cs].rearrange("p (a b) -> p a b", a=10),
                    )

            for j in range(NMT):
                mj = mt_sz(j)
                psT = ps2.tile([128, 512], bf16, name="psT", tag="ps2")
                for t in range(4):
                    nc.tensor.transpose(
                        psT[0:mj, 128 * t : 128 * (t + 1)],
                        v_sb[t][:, 128 * j : 128 * j + mj],
                        ident_sb[:],
                    )
                nc.vector.tensor_copy(
                    vT_g[j][0:mj, :, 0:64],
                    psT[0:mj, :].rearrange("p (h d) -> p h d", d=64),
                )

            # ---- pe: depthwise 3x3 as PE diagonal matmuls. Each (t, ch) job
            # is a contiguous ~1.7us PE burst, scheduled as HAM-warmth filler
            # between attention heads ----
            def make_pe_job(t, ch):
                def pejob():
                    vg = vpad[t].rearrange("p (a b) -> p a b", a=42)
                    ps = ps2.tile([128, 512], f32, name="pspe", tag="ps2")
                    for k9 in range(9):
                        dy, dx = k9 // 3 - 1, k9 % 3 - 1
                        rhs = vg[
                            :, 1 + 10 * ch + dy : 11 + 10 * ch + dy, 1 + dx : 41 + dx
                        ]
                        nc.tensor.matmul(
                            ps[:, 0:400],
                            pdg_sb[9 * t + k9][:],
                            rhs,
                            start=(k9 == 0),
                            stop=(k9 == 8),
                        )
                    nc.vector.tensor_scalar_add(
                        pe_sb[t][:, 400 * ch : 400 * (ch + 1)],
                        ps[:, 0:400],
                        bpe_sb[:, t : t + 1],
                    )

                return pejob

            pe_jobs = {t: [make_pe_job(t, ch) for ch in range(4)] for t in range(4)}

            pe_scr = pers.tile([128, 400], f16, name="pe_scr")

            def make_dummy_job(nmm=9):
                def djob():
                    vg = vpad[0].rearrange("p (a b) -> p a b", a=42)
                    ps = ps2.tile([128, 512], f32, name="psdm", tag="ps2")
                    for k9 in range(nmm):
                        dy, dx = k9 % 3 - 1, k9 // 3 - 1
                        rhs = vg[:, 1 + dy : 11 + dy, 1 + dx : 41 + dx]
                        nc.tensor.matmul(
                            ps[:, 0:400],
                            pdg_sb[k9][:],
                            rhs,
                            start=(k9 == 0),
                            stop=(k9 == nmm - 1),
                        )
                    nc.vector.tensor_copy(pe_scr[:], ps[:, 0:400])

                return djob

            # ---- assembly maker (per half): normalize, +pe, proj, out ----
            def make_assembly(half):
                hs = slice(HALF * half, HALF * (half + 1))
                jobs = []

                def make_sjob(i):
                    def sjob():
                        nc.vector.reciprocal_approx_fast(
                            s_g[i][:, hs], s_g[i][:, hs]
                        )
                        if gcp:
                            nc.gpsimd.tensor_copy(rbf[i][:, hs], s_g[i][:, hs])
                        else:
                            nc.vector.tensor_copy(rbf[i][:, hs], s_g[i][:, hs])

                    return sjob

                def make_tjob(t):
                    def tjob():
                        for i in range(2):
                            h2 = 2 * t + i
                            g = 8 * half + h2
                            sr2 = 32 * (h2 % 4)
                            for off, ncols in ((0, 512), (512, 288)):
                                rb = ps2.tile([128, 512], f32, name="rb", tag="ps2")
                                nc.tensor.matmul(
                                    rb[0:64, 0:ncols],
                                    ones32[sr2 : sr2 + 1, 0:64],
                                    rbf[h2 // 4][
                                        sr2 : sr2 + 1,
                                        HALF * half + off : HALF * half + off + ncols,
                                    ],
                                    tile_position=(sr2, 0),
                                )
                                nc.vector.tensor_tensor(
                                    z16[t][
                                        64 * i : 64 * (i + 1),
                                        HALF * half + off : HALF * half + off + ncols,
                                    ],
                                    zs[g][0:64, off : off + ncols],
                                    rb[0:64, 0:ncols],
                                    op=OP.mult,
                                )
                        if gadd:
                            nc.gpsimd.tensor_tensor(
                                z16[t][:, hs], z16[t][:, hs], pe_sb[t][:, hs],
                                op=OP.add,
                            )
                        else:
                            nc.vector.tensor_tensor(
                                z16[t][:, hs], z16[t][:, hs], pe_sb[t][:, hs],
                                op=OP.add,
                            )

                    return tjob

                def make_pjob(o, ch, drain=None):
                    def pjob():
                        cs = slice(
                            HALF * half + 400 * ch, HALF * half + 400 * (ch + 1)
                        )
                        pj = ps2.tile([128, 512], f32, name="pj", tag="ps2")
                        for c in range(4):
                            nc.tensor.matmul(
                                pj[:, 0:400],
                                wp_sb[c][:, 128 * o : 128 * (o + 1)],
                                z16[c][:, cs],
                                start=(c == 0),
                                stop=(c == 3),
                            )
                        yt = ystg.tile([128, 400], bf16, name="yt", tag="yt")
                        if drain == "scalar":
                            nc.scalar.activation(
                                yt[:], pj[:, 0:400], AF.Identity,
                                bias=bp_sb[:, o : o + 1],
                            )
                        else:
                            nc.vector.tensor_scalar_add(
                                yt[:], pj[:, 0:400], bp_sb[:, o : o + 1]
                            )
                        nc.sync.dma_start(y_d[128 * o : 128 * (o + 1), cs], yt[:])

                    return pjob

                jb = {}
                jb["s0"] = make_sjob(0)
                jb["s1"] = make_sjob(1)
                for t in range(4):
                    jb[f"t{t}"] = make_tjob(t)
                for o in range(4):
                    for ch in range(2):
                        jb[f"p{o}{ch}"] = make_pjob(o, ch)
                        jb[f"P{o}{ch}"] = make_pjob(o, ch, drain="scalar")
                return jb

            # ---- explicit per-head filler/assembly schedule ----
            # Keeps the PE densely busy every head (HAM K=8/8) while honoring
            # cross-engine dependencies with at least a one-head lead time.
            asm0 = make_assembly(0)
            asm1 = make_assembly(1)
            D = make_dummy_job
            head_order = {0: list(range(8)), 1: list(range(8))}
            slot = {}
            for h in range(4):
                slot[(0, h)] = [pe_jobs[0][h], pe_jobs[1][h]]
            slot[(0, 4)] = [asm0["s0"], pe_jobs[2][0], pe_jobs[2][1]]
            slot[(0, 5)] = [asm0["t0"], pe_jobs[2][2], pe_jobs[2][3]]
            slot[(0, 6)] = [asm0["t1"], pe_jobs[3][0], pe_jobs[3][1]]
            slot[(0, 7)] = [pe_jobs[3][2], pe_jobs[3][3], D()]
            slot[(1, 0)] = [asm0["s1"], D(), D()] if not trim else [asm0["s1"], D()]
            slot[(1, 1)] = [asm0["t2"], asm0["t3"], D()]
            pk = "P" if syt else "p"
            slot[(1, 2)] = [asm0[pk + "00"], asm0[pk + "01"], D()]
            slot[(1, 3)] = [asm0[pk + "10"], asm0[pk + "11"], D()]
            slot[(1, 4)] = [asm0[pk + "20"], asm0[pk + "21"], asm1["s0"], D()]
            slot[(1, 5)] = [asm0[pk + "30"], asm0[pk + "31"], asm1["t0"], D()]
            slot[(1, 6)] = [asm1["t1"], D(), D()] if not trim else [asm1["t1"], D()]
            slot[(1, 7)] = [D()]
            for h in range(8):
                ds = [D(extra_nmm) for _ in range(extra_d)]
                if dpre:
                    slot[(1, h)] = ds + slot[(1, h)]
                else:
                    slot[(1, h)] += ds
            tail = [asm1["s1"], D(), asm1["t2"], asm1["t3"], D(5)] + [
                asm1[f"P{o}{ch}"] for o in range(4) for ch in range(2)
            ]

            for half in range(2):
                hs2 = slice(HALF * half, HALF * (half + 1))
                c0 = slice(HALF * half, HALF * half + 512)
                c1 = slice(HALF * half + 512, HALF * half + 800)
                for hp, h in enumerate(head_order[half]):
                    t = h // 4
                    sr = 32 * (h % 4)
                    g = 8 * half + h
                    mm = mmp.tile([65, HALF], f32, name="mm", tag="mm")

                    def mm3(j, E):
                        mj = mt_sz(j)
                        lhsT = vT_g[j][0:mj, h, :]
                        nc.tensor.matmul(
                            mm[:, 0:512],
                            lhsT,
                            E[0:mj, 0:512],
                            start=(j == 0),
                            stop=(j == NMT - 1),
                        )
                        nc.tensor.matmul(
                            mm[:, 512:800],
                            lhsT,
                            E[0:mj, 512:800],
                            start=(j == 0),
                            stop=(j == NMT - 1),
                        )

                    pipe = []
                    for j in range(NMT):
                        mj = mt_sz(j)
                        ms = slice(128 * j, 128 * j + mj)
                        sc = scp.tile([128, HALF], f32, name="sc", tag="sc")
                        nc.tensor.matmul(
                            sc[0:mj, 0:512],
                            k_sb[t][sr : sr + 32, ms],
                            q_hi[t][sr : sr + 32, c0],
                            tile_position=(sr, 0),
                        )
                        if f32c1:
                            nc.tensor.matmul(
                                sc[0:mj, 512:800],
                                k32[t][sr : sr + 32, ms],
                                q32[t][sr : sr + 32, c1],
                                tile_position=(sr, 0),
                            )
                        else:
                            nc.tensor.matmul(
                                sc[0:mj, 512:800],
                                k_sb[t][sr : sr + 32, ms],
                                q_hi[t][sr : sr + 32, c1],
                                tile_position=(sr, 0),
                            )
                        E = ep.tile([128, HALF], bf16, name="E", tag="E")
                        nc.scalar.activation(E[0:mj, :], sc[0:mj, :], AF.Exp)
                        pipe.append((j, E))
                        if len(pipe) > 2:
                            mm3(*pipe.pop(0))
                    for it in pipe:
                        mm3(*it)
                    nc.vector.tensor_copy(zs[g][:], mm[0:64, :])
                    nc.vector.tensor_copy(s_g[h // 4][sr : sr + 1, hs2], mm[64:65, :])
                    for jobf in slot[(half, hp)]:
                        jobf()
            for jobf in tail:
                jobf()

            if dump:
                dbg_specs = [
                    ("q0", q_hi[0]),
                    ("k0", k_sb[0]),
                    ("v0", v_sb[0]),
                    ("vt0", vT_sb[0]),
                    ("pe0", pe_sb[0]),
                    ("zs0", zs[0]),
                    ("z160", z16[0]),
                ]
                for nm, t_sb in dbg_specs:
                    t_d = nc.dram_tensor(
                        f"dbg_{nm}", list(t_sb.shape), t_sb.dtype, kind="ExternalOutput"
                    ).ap()
                    nc.sync.dma_start(t_d[:], t_sb[:])

    nc.compile()
    return nc


def prep_weights(inputs):
    import ml_dtypes

    bfl = ml_dtypes.bfloat16
    d = lambda k: np.asarray(inputs[k], dtype=np.float64)
    inv = d("qkv_gamma") / np.sqrt(d("qkv_var") + EPS)
    W = d("qkv_w") * inv[:, None]
    bb = d("qkv_beta") - d("qkv_mean") * inv
    Wh = W.reshape(NH, 2 * KD + HD, DIM)
    bh = bb.reshape(NH, 2 * KD + HD)
    Wq = (Wh[:, :KD] * SCALE).reshape(NH * KD, DIM)
    bq = (bh[:, :KD] * SCALE).reshape(-1)
    Wk = Wh[:, KD : 2 * KD].reshape(NH * KD, DIM)
    bk = bh[:, KD : 2 * KD].reshape(-1)
    Wv = Wh[:, 2 * KD :].reshape(NH * HD, DIM)
    bv = bh[:, 2 * KD :].reshape(-1)

    ipe = d("pe_gamma") / np.sqrt(d("pe_var") + EPS)
    wpe = d("pe_w")[:, 0] * ipe[:, None, None]  # [512, 3, 3]
    bpe = d("pe_beta") - d("pe_mean") * ipe
    wpe_tap = np.zeros((128, 36), np.float64)
    for t in range(4):
        for k9 in range(9):
            wpe_tap[:, 9 * t + k9] = wpe[128 * t : 128 * (t + 1), k9 // 3, k9 % 3]
    pdg = np.zeros((36, 128, 128), np.float64)
    ar = np.arange(128)
    for t in range(4):
        for k9 in range(9):
            pdg[t * 9 + k9, ar, ar] = wpe[128 * t : 128 * (t + 1), k9 // 3, k9 % 3]

    ip = d("proj_gamma") / np.sqrt(d("proj_var") + EPS)
    Wp = d("proj_w") * ip[:, None]
    bp = d("proj_beta") - d("proj_mean") * ip

    c32 = lambda a: np.ascontiguousarray(a, dtype=np.float32)
    c16 = lambda a: np.ascontiguousarray(a.astype(np.float32), dtype=bfl)

    def grp(wT):
        # [512, m] -> [128, 4*m]: row p = concat over c of wT[128c+p, :]
        m = wT.shape[1]
        return wT.reshape(4, 128, m).transpose(1, 0, 2).reshape(128, 4 * m)

    return dict(
        wq16=c16(grp(Wq.T)),
        wk16=c16(grp(Wk.T)),
        wv16=c16(grp(Wv.T)),
        wp16=c16(grp(Wp.T)),
        bq=c32(bq.reshape(2, 128).T),
        bk=c32(bk.reshape(2, 128).T),
        bv=c32(bv.reshape(4, 128).T),
        bp=c32(bp.reshape(4, 128).T),
        wpe=c32(wpe_tap),
        bpe=c32(bpe.reshape(4, 128).T),
        ident=c16(np.eye(128)),
        ones=c16(np.ones((128, 64))),
        pdg=c16(pdg.transpose(1, 0, 2).reshape(128, 36 * 128)),
    )


def make_in_maps(inputs):
    import ml_dtypes

    w = prep_weights(inputs)
    x = np.asarray(inputs["x"], dtype=np.float32)
    B = x.shape[0]
    maps = []
    for i in range(B):
        xi = x[i].reshape(4, 128, NPOS).transpose(1, 0, 2).reshape(128, 4 * NPOS)
        maps.append({"x16": np.ascontiguousarray(xi).astype(ml_dtypes.bfloat16), **w})
    return maps


def kernel(**inputs):
    global _compiled_nc
    from concourse.bass_utils import run_bass_kernel_spmd

    if _compiled_nc is None:
        _compiled_nc = build_nc()
    in_maps = make_in_maps(inputs)
    res = run_bass_kernel_spmd(_compiled_nc, in_maps, core_ids=list(range(8)))
    y = np.stack(
        [
            np.asarray(res.results[i]["y"], dtype=np.float32).reshape(DIM, 40, 40)
            for i in range(8)
        ]
    )
    return y


if __name__ == "__main__":
    nc = build_nc()
    print("built ok")



# revision 21
# speedup vs baseline: 1.5091x; 1.0234x over previous
"""Trainium2 Bass kernel for nn_Attention_56822417326562 (dense transformer block).

Sharding: data-parallel over batch — core i computes batch element i entirely
(B=8 over 8 NeuronCores, no collectives).

Per-core math (x: [512, 1600]):
  BN folded into weights on host; softmax scale folded into q. All inputs are
  DMAed as channel-grouped [128, 4*m] tensors (one big-packet DMA each) on the
  sync queue, critical tensors first.
  Stage A (PE): q, k, v via 1x1 convs (bf16), then vT built by PE transposes
  of v (identity matmul) with a ones column appended per head for the softmax
  denominator. k is consumed via 32-row tile_position matmuls (no zero
  padding of the contraction dim), biases folded into the DVE PSUM drains.
  Attention (per half of n, per head): scores S^T[m,n] on PE (bf16), exp on
  ScalarE (its only work — the pacing engine), out_un[d,n] and s[n] in one PE
  accumulation via the vT ones column. The mm accumulator is drained by DVE
  copies (zs per head, s row into s_g); 1/s via full-tile DVE reciprocal at
  assembly time (single-partition reciprocal miscomputes on HW).
  pe = depthwise 3x3 as 9 diagonal bf16 matmuls over zero-padded v.
  Assembly (per half): 1/s broadcast via bf16 ones-matmul (tile_position row
  = head), z16 = out_un * (1/s) + pe (DVE), proj on PE, proj bias folded into
  the PSUM->SBUF drain, y DMAed out as bf16 per 400-col block.

HAM clock-gate management (the dominant perf effect): the PE clock sits at
1.2 GHz unless each free-running 3.4us activity window is ~fully busy; any
sparse window re-throttles to K=4/8 and halves PE speed for >=13.6us. The
per-head slot schedule interleaves contiguous ~1.7us PE bursts (the real
depthwise-conv jobs, the previous half's normalize/proj assembly jobs, and a
few discarded dummy bursts) between attention heads so the PE queue always
holds a multi-us backlog. Cross-engine dependencies get at least a one-head
lead so the in-order PE queue never stalls on DVE results. Separately, the
chip has a persistent P-state that can inflate all engines ~18% run to run;
only same-process paired A/B comparisons are meaningful.
"""
import sys

sys.path.insert(0, "/opt/trn_rl_repo")

import numpy as np

DIM = 512
NH = 8
HD = 64
KD = 32
NPOS = 1600
EPS = 1e-5
SCALE = float(KD) ** -0.5
NMT = 13  # position tiles: 12*128 + 64
HALF = 800

_compiled_nc = None


def build_nc(dump=False, f32c1=False, extra_d=0, extra_nmm=9, warmup=64, trim=True, gadd=True, gcp=False, syt=False, epb=6, dpre=False):
    import concourse.tile as tile
    from concourse import bacc, mybir

    f32 = mybir.dt.float32
    f32r = mybir.dt.float32r
    f16 = mybir.dt.float16
    bf16 = mybir.dt.bfloat16
    AF = mybir.ActivationFunctionType
    OP = mybir.AluOpType

    nc = bacc.Bacc("TRN2", target_bir_lowering=False, debug=False, num_devices=8)

    x16_d = nc.dram_tensor("x16", [128, 4 * NPOS], bf16, kind="ExternalInput").ap()
    wq16_d = nc.dram_tensor("wq16", [128, 4 * 256], bf16, kind="ExternalInput").ap()
    wk16_d = nc.dram_tensor("wk16", [128, 4 * 256], bf16, kind="ExternalInput").ap()
    wv16_d = nc.dram_tensor("wv16", [128, 4 * DIM], bf16, kind="ExternalInput").ap()
    wp16_d = nc.dram_tensor("wp16", [128, 4 * DIM], bf16, kind="ExternalInput").ap()
    bq_d = nc.dram_tensor("bq", [128, 2], f32, kind="ExternalInput").ap()
    bk_d = nc.dram_tensor("bk", [128, 2], f32, kind="ExternalInput").ap()
    bv_d = nc.dram_tensor("bv", [128, 4], f32, kind="ExternalInput").ap()
    bp_d = nc.dram_tensor("bp", [128, 4], f32, kind="ExternalInput").ap()
    wpe_d = nc.dram_tensor("wpe", [128, 36], f32, kind="ExternalInput").ap()
    bpe_d = nc.dram_tensor("bpe", [128, 4], f32, kind="ExternalInput").ap()
    ident_d = nc.dram_tensor("ident", [128, 128], bf16, kind="ExternalInput").ap()
    ones_d = nc.dram_tensor("ones", [128, 64], bf16, kind="ExternalInput").ap()
    pdg_d = nc.dram_tensor("pdg", [128, 36 * 128], bf16, kind="ExternalInput").ap()
    y_d = nc.dram_tensor("y", [DIM, NPOS], bf16, kind="ExternalOutput").ap()

    def mt_sz(j):
        return 64 if j == NMT - 1 else 128

    with tile.TileContext(nc) as tc:
        with (
            tc.tile_pool(name="pers", bufs=1) as pers,
            tc.tile_pool(name="ps2", bufs=2, space="PSUM") as ps2,
            tc.tile_pool(name="scp", bufs=2, space="PSUM") as scp,
            tc.tile_pool(name="mmp", bufs=1, space="PSUM") as mmp,
            tc.tile_pool(name="ep", bufs=epb) as ep,
            tc.tile_pool(name="ystg", bufs=4) as ystg,
        ):
            x16_all = pers.tile([128, 4 * NPOS], bf16, name="x16_all")
            wq_all = pers.tile([128, 4 * 256], bf16, name="wq_all")
            wk_all = pers.tile([128, 4 * 256], bf16, name="wk_all")
            wv_all = pers.tile([128, 4 * DIM], bf16, name="wv_all")
            wp_all = pers.tile([128, 4 * DIM], bf16, name="wp_all")
            x16_sb = [x16_all[:, NPOS * c : NPOS * (c + 1)] for c in range(4)]
            wq_sb = [wq_all[:, 256 * c : 256 * (c + 1)] for c in range(4)]
            wk_sb = [wk_all[:, 256 * c : 256 * (c + 1)] for c in range(4)]
            wv_sb = [wv_all[:, DIM * c : DIM * (c + 1)] for c in range(4)]
            wp_sb = [wp_all[:, DIM * c : DIM * (c + 1)] for c in range(4)]
            bq_sb = pers.tile([128, 2], f32, name="bq_sb")
            bk_sb = pers.tile([128, 2], f32, name="bk_sb")
            bv_sb = pers.tile([128, 4], f32, name="bv_sb")
            bp_sb = pers.tile([128, 4], f32, name="bp_sb")
            wpe_sb = pers.tile([128, 36], f32, name="wpe_sb")
            bpe_sb = pers.tile([128, 4], f32, name="bpe_sb")
            ident_sb = pers.tile([128, 128], bf16, name="ident_sb")
            ones32 = pers.tile([128, 64], bf16, name="ones32")
            q_hi = [pers.tile([128, NPOS], bf16, name=f"qhi{t}") for t in range(2)]
            k_sb = [pers.tile([128, NPOS], bf16, name=f"k{t}") for t in range(2)]
            if f32c1:
                q32 = [pers.tile([128, NPOS], f32, name=f"q32{t}") for t in range(2)]
                k32 = [pers.tile([128, NPOS], f32, name=f"k32{t}") for t in range(2)]
            v_sb = [pers.tile([128, NPOS], bf16, name=f"v{o}") for o in range(4)]
            vT_sb = [pers.tile([128, NH * 65], bf16, name=f"vT{j}") for j in range(NMT)]
            pe_sb = [pers.tile([128, NPOS], bf16, name=f"pe{t}") for t in range(4)]
            vpad = [pers.tile([128, 42 * 42], bf16, name=f"vpad{t}") for t in range(4)]
            pdg_all = pers.tile([128, 36 * 128], bf16, name="pdg_all")
            pdg_sb = [pdg_all[:, 128 * i : 128 * (i + 1)] for i in range(36)]
            z16 = [pers.tile([128, NPOS], bf16, name=f"z16{t}") for t in range(4)]
            zs = [pers.tile([64, HALF], bf16, name=f"zs{g}") for g in range(16)]
            s_g = [pers.tile([128, NPOS], f32, name=f"s_g{i}") for i in range(2)]
            rbf = [pers.tile([128, NPOS], bf16, name=f"rbf{i}") for i in range(2)]

            # ---- input DMAs: consolidated channel-grouped tensors,
            # x split across two hw queues; critical tensors first ----
            nc.sync.dma_start(ident_sb[:], ident_d[:])
            nc.sync.dma_start(x16_all[:, 0 : 2 * NPOS], x16_d[:, 0 : 2 * NPOS])
            nc.scalar.dma_start(
                x16_all[:, 2 * NPOS : 4 * NPOS], x16_d[:, 2 * NPOS : 4 * NPOS]
            )
            nc.sync.dma_start(wq_all[:], wq16_d[:])
            nc.sync.dma_start(wk_all[:], wk16_d[:])
            nc.sync.dma_start(wv_all[:], wv16_d[:])
            nc.scalar.dma_start(bq_sb[:], bq_d[:])
            nc.scalar.dma_start(bk_sb[:], bk_d[:])
            nc.scalar.dma_start(bv_sb[:], bv_d[:])
            nc.sync.dma_start(ones32[:], ones_d[:])
            nc.sync.dma_start(wpe_sb[:], wpe_d[:])
            nc.sync.dma_start(bpe_sb[:], bpe_d[:])
            nc.sync.dma_start(wp_all[:], wp16_d[:])
            nc.sync.dma_start(bp_sb[:], bp_d[:])
            nc.sync.dma_start(pdg_all[:], pdg_d[:])

            for i in range(2):
                nc.gpsimd.memset(s_g[i][:], 1.0)
            for t in range(4):
                vg = vpad[t].rearrange("p (a b) -> p a b", a=42)
                nc.gpsimd.memset(vg[:, 0:1, :], 0.0)
                nc.gpsimd.memset(vg[:, 41:42, :], 0.0)
                nc.gpsimd.memset(vg[:, 1:41, 0:1], 0.0)
                nc.gpsimd.memset(vg[:, 1:41, 41:42], 0.0)
            vT_g = [vT_sb[j].rearrange("p (h g) -> p h g", g=65) for j in range(NMT)]
            for j in range(NMT):
                nc.gpsimd.memset(vT_g[j][0 : mt_sz(j), :, 64:65], 1.0)

            # ---- HAM warm-up: occupy the PE on the identity tile during
            # the input-DMA window so stage A starts at full clock ----
            if warmup:
                wps = ps2.tile([128, 512], f32, name="wup", tag="ps2")
                for i in range(warmup):
                    nc.tensor.matmul(
                        wps[:, 0:128],
                        ident_sb[:],
                        ident_sb[:],
                        start=(i == 0),
                        stop=(i == warmup - 1),
                    )

            # ---- stage A: q, k (bf16, bias via DVE drain) ----
            for w_sb, b_sb, dst in ((wq_sb, bq_sb, q_hi), (wk_sb, bk_sb, k_sb)):
                for t in range(2):
                    for ch in range(4):
                        cs = slice(400 * ch, 400 * (ch + 1))
                        ps = ps2.tile([128, 512], f32, name="psqk", tag="ps2")
                        for c in range(4):
                            nc.tensor.matmul(
                                ps[:, 0:400],
                                w_sb[c][:, 128 * t : 128 * (t + 1)],
                                x16_sb[c][:, cs],
                                start=(c == 0),
                                stop=(c == 3),
                            )
                        nc.vector.tensor_scalar_add(
                            dst[t][:, cs], ps[:, 0:400], b_sb[:, t : t + 1]
                        )
                        if f32c1:
                            dst32 = q32 if dst is q_hi else k32
                            nc.vector.tensor_scalar_add(
                                dst32[t][:, cs], ps[:, 0:400], b_sb[:, t : t + 1]
                            )

            # ---- stage A: v natural, then vT via PE transpose ----
            for o in range(4):
                for ch in range(4):
                    cs = slice(400 * ch, 400 * (ch + 1))
                    ps = ps2.tile([128, 512], f32, name="psv", tag="ps2")
                    for c in range(4):
                        nc.tensor.matmul(
                            ps[:, 0:400],
                            wv_sb[c][:, 128 * o : 128 * (o + 1)],
                            x16_sb[c][:, cs],
                            start=(c == 0),
                            stop=(c == 3),
                        )
                    nc.vector.tensor_scalar_add(
                        v_sb[o][:, cs], ps[:, 0:400], bv_sb[:, o : o + 1]
                    )
                    nc.gpsimd.tensor_copy(
                        vpad[o].rearrange("p (a b) -> p a b", a=42)[
                            :, 1 + 10 * ch : 11 + 10 * ch, 1:41
                        ],
                        v_sb[o][:, cs].rearrange("p (a b) -> p a b", a=10),
                    )

            for j in range(NMT):
                mj = mt_sz(j)
                psT = ps2.tile([128, 512], bf16, name="psT", tag="ps2")
                for t in range(4):
                    nc.tensor.transpose(
                        psT[0:mj, 128 * t : 128 * (t + 1)],
                        v_sb[t][:, 128 * j : 128 * j + mj],
                        ident_sb[:],
                    )
                nc.vector.tensor_copy(
                    vT_g[j][0:mj, :, 0:64],
                    psT[0:mj, :].rearrange("p (h d) -> p h d", d=64),
                )

            # ---- pe: depthwise 3x3 as PE diagonal matmuls. Each (t, ch) job
            # is a contiguous ~1.7us PE burst, scheduled as HAM-warmth filler
            # between attention heads ----
            def make_pe_job(t, ch):
                def pejob():
                    vg = vpad[t].rearrange("p (a b) -> p a b", a=42)
                    ps = ps2.tile([128, 512], f32, name="pspe", tag="ps2")
                    for k9 in range(9):
                        dy, dx = k9 // 3 - 1, k9 % 3 - 1
                        rhs = vg[
                            :, 1 + 10 * ch + dy : 11 + 10 * ch + dy, 1 + dx : 41 + dx
                        ]
                        nc.tensor.matmul(
                            ps[:, 0:400],
                            pdg_sb[9 * t + k9][:],
                            rhs,
                            start=(k9 == 0),
                            stop=(k9 == 8),
                        )
                    nc.vector.tensor_scalar_add(
                        pe_sb[t][:, 400 * ch : 400 * (ch + 1)],
                        ps[:, 0:400],
                        bpe_sb[:, t : t + 1],
                    )

                return pejob

            pe_jobs = {t: [make_pe_job(t, ch) for ch in range(4)] for t in range(4)}

            pe_scr = pers.tile([128, 400], f16, name="pe_scr")

            def make_dummy_job(nmm=9):
                def djob():
                    vg = vpad[0].rearrange("p (a b) -> p a b", a=42)
                    ps = ps2.tile([128, 512], f32, name="psdm", tag="ps2")
                    for k9 in range(nmm):
                        dy, dx = k9 % 3 - 1, k9 // 3 - 1
                        rhs = vg[:, 1 + dy : 11 + dy, 1 + dx : 41 + dx]
                        nc.tensor.matmul(
                            ps[:, 0:400],
                            pdg_sb[k9][:],
                            rhs,
                            start=(k9 == 0),
                            stop=(k9 == nmm - 1),
                        )
                    nc.vector.tensor_copy(pe_scr[:], ps[:, 0:400])

                return djob

            # ---- assembly maker (per half): normalize, +pe, proj, out ----
            def make_assembly(half):
                hs = slice(HALF * half, HALF * (half + 1))
                jobs = []

                def make_sjob(i):
                    def sjob():
                        nc.vector.reciprocal_approx_fast(
                            s_g[i][:, hs], s_g[i][:, hs]
                        )
                        if gcp:
                            nc.gpsimd.tensor_copy(rbf[i][:, hs], s_g[i][:, hs])
                        else:
                            nc.vector.tensor_copy(rbf[i][:, hs], s_g[i][:, hs])

                    return sjob

                def make_tjob(t):
                    def tjob():
                        for i in range(2):
                            h2 = 2 * t + i
                            g = 8 * half + h2
                            sr2 = 32 * (h2 % 4)
                            for off, ncols in ((0, 512), (512, 288)):
                                rb = ps2.tile([128, 512], f32, name="rb", tag="ps2")
                                nc.tensor.matmul(
                                    rb[0:64, 0:ncols],
                                    ones32[sr2 : sr2 + 1, 0:64],
                                    rbf[h2 // 4][
                                        sr2 : sr2 + 1,
                                        HALF * half + off : HALF * half + off + ncols,
                                    ],
                                    tile_position=(sr2, 0),
                                )
                                nc.vector.tensor_tensor(
                                    z16[t][
                                        64 * i : 64 * (i + 1),
                                        HALF * half + off : HALF * half + off + ncols,
                                    ],
                                    zs[g][0:64, off : off + ncols],
                                    rb[0:64, 0:ncols],
                                    op=OP.mult,
                                )
                        if gadd:
                            nc.gpsimd.tensor_tensor(
                                z16[t][:, hs], z16[t][:, hs], pe_sb[t][:, hs],
                                op=OP.add,
                            )
                        else:
                            nc.vector.tensor_tensor(
                                z16[t][:, hs], z16[t][:, hs], pe_sb[t][:, hs],
                                op=OP.add,
                            )

                    return tjob

                def make_pjob(o, ch, drain=None):
                    def pjob():
                        cs = slice(
                            HALF * half + 400 * ch, HALF * half + 400 * (ch + 1)
                        )
                        pj = ps2.tile([128, 512], f32, name="pj", tag="ps2")
                        for c in range(4):
                            nc.tensor.matmul(
                                pj[:, 0:400],
                                wp_sb[c][:, 128 * o : 128 * (o + 1)],
                                z16[c][:, cs],
                                start=(c == 0),
                                stop=(c == 3),
                            )
                        yt = ystg.tile([128, 400], bf16, name="yt", tag="yt")
                        if drain == "scalar":
                            nc.scalar.activation(
                                yt[:], pj[:, 0:400], AF.Identity,
                                bias=bp_sb[:, o : o + 1],
                            )
                        else:
                            nc.vector.tensor_scalar_add(
                                yt[:], pj[:, 0:400], bp_sb[:, o : o + 1]
                            )
                        nc.sync.dma_start(y_d[128 * o : 128 * (o + 1), cs], yt[:])

                    return pjob

                jb = {}
                jb["s0"] = make_sjob(0)
                jb["s1"] = make_sjob(1)
                for t in range(4):
                    jb[f"t{t}"] = make_tjob(t)
                for o in range(4):
                    for ch in range(2):
                        jb[f"p{o}{ch}"] = make_pjob(o, ch)
                        jb[f"P{o}{ch}"] = make_pjob(o, ch, drain="scalar")
                return jb

            # ---- explicit per-head filler/assembly schedule ----
            # Keeps the PE densely busy every head (HAM K=8/8) while honoring
            # cross-engine dependencies with at least a one-head lead time.
            asm0 = make_assembly(0)
            asm1 = make_assembly(1)
            D = make_dummy_job
            head_order = {0: list(range(8)), 1: list(range(8))}
            slot = {}
            for h in range(4):
                slot[(0, h)] = [pe_jobs[0][h], pe_jobs[1][h]]
            slot[(0, 4)] = [asm0["s0"], pe_jobs[2][0], pe_jobs[2][1]]
            slot[(0, 5)] = [asm0["t0"], pe_jobs[2][2], pe_jobs[2][3]]
            slot[(0, 6)] = [asm0["t1"], pe_jobs[3][0], pe_jobs[3][1]]
            slot[(0, 7)] = [pe_jobs[3][2], pe_jobs[3][3], D()]
            slot[(1, 0)] = [asm0["s1"], D(), D()] if not trim else [asm0["s1"], D()]
            slot[(1, 1)] = [asm0["t2"], asm0["t3"], D()]
            pk = "P" if syt else "p"
            slot[(1, 2)] = [asm0[pk + "00"], asm0[pk + "01"], D()]
            slot[(1, 3)] = [asm0[pk + "10"], asm0[pk + "11"], D()]
            slot[(1, 4)] = [asm0[pk + "20"], asm0[pk + "21"], asm1["s0"], D()]
            slot[(1, 5)] = [asm0[pk + "30"], asm0[pk + "31"], asm1["t0"], D()]
            slot[(1, 6)] = [asm1["t1"], D(), D()] if not trim else [asm1["t1"], D()]
            slot[(1, 7)] = [D()]
            for h in range(8):
                ds = [D(extra_nmm) for _ in range(extra_d)]
                if dpre:
                    slot[(1, h)] = ds + slot[(1, h)]
                else:
                    slot[(1, h)] += ds
            tail = [asm1["s1"], D(), asm1["t2"], asm1["t3"], D(5)] + [
                asm1[f"P{o}{ch}"] for o in range(4) for ch in range(2)
            ]

            for half in range(2):
                hs2 = slice(HALF * half, HALF * (half + 1))
                c0 = slice(HALF * half, HALF * half + 512)
                c1 = slice(HALF * half + 512, HALF * half + 800)
                for hp, h in enumerate(head_order[half]):
                    t = h // 4
                    sr = 32 * (h % 4)
                    g = 8 * half + h
                    mm = mmp.tile([65, HALF], f32, name="mm", tag="mm")

                    def mm3(j, E):
                        mj = mt_sz(j)
                        lhsT = vT_g[j][0:mj, h, :]
                        nc.tensor.matmul(
                            mm[:, 0:512],
                            lhsT,
                            E[0:mj, 0:512],
                            start=(j == 0),
                            stop=(j == NMT - 1),
                        )
                        nc.tensor.matmul(
                            mm[:, 512:800],
                            lhsT,
                            E[0:mj, 512:800],
                            start=(j == 0),
                            stop=(j == NMT - 1),
                        )

                    pipe = []
                    for j in range(NMT):
                        mj = mt_sz(j)
                        ms = slice(128 * j, 128 * j + mj)
                        sc = scp.tile([128, HALF], f32, name="sc", tag="sc")
                        nc.tensor.matmul(
                            sc[0:mj, 0:512],
                            k_sb[t][sr : sr + 32, ms],
                            q_hi[t][sr : sr + 32, c0],
                            tile_position=(sr, 0),
                        )
                        if f32c1:
                            nc.tensor.matmul(
                                sc[0:mj, 512:800],
                                k32[t][sr : sr + 32, ms],
                                q32[t][sr : sr + 32, c1],
                                tile_position=(sr, 0),
                            )
                        else:
                            nc.tensor.matmul(
                                sc[0:mj, 512:800],
                                k_sb[t][sr : sr + 32, ms],
                                q_hi[t][sr : sr + 32, c1],
                                tile_position=(sr, 0),
                            )
                        E = ep.tile([128, HALF], bf16, name="E", tag="E")
                        nc.scalar.activation(E[0:mj, :], sc[0:mj, :], AF.Exp)
                        pipe.append((j, E))
                        if len(pipe) > 2:
                            mm3(*pipe.pop(0))
                    for it in pipe:
                        mm3(*it)
                    nc.vector.tensor_copy(zs[g][:], mm[0:64, :])
                    nc.vector.tensor_copy(s_g[h // 4][sr : sr + 1, hs2], mm[64:65, :])
                    for jobf in slot[(half, hp)]:
                        jobf()
            for jobf in tail:
                jobf()

            if dump:
                dbg_specs = [
                    ("q0", q_hi[0]),
                    ("k0", k_sb[0]),
                    ("v0", v_sb[0]),
                    ("vt0", vT_sb[0]),
                    ("pe0", pe_sb[0]),
                    ("zs0", zs[0]),
                    ("z160", z16[0]),
                ]
                for nm, t_sb in dbg_specs:
                    t_d = nc.dram_tensor(
                        f"dbg_{nm}", list(t_sb.shape), t_sb.dtype, kind="ExternalOutput"
                    ).ap()
                    nc.sync.dma_start(t_d[:], t_sb[:])

    nc.compile()
    return nc


def prep_weights(inputs):
    import ml_dtypes

    bfl = ml_dtypes.bfloat16
    d = lambda k: np.asarray(inputs[k], dtype=np.float64)
    inv = d("qkv_gamma") / np.sqrt(d("qkv_var") + EPS)
    W = d("qkv_w") * inv[:, None]
    bb = d("qkv_beta") - d("qkv_mean") * inv
    Wh = W.reshape(NH, 2 * KD + HD, DIM)
    bh = bb.reshape(NH, 2 * KD + HD)
    Wq = (Wh[:, :KD] * SCALE).reshape(NH * KD, DIM)
    bq = (bh[:, :KD] * SCALE).reshape(-1)
    Wk = Wh[:, KD : 2 * KD].reshape(NH * KD, DIM)
    bk = bh[:, KD : 2 * KD].reshape(-1)
    Wv = Wh[:, 2 * KD :].reshape(NH * HD, DIM)
    bv = bh[:, 2 * KD :].reshape(-1)

    ipe = d("pe_gamma") / np.sqrt(d("pe_var") + EPS)
    wpe = d("pe_w")[:, 0] * ipe[:, None, None]  # [512, 3, 3]
    bpe = d("pe_beta") - d("pe_mean") * ipe
    wpe_tap = np.zeros((128, 36), np.float64)
    for t in range(4):
        for k9 in range(9):
            wpe_tap[:, 9 * t + k9] = wpe[128 * t : 128 * (t + 1), k9 // 3, k9 % 3]
    pdg = np.zeros((36, 128, 128), np.float64)
    ar = np.arange(128)
    for t in range(4):
        for k9 in range(9):
            pdg[t * 9 + k9, ar, ar] = wpe[128 * t : 128 * (t + 1), k9 // 3, k9 % 3]

    ip = d("proj_gamma") / np.sqrt(d("proj_var") + EPS)
    Wp = d("proj_w") * ip[:, None]
    bp = d("proj_beta") - d("proj_mean") * ip

    c32 = lambda a: np.ascontiguousarray(a, dtype=np.float32)
    c16 = lambda a: np.ascontiguousarray(a.astype(np.float32), dtype=bfl)

    def grp(wT):
        # [512, m] -> [128, 4*m]: row p = concat over c of wT[128c+p, :]
        m = wT.shape[1]
        return wT.reshape(4, 128, m).transpose(1, 0, 2).reshape(128, 4 * m)

    return dict(
        wq16=c16(grp(Wq.T)),
        wk16=c16(grp(Wk.T)),
        wv16=c16(grp(Wv.T)),
        wp16=c16(grp(Wp.T)),
        bq=c32(bq.reshape(2, 128).T),
        bk=c32(bk.reshape(2, 128).T),
        bv=c32(bv.reshape(4, 128).T),
        bp=c32(bp.reshape(4, 128).T),
        wpe=c32(wpe_tap),
        bpe=c32(bpe.reshape(4, 128).T),
        ident=c16(np.eye(128)),
        ones=c16(np.ones((128, 64))),
        pdg=c16(pdg.transpose(1, 0, 2).reshape(128, 36 * 128)),
    )


def make_in_maps(inputs):
    import ml_dtypes

    w = prep_weights(inputs)
    x = np.asarray(inputs["x"], dtype=np.float32)
    B = x.shape[0]
    maps = []
    for i in range(B):
        xi = x[i].reshape(4, 128, NPOS).transpose(1, 0, 2).reshape(128, 4 * NPOS)
        maps.append({"x16": np.ascontiguousarray(xi).astype(ml_dtypes.bfloat16), **w})
    return maps


def kernel(**inputs):
    global _compiled_nc
    from concourse.bass_utils import run_bass_kernel_spmd

    if _compiled_nc is None:
        _compiled_nc = build_nc()
    in_maps = make_in_maps(inputs)
    res = run_bass_kernel_spmd(_compiled_nc, in_maps, core_ids=list(range(8)))
    y = np.stack(
        [
            np.asarray(res.results[i]["y"], dtype=np.float32).reshape(DIM, 40, 40)
            for i in range(8)
        ]
    )
    return y


if __name__ == "__main__":
    nc = build_nc()
    print("built ok")



# revision 22
# speedup vs baseline: 1.5095x; 1.0002x over previous
"""Trainium2 Bass kernel for nn_Attention_56822417326562 (dense transformer block).

Sharding: data-parallel over batch — core i computes batch element i entirely
(B=8 over 8 NeuronCores, no collectives).

Per-core math (x: [512, 1600]):
  BN folded into weights on host; softmax scale folded into q. All inputs are
  DMAed as channel-grouped [128, 4*m] tensors (one big-packet DMA each) on the
  sync queue, critical tensors first.
  Stage A (PE): q, k, v via 1x1 convs (bf16), then vT built by PE transposes
  of v (identity matmul) with a ones column appended per head for the softmax
  denominator. k is consumed via 32-row tile_position matmuls (no zero
  padding of the contraction dim), biases folded into the DVE PSUM drains.
  Attention (per half of n, per head): scores S^T[m,n] on PE (bf16), exp on
  ScalarE (its only work — the pacing engine), out_un[d,n] and s[n] in one PE
  accumulation via the vT ones column. The mm accumulator is drained by DVE
  copies (zs per head, s row into s_g); 1/s via full-tile DVE reciprocal at
  assembly time (single-partition reciprocal miscomputes on HW).
  pe = depthwise 3x3 as 9 diagonal bf16 matmuls over zero-padded v.
  Assembly (per half): 1/s broadcast via bf16 ones-matmul (tile_position row
  = head), z16 = out_un * (1/s) + pe (DVE), proj on PE, proj bias folded into
  the PSUM->SBUF drain, y DMAed out as bf16 per 400-col block.

HAM clock-gate management (the dominant perf effect): the PE clock sits at
1.2 GHz unless each free-running 3.4us activity window is ~fully busy; any
sparse window re-throttles to K=4/8 and halves PE speed for >=13.6us. The
per-head slot schedule interleaves contiguous ~1.7us PE bursts (the real
depthwise-conv jobs, the previous half's normalize/proj assembly jobs, and a
few discarded dummy bursts) between attention heads so the PE queue always
holds a multi-us backlog. Cross-engine dependencies get at least a one-head
lead so the in-order PE queue never stalls on DVE results. Separately, the
chip has a persistent P-state that can inflate all engines ~18% run to run;
only same-process paired A/B comparisons are meaningful.
"""
import sys

sys.path.insert(0, "/opt/trn_rl_repo")

import numpy as np

DIM = 512
NH = 8
HD = 64
KD = 32
NPOS = 1600
EPS = 1e-5
SCALE = float(KD) ** -0.5
NMT = 13  # position tiles: 12*128 + 64
HALF = 800

_compiled_nc = None


def build_nc(dump=False, f32c1=False, extra_d=0, extra_nmm=9, warmup=80, trim=True, gadd=True, gcp=False, syt=False, epb=6, dpre=False):
    import concourse.tile as tile
    from concourse import bacc, mybir

    f32 = mybir.dt.float32
    f32r = mybir.dt.float32r
    f16 = mybir.dt.float16
    bf16 = mybir.dt.bfloat16
    AF = mybir.ActivationFunctionType
    OP = mybir.AluOpType

    nc = bacc.Bacc("TRN2", target_bir_lowering=False, debug=False, num_devices=8)

    x16_d = nc.dram_tensor("x16", [128, 4 * NPOS], bf16, kind="ExternalInput").ap()
    wq16_d = nc.dram_tensor("wq16", [128, 4 * 256], bf16, kind="ExternalInput").ap()
    wk16_d = nc.dram_tensor("wk16", [128, 4 * 256], bf16, kind="ExternalInput").ap()
    wv16_d = nc.dram_tensor("wv16", [128, 4 * DIM], bf16, kind="ExternalInput").ap()
    wp16_d = nc.dram_tensor("wp16", [128, 4 * DIM], bf16, kind="ExternalInput").ap()
    bq_d = nc.dram_tensor("bq", [128, 2], f32, kind="ExternalInput").ap()
    bk_d = nc.dram_tensor("bk", [128, 2], f32, kind="ExternalInput").ap()
    bv_d = nc.dram_tensor("bv", [128, 4], f32, kind="ExternalInput").ap()
    bp_d = nc.dram_tensor("bp", [128, 4], f32, kind="ExternalInput").ap()
    wpe_d = nc.dram_tensor("wpe", [128, 36], f32, kind="ExternalInput").ap()
    bpe_d = nc.dram_tensor("bpe", [128, 4], f32, kind="ExternalInput").ap()
    ident_d = nc.dram_tensor("ident", [128, 128], bf16, kind="ExternalInput").ap()
    ones_d = nc.dram_tensor("ones", [128, 64], bf16, kind="ExternalInput").ap()
    pdg_d = nc.dram_tensor("pdg", [128, 36 * 128], bf16, kind="ExternalInput").ap()
    y_d = nc.dram_tensor("y", [DIM, NPOS], bf16, kind="ExternalOutput").ap()

    def mt_sz(j):
        return 64 if j == NMT - 1 else 128

    with tile.TileContext(nc) as tc:
        with (
            tc.tile_pool(name="pers", bufs=1) as pers,
            tc.tile_pool(name="ps2", bufs=2, space="PSUM") as ps2,
            tc.tile_pool(name="scp", bufs=2, space="PSUM") as scp,
            tc.tile_pool(name="mmp", bufs=1, space="PSUM") as mmp,
            tc.tile_pool(name="ep", bufs=epb) as ep,
            tc.tile_pool(name="ystg", bufs=4) as ystg,
        ):
            x16_all = pers.tile([128, 4 * NPOS], bf16, name="x16_all")
            wq_all = pers.tile([128, 4 * 256], bf16, name="wq_all")
            wk_all = pers.tile([128, 4 * 256], bf16, name="wk_all")
            wv_all = pers.tile([128, 4 * DIM], bf16, name="wv_all")
            wp_all = pers.tile([128, 4 * DIM], bf16, name="wp_all")
            x16_sb = [x16_all[:, NPOS * c : NPOS * (c + 1)] for c in range(4)]
            wq_sb = [wq_all[:, 256 * c : 256 * (c + 1)] for c in range(4)]
            wk_sb = [wk_all[:, 256 * c : 256 * (c + 1)] for c in range(4)]
            wv_sb = [wv_all[:, DIM * c : DIM * (c + 1)] for c in range(4)]
            wp_sb = [wp_all[:, DIM * c : DIM * (c + 1)] for c in range(4)]
            bq_sb = pers.tile([128, 2], f32, name="bq_sb")
            bk_sb = pers.tile([128, 2], f32, name="bk_sb")
            bv_sb = pers.tile([128, 4], f32, name="bv_sb")
            bp_sb = pers.tile([128, 4], f32, name="bp_sb")
            wpe_sb = pers.tile([128, 36], f32, name="wpe_sb")
            bpe_sb = pers.tile([128, 4], f32, name="bpe_sb")
            ident_sb = pers.tile([128, 128], bf16, name="ident_sb")
            warm_sb = pers.tile([128, 128], bf16, name="warm_sb")
            ones32 = pers.tile([128, 64], bf16, name="ones32")
            q_hi = [pers.tile([128, NPOS], bf16, name=f"qhi{t}") for t in range(2)]
            k_sb = [pers.tile([128, NPOS], bf16, name=f"k{t}") for t in range(2)]
            if f32c1:
                q32 = [pers.tile([128, NPOS], f32, name=f"q32{t}") for t in range(2)]
                k32 = [pers.tile([128, NPOS], f32, name=f"k32{t}") for t in range(2)]
            v_sb = [pers.tile([128, NPOS], bf16, name=f"v{o}") for o in range(4)]
            vT_sb = [pers.tile([128, NH * 65], bf16, name=f"vT{j}") for j in range(NMT)]
            pe_sb = [pers.tile([128, NPOS], bf16, name=f"pe{t}") for t in range(4)]
            vpad = [pers.tile([128, 42 * 42], bf16, name=f"vpad{t}") for t in range(4)]
            pdg_all = pers.tile([128, 36 * 128], bf16, name="pdg_all")
            pdg_sb = [pdg_all[:, 128 * i : 128 * (i + 1)] for i in range(36)]
            z16 = [pers.tile([128, NPOS], bf16, name=f"z16{t}") for t in range(4)]
            zs = [pers.tile([64, HALF], bf16, name=f"zs{g}") for g in range(16)]
            s_g = [pers.tile([128, NPOS], f32, name=f"s_g{i}") for i in range(2)]
            rbf = [pers.tile([128, NPOS], bf16, name=f"rbf{i}") for i in range(2)]

            # ---- input DMAs: consolidated channel-grouped tensors,
            # x split across two hw queues; critical tensors first ----
            nc.sync.dma_start(ident_sb[:], ident_d[:])
            nc.sync.dma_start(x16_all[:, 0 : 2 * NPOS], x16_d[:, 0 : 2 * NPOS])
            nc.scalar.dma_start(
                x16_all[:, 2 * NPOS : 4 * NPOS], x16_d[:, 2 * NPOS : 4 * NPOS]
            )
            nc.sync.dma_start(wq_all[:], wq16_d[:])
            nc.sync.dma_start(wk_all[:], wk16_d[:])
            nc.sync.dma_start(wv_all[:], wv16_d[:])
            nc.scalar.dma_start(bq_sb[:], bq_d[:])
            nc.scalar.dma_start(bk_sb[:], bk_d[:])
            nc.scalar.dma_start(bv_sb[:], bv_d[:])
            nc.sync.dma_start(ones32[:], ones_d[:])
            nc.sync.dma_start(wpe_sb[:], wpe_d[:])
            nc.sync.dma_start(bpe_sb[:], bpe_d[:])
            nc.sync.dma_start(wp_all[:], wp16_d[:])
            nc.sync.dma_start(bp_sb[:], bp_d[:])
            nc.sync.dma_start(pdg_all[:], pdg_d[:])

            nc.gpsimd.memset(warm_sb[:], 0.5)
            for i in range(2):
                nc.gpsimd.memset(s_g[i][:], 1.0)
            for t in range(4):
                vg = vpad[t].rearrange("p (a b) -> p a b", a=42)
                nc.gpsimd.memset(vg[:, 0:1, :], 0.0)
                nc.gpsimd.memset(vg[:, 41:42, :], 0.0)
                nc.gpsimd.memset(vg[:, 1:41, 0:1], 0.0)
                nc.gpsimd.memset(vg[:, 1:41, 41:42], 0.0)
            vT_g = [vT_sb[j].rearrange("p (h g) -> p h g", g=65) for j in range(NMT)]
            for j in range(NMT):
                nc.gpsimd.memset(vT_g[j][0 : mt_sz(j), :, 64:65], 1.0)

            # ---- HAM warm-up: occupy the PE on the identity tile during
            # the input-DMA window so stage A starts at full clock ----
            if warmup:
                wps = ps2.tile([128, 512], f32, name="wup", tag="ps2")
                for i in range(warmup):
                    nc.tensor.matmul(
                        wps[:, 0:128],
                        warm_sb[:],
                        warm_sb[:],
                        start=(i == 0),
                        stop=(i == warmup - 1),
                    )

            # ---- stage A: q, k (bf16, bias via DVE drain) ----
            for w_sb, b_sb, dst in ((wq_sb, bq_sb, q_hi), (wk_sb, bk_sb, k_sb)):
                for t in range(2):
                    for ch in range(4):
                        cs = slice(400 * ch, 400 * (ch + 1))
                        ps = ps2.tile([128, 512], f32, name="psqk", tag="ps2")
                        for c in range(4):
                            nc.tensor.matmul(
                                ps[:, 0:400],
                                w_sb[c][:, 128 * t : 128 * (t + 1)],
                                x16_sb[c][:, cs],
                                start=(c == 0),
                                stop=(c == 3),
                            )
                        nc.vector.tensor_scalar_add(
                            dst[t][:, cs], ps[:, 0:400], b_sb[:, t : t + 1]
                        )
                        if f32c1:
                            dst32 = q32 if dst is q_hi else k32
                            nc.vector.tensor_scalar_add(
                                dst32[t][:, cs], ps[:, 0:400], b_sb[:, t : t + 1]
                            )

            # ---- stage A: v natural, then vT via PE transpose ----
            for o in range(4):
                for ch in range(4):
                    cs = slice(400 * ch, 400 * (ch + 1))
                    ps = ps2.tile([128, 512], f32, name="psv", tag="ps2")
                    for c in range(4):
                        nc.tensor.matmul(
                            ps[:, 0:400],
                            wv_sb[c][:, 128 * o : 128 * (o + 1)],
                            x16_sb[c][:, cs],
                            start=(c == 0),
                            stop=(c == 3),
                        )
                    nc.vector.tensor_scalar_add(
                        v_sb[o][:, cs], ps[:, 0:400], bv_sb[:, o : o + 1]
                    )
                    nc.gpsimd.tensor_copy(
                        vpad[o].rearrange("p (a b) -> p a b", a=42)[
                            :, 1 + 10 * ch : 11 + 10 * ch, 1:41
                        ],
                        v_sb[o][:, cs].rearrange("p (a b) -> p a b", a=10),
                    )

            for j in range(NMT):
                mj = mt_sz(j)
                psT = ps2.tile([128, 512], bf16, name="psT", tag="ps2")
                for t in range(4):
                    nc.tensor.transpose(
                        psT[0:mj, 128 * t : 128 * (t + 1)],
                        v_sb[t][:, 128 * j : 128 * j + mj],
                        ident_sb[:],
                    )
                nc.vector.tensor_copy(
                    vT_g[j][0:mj, :, 0:64],
                    psT[0:mj, :].rearrange("p (h d) -> p h d", d=64),
                )

            # ---- pe: depthwise 3x3 as PE diagonal matmuls. Each (t, ch) job
            # is a contiguous ~1.7us PE burst, scheduled as HAM-warmth filler
            # between attention heads ----
            def make_pe_job(t, ch):
                def pejob():
                    vg = vpad[t].rearrange("p (a b) -> p a b", a=42)
                    ps = ps2.tile([128, 512], f32, name="pspe", tag="ps2")
                    for k9 in range(9):
                        dy, dx = k9 // 3 - 1, k9 % 3 - 1
                        rhs = vg[
                            :, 1 + 10 * ch + dy : 11 + 10 * ch + dy, 1 + dx : 41 + dx
                        ]
                        nc.tensor.matmul(
                            ps[:, 0:400],
                            pdg_sb[9 * t + k9][:],
                            rhs,
                            start=(k9 == 0),
                            stop=(k9 == 8),
                        )
                    nc.vector.tensor_scalar_add(
                        pe_sb[t][:, 400 * ch : 400 * (ch + 1)],
                        ps[:, 0:400],
                        bpe_sb[:, t : t + 1],
                    )

                return pejob

            pe_jobs = {t: [make_pe_job(t, ch) for ch in range(4)] for t in range(4)}

            pe_scr = pers.tile([128, 400], f16, name="pe_scr")

            def make_dummy_job(nmm=9):
                def djob():
                    vg = vpad[0].rearrange("p (a b) -> p a b", a=42)
                    ps = ps2.tile([128, 512], f32, name="psdm", tag="ps2")
                    for k9 in range(nmm):
                        dy, dx = k9 % 3 - 1, k9 // 3 - 1
                        rhs = vg[:, 1 + dy : 11 + dy, 1 + dx : 41 + dx]
                        nc.tensor.matmul(
                            ps[:, 0:400],
                            pdg_sb[k9][:],
                            rhs,
                            start=(k9 == 0),
                            stop=(k9 == nmm - 1),
                        )
                    nc.vector.tensor_copy(pe_scr[:], ps[:, 0:400])

                return djob

            # ---- assembly maker (per half): normalize, +pe, proj, out ----
            def make_assembly(half):
                hs = slice(HALF * half, HALF * (half + 1))
                jobs = []

                def make_sjob(i):
                    def sjob():
                        nc.vector.reciprocal_approx_fast(
                            s_g[i][:, hs], s_g[i][:, hs]
                        )
                        if gcp:
                            nc.gpsimd.tensor_copy(rbf[i][:, hs], s_g[i][:, hs])
                        else:
                            nc.vector.tensor_copy(rbf[i][:, hs], s_g[i][:, hs])

                    return sjob

                def make_tjob(t):
                    def tjob():
                        for i in range(2):
                            h2 = 2 * t + i
                            g = 8 * half + h2
                            sr2 = 32 * (h2 % 4)
                            for off, ncols in ((0, 512), (512, 288)):
                                rb = ps2.tile([128, 512], f32, name="rb", tag="ps2")
                                nc.tensor.matmul(
                                    rb[0:64, 0:ncols],
                                    ones32[sr2 : sr2 + 1, 0:64],
                                    rbf[h2 // 4][
                                        sr2 : sr2 + 1,
                                        HALF * half + off : HALF * half + off + ncols,
                                    ],
                                    tile_position=(sr2, 0),
                                )
                                nc.vector.tensor_tensor(
                                    z16[t][
                                        64 * i : 64 * (i + 1),
                                        HALF * half + off : HALF * half + off + ncols,
                                    ],
                                    zs[g][0:64, off : off + ncols],
                                    rb[0:64, 0:ncols],
                                    op=OP.mult,
                                )
                        if gadd:
                            nc.gpsimd.tensor_tensor(
                                z16[t][:, hs], z16[t][:, hs], pe_sb[t][:, hs],
                                op=OP.add,
                            )
                        else:
                            nc.vector.tensor_tensor(
                                z16[t][:, hs], z16[t][:, hs], pe_sb[t][:, hs],
                                op=OP.add,
                            )

                    return tjob

                def make_pjob(o, ch, drain=None):
                    def pjob():
                        cs = slice(
                            HALF * half + 400 * ch, HALF * half + 400 * (ch + 1)
                        )
                        pj = ps2.tile([128, 512], f32, name="pj", tag="ps2")
                        for c in range(4):
                            nc.tensor.matmul(
                                pj[:, 0:400],
                                wp_sb[c][:, 128 * o : 128 * (o + 1)],
                                z16[c][:, cs],
                                start=(c == 0),
                                stop=(c == 3),
                            )
                        yt = ystg.tile([128, 400], bf16, name="yt", tag="yt")
                        if drain == "scalar":
                            nc.scalar.activation(
                                yt[:], pj[:, 0:400], AF.Identity,
                                bias=bp_sb[:, o : o + 1],
                            )
                        else:
                            nc.vector.tensor_scalar_add(
                                yt[:], pj[:, 0:400], bp_sb[:, o : o + 1]
                            )
                        nc.sync.dma_start(y_d[128 * o : 128 * (o + 1), cs], yt[:])

                    return pjob

                jb = {}
                jb["s0"] = make_sjob(0)
                jb["s1"] = make_sjob(1)
                for t in range(4):
                    jb[f"t{t}"] = make_tjob(t)
                for o in range(4):
                    for ch in range(2):
                        jb[f"p{o}{ch}"] = make_pjob(o, ch)
                        jb[f"P{o}{ch}"] = make_pjob(o, ch, drain="scalar")
                return jb

            # ---- explicit per-head filler/assembly schedule ----
            # Keeps the PE densely busy every head (HAM K=8/8) while honoring
            # cross-engine dependencies with at least a one-head lead time.
            asm0 = make_assembly(0)
            asm1 = make_assembly(1)
            D = make_dummy_job
            head_order = {0: list(range(8)), 1: list(range(8))}
            slot = {}
            for h in range(4):
                slot[(0, h)] = [pe_jobs[0][h], pe_jobs[1][h]]
            slot[(0, 4)] = [asm0["s0"], pe_jobs[2][0], pe_jobs[2][1]]
            slot[(0, 5)] = [asm0["t0"], pe_jobs[2][2], pe_jobs[2][3]]
            slot[(0, 6)] = [asm0["t1"], pe_jobs[3][0], pe_jobs[3][1]]
            slot[(0, 7)] = [pe_jobs[3][2], pe_jobs[3][3], D()]
            slot[(1, 0)] = [asm0["s1"], D(), D()] if not trim else [asm0["s1"], D()]
            slot[(1, 1)] = [asm0["t2"], asm0["t3"], D()]
            pk = "P" if syt else "p"
            slot[(1, 2)] = [asm0[pk + "00"], asm0[pk + "01"], D()]
            slot[(1, 3)] = [asm0[pk + "10"], asm0[pk + "11"], D()]
            slot[(1, 4)] = [asm0[pk + "20"], asm0[pk + "21"], asm1["s0"], D()]
            slot[(1, 5)] = [asm0[pk + "30"], asm0[pk + "31"], asm1["t0"], D()]
            slot[(1, 6)] = [asm1["t1"], D(), D()] if not trim else [asm1["t1"], D()]
            slot[(1, 7)] = [D()]
            for h in range(8):
                ds = [D(extra_nmm) for _ in range(extra_d)]
                if dpre:
                    slot[(1, h)] = ds + slot[(1, h)]
                else:
                    slot[(1, h)] += ds
            tail = [asm1["s1"], D(), asm1["t2"], asm1["t3"], D(5)] + [
                asm1[f"P{o}{ch}"] for o in range(4) for ch in range(2)
            ]

            for half in range(2):
                hs2 = slice(HALF * half, HALF * (half + 1))
                c0 = slice(HALF * half, HALF * half + 512)
                c1 = slice(HALF * half + 512, HALF * half + 800)
                for hp, h in enumerate(head_order[half]):
                    t = h // 4
                    sr = 32 * (h % 4)
                    g = 8 * half + h
                    mm = mmp.tile([65, HALF], f32, name="mm", tag="mm")

                    def mm3(j, E):
                        mj = mt_sz(j)
                        lhsT = vT_g[j][0:mj, h, :]
                        nc.tensor.matmul(
                            mm[:, 0:512],
                            lhsT,
                            E[0:mj, 0:512],
                            start=(j == 0),
                            stop=(j == NMT - 1),
                        )
                        nc.tensor.matmul(
                            mm[:, 512:800],
                            lhsT,
                            E[0:mj, 512:800],
                            start=(j == 0),
                            stop=(j == NMT - 1),
                        )

                    pipe = []
                    for j in range(NMT):
                        mj = mt_sz(j)
                        ms = slice(128 * j, 128 * j + mj)
                        sc = scp.tile([128, HALF], f32, name="sc", tag="sc")
                        nc.tensor.matmul(
                            sc[0:mj, 0:512],
                            k_sb[t][sr : sr + 32, ms],
                            q_hi[t][sr : sr + 32, c0],
                            tile_position=(sr, 0),
                        )
                        if f32c1:
                            nc.tensor.matmul(
                                sc[0:mj, 512:800],
                                k32[t][sr : sr + 32, ms],
                                q32[t][sr : sr + 32, c1],
                                tile_position=(sr, 0),
                            )
                        else:
                            nc.tensor.matmul(
                                sc[0:mj, 512:800],
                                k_sb[t][sr : sr + 32, ms],
                                q_hi[t][sr : sr + 32, c1],
                                tile_position=(sr, 0),
                            )
                        E = ep.tile([128, HALF], bf16, name="E", tag="E")
                        nc.scalar.activation(E[0:mj, :], sc[0:mj, :], AF.Exp)
                        pipe.append((j, E))
                        if len(pipe) > 2:
                            mm3(*pipe.pop(0))
                    for it in pipe:
                        mm3(*it)
                    nc.vector.tensor_copy(zs[g][:], mm[0:64, :])
                    nc.vector.tensor_copy(s_g[h // 4][sr : sr + 1, hs2], mm[64:65, :])
                    for jobf in slot[(half, hp)]:
                        jobf()
            for jobf in tail:
                jobf()

            if dump:
                dbg_specs = [
                    ("q0", q_hi[0]),
                    ("k0", k_sb[0]),
                    ("v0", v_sb[0]),
                    ("vt0", vT_sb[0]),
                    ("pe0", pe_sb[0]),
                    ("zs0", zs[0]),
                    ("z160", z16[0]),
                ]
                for nm, t_sb in dbg_specs:
                    t_d = nc.dram_tensor(
                        f"dbg_{nm}", list(t_sb.shape), t_sb.dtype, kind="ExternalOutput"
                    ).ap()
                    nc.sync.dma_start(t_d[:], t_sb[:])

    nc.compile()
    return nc


def prep_weights(inputs):
    import ml_dtypes

    bfl = ml_dtypes.bfloat16
    d = lambda k: np.asarray(inputs[k], dtype=np.float64)
    inv = d("qkv_gamma") / np.sqrt(d("qkv_var") + EPS)
    W = d("qkv_w") * inv[:, None]
    bb = d("qkv_beta") - d("qkv_mean") * inv
    Wh = W.reshape(NH, 2 * KD + HD, DIM)
    bh = bb.reshape(NH, 2 * KD + HD)
    Wq = (Wh[:, :KD] * SCALE).reshape(NH * KD, DIM)
    bq = (bh[:, :KD] * SCALE).reshape(-1)
    Wk = Wh[:, KD : 2 * KD].reshape(NH * KD, DIM)
    bk = bh[:, KD : 2 * KD].reshape(-1)
    Wv = Wh[:, 2 * KD :].reshape(NH * HD, DIM)
    bv = bh[:, 2 * KD :].reshape(-1)

    ipe = d("pe_gamma") / np.sqrt(d("pe_var") + EPS)
    wpe = d("pe_w")[:, 0] * ipe[:, None, None]  # [512, 3, 3]
    bpe = d("pe_beta") - d("pe_mean") * ipe
    wpe_tap = np.zeros((128, 36), np.float64)
    for t in range(4):
        for k9 in range(9):
            wpe_tap[:, 9 * t + k9] = wpe[128 * t : 128 * (t + 1), k9 // 3, k9 % 3]
    pdg = np.zeros((36, 128, 128), np.float64)
    ar = np.arange(128)
    for t in range(4):
        for k9 in range(9):
            pdg[t * 9 + k9, ar, ar] = wpe[128 * t : 128 * (t + 1), k9 // 3, k9 % 3]

    ip = d("proj_gamma") / np.sqrt(d("proj_var") + EPS)
    Wp = d("proj_w") * ip[:, None]
    bp = d("proj_beta") - d("proj_mean") * ip

    c32 = lambda a: np.ascontiguousarray(a, dtype=np.float32)
    c16 = lambda a: np.ascontiguousarray(a.astype(np.float32), dtype=bfl)

    def grp(wT):
        # [512, m] -> [128, 4*m]: row p = concat over c of wT[128c+p, :]
        m = wT.shape[1]
        return wT.reshape(4, 128, m).transpose(1, 0, 2).reshape(128, 4 * m)

    return dict(
        wq16=c16(grp(Wq.T)),
        wk16=c16(grp(Wk.T)),
        wv16=c16(grp(Wv.T)),
        wp16=c16(grp(Wp.T)),
        bq=c32(bq.reshape(2, 128).T),
        bk=c32(bk.reshape(2, 128).T),
        bv=c32(bv.reshape(4, 128).T),
        bp=c32(bp.reshape(4, 128).T),
        wpe=c32(wpe_tap),
        bpe=c32(bpe.reshape(4, 128).T),
        ident=c16(np.eye(128)),
        ones=c16(np.ones((128, 64))),
        pdg=c16(pdg.transpose(1, 0, 2).reshape(128, 36 * 128)),
    )


def make_in_maps(inputs):
    import ml_dtypes

    w = prep_weights(inputs)
    x = np.asarray(inputs["x"], dtype=np.float32)
    B = x.shape[0]
    maps = []
    for i in range(B):
        xi = x[i].reshape(4, 128, NPOS).transpose(1, 0, 2).reshape(128, 4 * NPOS)
        maps.append({"x16": np.ascontiguousarray(xi).astype(ml_dtypes.bfloat16), **w})
    return maps


def kernel(**inputs):
    global _compiled_nc
    from concourse.bass_utils import run_bass_kernel_spmd

    if _compiled_nc is None:
        _compiled_nc = build_nc()
    in_maps = make_in_maps(inputs)
    res = run_bass_kernel_spmd(_compiled_nc, in_maps, core_ids=list(range(8)))
    y = np.stack(
        [
            np.asarray(res.results[i]["y"], dtype=np.float32).reshape(DIM, 40, 40)
            for i in range(8)
        ]
    )
    return y


if __name__ == "__main__":
    nc = build_nc()
    print("built ok")



# revision 23
# speedup vs baseline: 1.5368x; 1.0181x over previous
"""Trainium2 Bass kernel for nn_Attention_56822417326562 (dense transformer block).

Sharding: data-parallel over batch — core i computes batch element i entirely
(B=8 over 8 NeuronCores, no collectives).

Per-core math (x: [512, 1600]):
  BN folded into weights on host; softmax scale folded into q. All inputs are
  DMAed as channel-grouped [128, 4*m] tensors (one big-packet DMA each) on the
  sync queue, critical tensors first.
  Stage A (PE): q, k, v via 1x1 convs (bf16), then vT built by PE transposes
  of v (identity matmul) with a ones column appended per head for the softmax
  denominator. k is consumed via 32-row tile_position matmuls (no zero
  padding of the contraction dim), biases folded into the DVE PSUM drains.
  Attention (per half of n, per head): scores S^T[m,n] on PE (bf16), exp on
  ScalarE (its only work — the pacing engine), out_un[d,n] and s[n] in one PE
  accumulation via the vT ones column. The mm accumulator is drained by DVE
  copies (zs per head, s row into s_g); 1/s via full-tile DVE reciprocal at
  assembly time (single-partition reciprocal miscomputes on HW).
  pe = depthwise 3x3 as 9 diagonal bf16 matmuls over zero-padded v.
  Assembly (per half): 1/s broadcast via bf16 ones-matmul (tile_position row
  = head), z16 = out_un * (1/s) + pe (DVE), proj on PE, proj bias folded into
  the PSUM->SBUF drain, y DMAed out as bf16 per 400-col block.

HAM clock-gate management (the dominant perf effect): the PE clock sits at
1.2 GHz unless each free-running 3.4us activity window is ~fully busy; any
sparse window re-throttles to K=4/8 and halves PE speed for >=13.6us. The
per-head slot schedule interleaves contiguous ~1.7us PE bursts (the real
depthwise-conv jobs, the previous half's normalize/proj assembly jobs, and a
few discarded dummy bursts) between attention heads so the PE queue always
holds a multi-us backlog. Cross-engine dependencies get at least a one-head
lead so the in-order PE queue never stalls on DVE results. Separately, the
chip has a persistent P-state that can inflate all engines ~18% run to run;
only same-process paired A/B comparisons are meaningful.
"""
import sys

sys.path.insert(0, "/opt/trn_rl_repo")

import numpy as np

DIM = 512
NH = 8
HD = 64
KD = 32
NPOS = 1600
EPS = 1e-5
SCALE = float(KD) ** -0.5
NMT = 13  # position tiles: 12*128 + 64
HALF = 800

_compiled_nc = None


def build_nc(dump=False, f32c1=False, extra_d=0, extra_nmm=9, warmup=80, trim=True, gadd=True, gcp=False, syt=False, epb=6, dpre=False):
    import concourse.tile as tile
    from concourse import bacc, mybir

    f32 = mybir.dt.float32
    f32r = mybir.dt.float32r
    f16 = mybir.dt.float16
    bf16 = mybir.dt.bfloat16
    AF = mybir.ActivationFunctionType
    OP = mybir.AluOpType

    nc = bacc.Bacc("TRN2", target_bir_lowering=False, debug=False, num_devices=8)

    x16_d = nc.dram_tensor("x16", [128, 4 * NPOS], bf16, kind="ExternalInput").ap()
    wq16_d = nc.dram_tensor("wq16", [128, 4 * 256], bf16, kind="ExternalInput").ap()
    wk16_d = nc.dram_tensor("wk16", [128, 4 * 256], bf16, kind="ExternalInput").ap()
    wv16_d = nc.dram_tensor("wv16", [128, 4 * DIM], bf16, kind="ExternalInput").ap()
    wp16_d = nc.dram_tensor("wp16", [128, 4 * DIM], bf16, kind="ExternalInput").ap()
    bq_d = nc.dram_tensor("bq", [128, 2], f32, kind="ExternalInput").ap()
    bk_d = nc.dram_tensor("bk", [128, 2], f32, kind="ExternalInput").ap()
    bv_d = nc.dram_tensor("bv", [128, 4], f32, kind="ExternalInput").ap()
    bp_d = nc.dram_tensor("bp", [128, 4], f32, kind="ExternalInput").ap()
    wpe_d = nc.dram_tensor("wpe", [128, 36], f32, kind="ExternalInput").ap()
    bpe_d = nc.dram_tensor("bpe", [128, 4], f32, kind="ExternalInput").ap()
    ident_d = nc.dram_tensor("ident", [128, 128], bf16, kind="ExternalInput").ap()
    ones_d = nc.dram_tensor("ones", [128, 64], bf16, kind="ExternalInput").ap()
    pdg_d = nc.dram_tensor("pdg", [128, 36 * 128], bf16, kind="ExternalInput").ap()
    y_d = nc.dram_tensor("y", [DIM, NPOS], bf16, kind="ExternalOutput").ap()

    def mt_sz(j):
        return 64 if j == NMT - 1 else 128

    with tile.TileContext(nc) as tc:
        with (
            tc.tile_pool(name="pers", bufs=1) as pers,
            tc.tile_pool(name="ps2", bufs=2, space="PSUM") as ps2,
            tc.tile_pool(name="scp", bufs=2, space="PSUM") as scp,
            tc.tile_pool(name="mmp", bufs=1, space="PSUM") as mmp,
            tc.tile_pool(name="ep", bufs=epb) as ep,
            tc.tile_pool(name="ystg", bufs=4) as ystg,
        ):
            x16_all = pers.tile([128, 4 * NPOS], bf16, name="x16_all")
            wq_all = pers.tile([128, 4 * 256], bf16, name="wq_all")
            wk_all = pers.tile([128, 4 * 256], bf16, name="wk_all")
            wv_all = pers.tile([128, 4 * DIM], bf16, name="wv_all")
            wp_all = pers.tile([128, 4 * DIM], bf16, name="wp_all")
            x16_sb = [x16_all[:, NPOS * c : NPOS * (c + 1)] for c in range(4)]
            wq_sb = [wq_all[:, 256 * c : 256 * (c + 1)] for c in range(4)]
            wk_sb = [wk_all[:, 256 * c : 256 * (c + 1)] for c in range(4)]
            wv_sb = [wv_all[:, DIM * c : DIM * (c + 1)] for c in range(4)]
            wp_sb = [wp_all[:, DIM * c : DIM * (c + 1)] for c in range(4)]
            bq_sb = pers.tile([128, 2], f32, name="bq_sb")
            bk_sb = pers.tile([128, 2], f32, name="bk_sb")
            bv_sb = pers.tile([128, 4], f32, name="bv_sb")
            bp_sb = pers.tile([128, 4], f32, name="bp_sb")
            wpe_sb = pers.tile([128, 36], f32, name="wpe_sb")
            bpe_sb = pers.tile([128, 4], f32, name="bpe_sb")
            ident_sb = pers.tile([128, 128], bf16, name="ident_sb")
            warm_sb = pers.tile([128, 128], bf16, name="warm_sb")
            ones32 = pers.tile([128, 64], bf16, name="ones32")
            q_hi = [pers.tile([128, NPOS], bf16, name=f"qhi{t}") for t in range(2)]
            k_sb = [pers.tile([128, NPOS], bf16, name=f"k{t}") for t in range(2)]
            if f32c1:
                q32 = [pers.tile([128, NPOS], f32, name=f"q32{t}") for t in range(2)]
                k32 = [pers.tile([128, NPOS], f32, name=f"k32{t}") for t in range(2)]
            v_sb = [pers.tile([128, NPOS], bf16, name=f"v{o}") for o in range(4)]
            vT_sb = [pers.tile([128, NH * 65], bf16, name=f"vT{j}") for j in range(NMT)]
            pe_sb = [pers.tile([128, NPOS], bf16, name=f"pe{t}") for t in range(4)]
            vpad = [pers.tile([128, 42 * 42], bf16, name=f"vpad{t}") for t in range(4)]
            pdg_all = pers.tile([128, 36 * 128], bf16, name="pdg_all")
            pdg_sb = [pdg_all[:, 128 * i : 128 * (i + 1)] for i in range(36)]
            z16 = [pers.tile([128, NPOS], bf16, name=f"z16{t}") for t in range(4)]
            zs = [pers.tile([64, HALF], bf16, name=f"zs{g}") for g in range(16)]
            s_g = [pers.tile([128, NPOS], f32, name=f"s_g{i}") for i in range(2)]
            rbf = [pers.tile([128, NPOS], bf16, name=f"rbf{i}") for i in range(2)]

            # ---- input DMAs: consolidated channel-grouped tensors,
            # x split across two hw queues; critical tensors first ----
            # critical path: q conv needs x (both halves) + wq + wk; balance
            # ~1.1MB per hw queue so they land together, ident/wv later
            nc.sync.dma_start(x16_all[:, 0 : 2 * NPOS], x16_d[:, 0 : 2 * NPOS])
            nc.scalar.dma_start(
                x16_all[:, 2 * NPOS : 4 * NPOS], x16_d[:, 2 * NPOS : 4 * NPOS]
            )
            nc.sync.dma_start(wq_all[:], wq16_d[:])
            nc.scalar.dma_start(wk_all[:], wk16_d[:])
            nc.scalar.dma_start(bq_sb[:], bq_d[:])
            nc.scalar.dma_start(bk_sb[:], bk_d[:])
            nc.scalar.dma_start(bv_sb[:], bv_d[:])
            nc.sync.dma_start(ident_sb[:], ident_d[:])
            nc.sync.dma_start(wv_all[:], wv16_d[:])
            nc.sync.dma_start(ones32[:], ones_d[:])
            nc.sync.dma_start(wpe_sb[:], wpe_d[:])
            nc.sync.dma_start(bpe_sb[:], bpe_d[:])
            nc.sync.dma_start(wp_all[:], wp16_d[:])
            nc.sync.dma_start(bp_sb[:], bp_d[:])
            nc.sync.dma_start(pdg_all[:], pdg_d[:])

            nc.gpsimd.memset(warm_sb[:], 0.5)
            for i in range(2):
                nc.gpsimd.memset(s_g[i][:], 1.0)
            for t in range(4):
                vg = vpad[t].rearrange("p (a b) -> p a b", a=42)
                nc.gpsimd.memset(vg[:, 0:1, :], 0.0)
                nc.gpsimd.memset(vg[:, 41:42, :], 0.0)
                nc.gpsimd.memset(vg[:, 1:41, 0:1], 0.0)
                nc.gpsimd.memset(vg[:, 1:41, 41:42], 0.0)
            vT_g = [vT_sb[j].rearrange("p (h g) -> p h g", g=65) for j in range(NMT)]
            for j in range(NMT):
                nc.gpsimd.memset(vT_g[j][0 : mt_sz(j), :, 64:65], 1.0)

            # ---- HAM warm-up: occupy the PE on the identity tile during
            # the input-DMA window so stage A starts at full clock ----
            if warmup:
                wps = ps2.tile([128, 512], f32, name="wup", tag="ps2")
                for i in range(warmup):
                    nc.tensor.matmul(
                        wps[:, 0:128],
                        warm_sb[:],
                        warm_sb[:],
                        start=(i == 0),
                        stop=(i == warmup - 1),
                    )

            # ---- stage A: q, k (bf16, bias via DVE drain) ----
            for w_sb, b_sb, dst in ((wq_sb, bq_sb, q_hi), (wk_sb, bk_sb, k_sb)):
                for t in range(2):
                    for ch in range(4):
                        cs = slice(400 * ch, 400 * (ch + 1))
                        ps = ps2.tile([128, 512], f32, name="psqk", tag="ps2")
                        for c in range(4):
                            nc.tensor.matmul(
                                ps[:, 0:400],
                                w_sb[c][:, 128 * t : 128 * (t + 1)],
                                x16_sb[c][:, cs],
                                start=(c == 0),
                                stop=(c == 3),
                            )
                        nc.vector.tensor_scalar_add(
                            dst[t][:, cs], ps[:, 0:400], b_sb[:, t : t + 1]
                        )
                        if f32c1:
                            dst32 = q32 if dst is q_hi else k32
                            nc.vector.tensor_scalar_add(
                                dst32[t][:, cs], ps[:, 0:400], b_sb[:, t : t + 1]
                            )

            # ---- stage A: v natural, then vT via PE transpose ----
            for o in range(4):
                for ch in range(4):
                    cs = slice(400 * ch, 400 * (ch + 1))
                    ps = ps2.tile([128, 512], f32, name="psv", tag="ps2")
                    for c in range(4):
                        nc.tensor.matmul(
                            ps[:, 0:400],
                            wv_sb[c][:, 128 * o : 128 * (o + 1)],
                            x16_sb[c][:, cs],
                            start=(c == 0),
                            stop=(c == 3),
                        )
                    nc.vector.tensor_scalar_add(
                        v_sb[o][:, cs], ps[:, 0:400], bv_sb[:, o : o + 1]
                    )
                    nc.gpsimd.tensor_copy(
                        vpad[o].rearrange("p (a b) -> p a b", a=42)[
                            :, 1 + 10 * ch : 11 + 10 * ch, 1:41
                        ],
                        v_sb[o][:, cs].rearrange("p (a b) -> p a b", a=10),
                    )

            for j in range(NMT):
                mj = mt_sz(j)
                psT = ps2.tile([128, 512], bf16, name="psT", tag="ps2")
                for t in range(4):
                    nc.tensor.transpose(
                        psT[0:mj, 128 * t : 128 * (t + 1)],
                        v_sb[t][:, 128 * j : 128 * j + mj],
                        ident_sb[:],
                    )
                nc.vector.tensor_copy(
                    vT_g[j][0:mj, :, 0:64],
                    psT[0:mj, :].rearrange("p (h d) -> p h d", d=64),
                )

            # ---- pe: depthwise 3x3 as PE diagonal matmuls. Each (t, ch) job
            # is a contiguous ~1.7us PE burst, scheduled as HAM-warmth filler
            # between attention heads ----
            def make_pe_job(t, ch):
                def pejob():
                    vg = vpad[t].rearrange("p (a b) -> p a b", a=42)
                    ps = ps2.tile([128, 512], f32, name="pspe", tag="ps2")
                    for k9 in range(9):
                        dy, dx = k9 // 3 - 1, k9 % 3 - 1
                        rhs = vg[
                            :, 1 + 10 * ch + dy : 11 + 10 * ch + dy, 1 + dx : 41 + dx
                        ]
                        nc.tensor.matmul(
                            ps[:, 0:400],
                            pdg_sb[9 * t + k9][:],
                            rhs,
                            start=(k9 == 0),
                            stop=(k9 == 8),
                        )
                    nc.vector.tensor_scalar_add(
                        pe_sb[t][:, 400 * ch : 400 * (ch + 1)],
                        ps[:, 0:400],
                        bpe_sb[:, t : t + 1],
                    )

                return pejob

            pe_jobs = {t: [make_pe_job(t, ch) for ch in range(4)] for t in range(4)}

            pe_scr = pers.tile([128, 400], f16, name="pe_scr")

            def make_dummy_job(nmm=9):
                def djob():
                    vg = vpad[0].rearrange("p (a b) -> p a b", a=42)
                    ps = ps2.tile([128, 512], f32, name="psdm", tag="ps2")
                    for k9 in range(nmm):
                        dy, dx = k9 % 3 - 1, k9 // 3 - 1
                        rhs = vg[:, 1 + dy : 11 + dy, 1 + dx : 41 + dx]
                        nc.tensor.matmul(
                            ps[:, 0:400],
                            pdg_sb[k9][:],
                            rhs,
                            start=(k9 == 0),
                            stop=(k9 == nmm - 1),
                        )
                    nc.vector.tensor_copy(pe_scr[:], ps[:, 0:400])

                return djob

            # ---- assembly maker (per half): normalize, +pe, proj, out ----
            def make_assembly(half):
                hs = slice(HALF * half, HALF * (half + 1))
                jobs = []

                def make_sjob(i):
                    def sjob():
                        nc.vector.reciprocal_approx_fast(
                            s_g[i][:, hs], s_g[i][:, hs]
                        )
                        if gcp:
                            nc.gpsimd.tensor_copy(rbf[i][:, hs], s_g[i][:, hs])
                        else:
                            nc.vector.tensor_copy(rbf[i][:, hs], s_g[i][:, hs])

                    return sjob

                def make_tjob(t):
                    def tjob():
                        for i in range(2):
                            h2 = 2 * t + i
                            g = 8 * half + h2
                            sr2 = 32 * (h2 % 4)
                            for off, ncols in ((0, 512), (512, 288)):
                                rb = ps2.tile([128, 512], f32, name="rb", tag="ps2")
                                nc.tensor.matmul(
                                    rb[0:64, 0:ncols],
                                    ones32[sr2 : sr2 + 1, 0:64],
                                    rbf[h2 // 4][
                                        sr2 : sr2 + 1,
                                        HALF * half + off : HALF * half + off + ncols,
                                    ],
                                    tile_position=(sr2, 0),
                                )
                                nc.vector.tensor_tensor(
                                    z16[t][
                                        64 * i : 64 * (i + 1),
                                        HALF * half + off : HALF * half + off + ncols,
                                    ],
                                    zs[g][0:64, off : off + ncols],
                                    rb[0:64, 0:ncols],
                                    op=OP.mult,
                                )
                        if gadd:
                            nc.gpsimd.tensor_tensor(
                                z16[t][:, hs], z16[t][:, hs], pe_sb[t][:, hs],
                                op=OP.add,
                            )
                        else:
                            nc.vector.tensor_tensor(
                                z16[t][:, hs], z16[t][:, hs], pe_sb[t][:, hs],
                                op=OP.add,
                            )

                    return tjob

                def make_pjob(o, ch, drain=None):
                    def pjob():
                        cs = slice(
                            HALF * half + 400 * ch, HALF * half + 400 * (ch + 1)
                        )
                        pj = ps2.tile([128, 512], f32, name="pj", tag="ps2")
                        for c in range(4):
                            nc.tensor.matmul(
                                pj[:, 0:400],
                                wp_sb[c][:, 128 * o : 128 * (o + 1)],
                                z16[c][:, cs],
                                start=(c == 0),
                                stop=(c == 3),
                            )
                        yt = ystg.tile([128, 400], bf16, name="yt", tag="yt")
                        if drain == "scalar":
                            nc.scalar.activation(
                                yt[:], pj[:, 0:400], AF.Identity,
                                bias=bp_sb[:, o : o + 1],
                            )
                        else:
                            nc.vector.tensor_scalar_add(
                                yt[:], pj[:, 0:400], bp_sb[:, o : o + 1]
                            )
                        nc.sync.dma_start(y_d[128 * o : 128 * (o + 1), cs], yt[:])

                    return pjob

                jb = {}
                jb["s0"] = make_sjob(0)
                jb["s1"] = make_sjob(1)
                for t in range(4):
                    jb[f"t{t}"] = make_tjob(t)
                for o in range(4):
                    for ch in range(2):
                        jb[f"p{o}{ch}"] = make_pjob(o, ch)
                        jb[f"P{o}{ch}"] = make_pjob(o, ch, drain="scalar")
                return jb

            # ---- explicit per-head filler/assembly schedule ----
            # Keeps the PE densely busy every head (HAM K=8/8) while honoring
            # cross-engine dependencies with at least a one-head lead time.
            asm0 = make_assembly(0)
            asm1 = make_assembly(1)
            D = make_dummy_job
            head_order = {0: list(range(8)), 1: list(range(8))}
            slot = {}
            for h in range(4):
                slot[(0, h)] = [pe_jobs[0][h], pe_jobs[1][h]]
            slot[(0, 4)] = [asm0["s0"], pe_jobs[2][0], pe_jobs[2][1]]
            slot[(0, 5)] = [asm0["t0"], pe_jobs[2][2], pe_jobs[2][3]]
            slot[(0, 6)] = [asm0["t1"], pe_jobs[3][0], pe_jobs[3][1]]
            slot[(0, 7)] = [pe_jobs[3][2], pe_jobs[3][3], D()]
            slot[(1, 0)] = [asm0["s1"], D(), D()] if not trim else [asm0["s1"], D()]
            slot[(1, 1)] = [asm0["t2"], asm0["t3"], D()]
            pk = "P" if syt else "p"
            slot[(1, 2)] = [asm0[pk + "00"], asm0[pk + "01"], D()]
            slot[(1, 3)] = [asm0[pk + "10"], asm0[pk + "11"], D()]
            slot[(1, 4)] = [asm0[pk + "20"], asm0[pk + "21"], asm1["s0"], D()]
            slot[(1, 5)] = [asm0[pk + "30"], asm0[pk + "31"], asm1["t0"], D()]
            slot[(1, 6)] = [asm1["t1"], D(), D()] if not trim else [asm1["t1"], D()]
            slot[(1, 7)] = [D()]
            for h in range(8):
                ds = [D(extra_nmm) for _ in range(extra_d)]
                if dpre:
                    slot[(1, h)] = ds + slot[(1, h)]
                else:
                    slot[(1, h)] += ds
            tail = [asm1["s1"], D(), asm1["t2"], asm1["t3"], D(5)] + [
                asm1[f"P{o}{ch}"] for o in range(4) for ch in range(2)
            ]

            for half in range(2):
                hs2 = slice(HALF * half, HALF * (half + 1))
                c0 = slice(HALF * half, HALF * half + 512)
                c1 = slice(HALF * half + 512, HALF * half + 800)
                for hp, h in enumerate(head_order[half]):
                    t = h // 4
                    sr = 32 * (h % 4)
                    g = 8 * half + h
                    mm = mmp.tile([65, HALF], f32, name="mm", tag="mm")

                    def mm3(j, E):
                        mj = mt_sz(j)
                        lhsT = vT_g[j][0:mj, h, :]
                        nc.tensor.matmul(
                            mm[:, 0:512],
                            lhsT,
                            E[0:mj, 0:512],
                            start=(j == 0),
                            stop=(j == NMT - 1),
                        )
                        nc.tensor.matmul(
                            mm[:, 512:800],
                            lhsT,
                            E[0:mj, 512:800],
                            start=(j == 0),
                            stop=(j == NMT - 1),
                        )

                    pipe = []
                    for j in range(NMT):
                        mj = mt_sz(j)
                        ms = slice(128 * j, 128 * j + mj)
                        sc = scp.tile([128, HALF], f32, name="sc", tag="sc")
                        nc.tensor.matmul(
                            sc[0:mj, 0:512],
                            k_sb[t][sr : sr + 32, ms],
                            q_hi[t][sr : sr + 32, c0],
                            tile_position=(sr, 0),
                        )
                        if f32c1:
                            nc.tensor.matmul(
                                sc[0:mj, 512:800],
                                k32[t][sr : sr + 32, ms],
                                q32[t][sr : sr + 32, c1],
                                tile_position=(sr, 0),
                            )
                        else:
                            nc.tensor.matmul(
                                sc[0:mj, 512:800],
                                k_sb[t][sr : sr + 32, ms],
                                q_hi[t][sr : sr + 32, c1],
                                tile_position=(sr, 0),
                            )
                        E = ep.tile([128, HALF], bf16, name="E", tag="E")
                        nc.scalar.activation(E[0:mj, :], sc[0:mj, :], AF.Exp)
                        pipe.append((j, E))
                        if len(pipe) > 2:
                            mm3(*pipe.pop(0))
                    for it in pipe:
                        mm3(*it)
                    nc.vector.tensor_copy(zs[g][:], mm[0:64, :])
                    nc.vector.tensor_copy(s_g[h // 4][sr : sr + 1, hs2], mm[64:65, :])
                    for jobf in slot[(half, hp)]:
                        jobf()
            for jobf in tail:
                jobf()

            if dump:
                dbg_specs = [
                    ("q0", q_hi[0]),
                    ("k0", k_sb[0]),
                    ("v0", v_sb[0]),
                    ("vt0", vT_sb[0]),
                    ("pe0", pe_sb[0]),
                    ("zs0", zs[0]),
                    ("z160", z16[0]),
                ]
                for nm, t_sb in dbg_specs:
                    t_d = nc.dram_tensor(
                        f"dbg_{nm}", list(t_sb.shape), t_sb.dtype, kind="ExternalOutput"
                    ).ap()
                    nc.sync.dma_start(t_d[:], t_sb[:])

    nc.compile()
    return nc


def prep_weights(inputs):
    import ml_dtypes

    bfl = ml_dtypes.bfloat16
    d = lambda k: np.asarray(inputs[k], dtype=np.float64)
    inv = d("qkv_gamma") / np.sqrt(d("qkv_var") + EPS)
    W = d("qkv_w") * inv[:, None]
    bb = d("qkv_beta") - d("qkv_mean") * inv
    Wh = W.reshape(NH, 2 * KD + HD, DIM)
    bh = bb.reshape(NH, 2 * KD + HD)
    Wq = (Wh[:, :KD] * SCALE).reshape(NH * KD, DIM)
    bq = (bh[:, :KD] * SCALE).reshape(-1)
    Wk = Wh[:, KD : 2 * KD].reshape(NH * KD, DIM)
    bk = bh[:, KD : 2 * KD].reshape(-1)
    Wv = Wh[:, 2 * KD :].reshape(NH * HD, DIM)
    bv = bh[:, 2 * KD :].reshape(-1)

    ipe = d("pe_gamma") / np.sqrt(d("pe_var") + EPS)
    wpe = d("pe_w")[:, 0] * ipe[:, None, None]  # [512, 3, 3]
    bpe = d("pe_beta") - d("pe_mean") * ipe
    wpe_tap = np.zeros((128, 36), np.float64)
    for t in range(4):
        for k9 in range(9):
            wpe_tap[:, 9 * t + k9] = wpe[128 * t : 128 * (t + 1), k9 // 3, k9 % 3]
    pdg = np.zeros((36, 128, 128), np.float64)
    ar = np.arange(128)
    for t in range(4):
        for k9 in range(9):
            pdg[t * 9 + k9, ar, ar] = wpe[128 * t : 128 * (t + 1), k9 // 3, k9 % 3]

    ip = d("proj_gamma") / np.sqrt(d("proj_var") + EPS)
    Wp = d("proj_w") * ip[:, None]
    bp = d("proj_beta") - d("proj_mean") * ip

    c32 = lambda a: np.ascontiguousarray(a, dtype=np.float32)
    c16 = lambda a: np.ascontiguousarray(a.astype(np.float32), dtype=bfl)

    def grp(wT):
        # [512, m] -> [128, 4*m]: row p = concat over c of wT[128c+p, :]
        m = wT.shape[1]
        return wT.reshape(4, 128, m).transpose(1, 0, 2).reshape(128, 4 * m)

    return dict(
        wq16=c16(grp(Wq.T)),
        wk16=c16(grp(Wk.T)),
        wv16=c16(grp(Wv.T)),
        wp16=c16(grp(Wp.T)),
        bq=c32(bq.reshape(2, 128).T),
        bk=c32(bk.reshape(2, 128).T),
        bv=c32(bv.reshape(4, 128).T),
        bp=c32(bp.reshape(4, 128).T),
        wpe=c32(wpe_tap),
        bpe=c32(bpe.reshape(4, 128).T),
        ident=c16(np.eye(128)),
        ones=c16(np.ones((128, 64))),
        pdg=c16(pdg.transpose(1, 0, 2).reshape(128, 36 * 128)),
    )


def make_in_maps(inputs):
    import ml_dtypes

    w = prep_weights(inputs)
    x = np.asarray(inputs["x"], dtype=np.float32)
    B = x.shape[0]
    maps = []
    for i in range(B):
        xi = x[i].reshape(4, 128, NPOS).transpose(1, 0, 2).reshape(128, 4 * NPOS)
        maps.append({"x16": np.ascontiguousarray(xi).astype(ml_dtypes.bfloat16), **w})
    return maps


def kernel(**inputs):
    global _compiled_nc
    from concourse.bass_utils import run_bass_kernel_spmd

    if _compiled_nc is None:
        _compiled_nc = build_nc()
    in_maps = make_in_maps(inputs)
    res = run_bass_kernel_spmd(_compiled_nc, in_maps, core_ids=list(range(8)))
    y = np.stack(
        [
            np.asarray(res.results[i]["y"], dtype=np.float32).reshape(DIM, 40, 40)
            for i in range(8)
        ]
    )
    return y


if __name__ == "__main__":
    nc = build_nc()
    print("built ok")

